# revision 1
# baseline (speedup 1.0000x reference)
"""Trainium2 Bass kernel for Ernie4.5-VL vision attention (ragged segments).

Contract: kernel(**inputs) takes the FULL unsharded inputs (keyed as in
setup_inputs()) and returns the FULL [S, D] float32 output.

Strategy
--------
All matmuls run on the PE array in float32r (full-rate fp32, ~1.5e-4 rel
err); everything else is fp32. Attention is computed per segment
(block-diagonal, no masks) in a flash-like streaming form that only ever
materializes transposed score tiles:

  qkvT = Wpack @ hidden.T          (dims on partitions, tokens on free)
  RoPE on qT/kT slices (DVE/GPSIMD elementwise)
  v_aug = transpose(vT) with a ones column appended   (PE transposes)
  per (head, segment, 1024-query chunk):
     for each 128-key tile: ST = kT-tile.T @ qT-chunk  (scores^T in PSUM)
                            PT = exp(ST)               (ACT, no max-sub)
                            outT_aug += v_aug.T @ PT   (PSUM accumulate)
     attn_outT = outT_aug[:80] * (1 / outT_aug[80])    (denominator row)
  projT_partial = WprojT_shard.T @ attn_outT           (per-core partial)

Sharding (8 cores, SPMD - one program, per-core data):
  - uniform 4x1024 segments: 2 head-groups x 4 segments (24 MB/core DMA)
  - any other cu_seqlens: 8-way head parallel, every core sees all
    segments (identical program regardless of segment raggedness)
Host does only O(S*D) glue: input transposes/packing, summing the 2 (or
8) per-token partial projections, and the bias adds.
"""

import os
import sys

import numpy as np

H = 16
HD = 80
BLK = 40  # rotate_half half-width
SCALE = HD ** -0.5
N_CORES = 8
D = 1280
NK = D // 128  # contraction tiles for the qkv matmul
ATTN_STRIDE = 96  # head row pitch in the packed attention output
MM_DT_NAME = os.environ.get("KERNEL_MM_DT", "float32r")  # or "float32"
KERNEL_DEBUG = bool(int(os.environ.get("KERNEL_DEBUG", "0")))


def _segments(cu_seqlens, S):
    """Intervals matching reference's searchsorted(cu[1:], i, 'right')."""
    b = np.clip(np.sort(np.asarray(cu_seqlens, dtype=np.int64)[1:5]), 0, S)
    bounds = [0] + list(b) + [S]
    segs = []
    for a, e in zip(bounds[:-1], bounds[1:]):
        if e > a:
            segs.append((int(a), int(e)))
    return segs


def _pack_layout(n_h):
    """Pack per-core qkv dims as 40-row blocks, 3 per 128-row tile (8 pad).

    Each tile holds one v-block at row 0 (PE transpose operands must start
    at a 32-aligned partition) and two q/k blocks at rows 40 and 80.
    Returns pos[(sec, h, half)] = (tile, row) and the number of tiles.
    """
    ntiles = 2 * n_h
    pos = {}
    for h in range(n_h):
        for half in (0, 1):
            pos[("v", h, half)] = (2 * h + half, 0)
    qk = [("q", h, half) for h in range(n_h) for half in (0, 1)]
    qk += [("k", h, half) for h in range(n_h) for half in (0, 1)]
    for j, blk in enumerate(qk):
        pos[blk] = (j // 2, BLK + BLK * (j % 2))
    return pos, ntiles


def _pieces(start, length, tile_rows=128):
    """Split global row range [start, start+length) into per-tile pieces."""
    out = []
    off = 0
    while off < length:
        g = start + off
        t, r = g // tile_rows, g % tile_rows
        n = min(tile_rows - r, length - off)
        out.append((t, r, n, off))
        off += n
    return out


def _proj_k_tiles(n_h):
    rows = ATTN_STRIDE * n_h
    kt = [128] * (rows // 128)
    if rows % 128:
        kt.append(rows % 128)
    return kt


def _build_program(n_h, S_core, segs_local, resident_hidden):
    """Emit the SPMD program. Same structure for every core.

    Engine-AP partition rules on TRN2 (walrus birverifier): compute-engine
    accesses must start at a 32-aligned partition and must not cross a
    64-boundary unless they start on one; cross-partition data movement
    must go through DMA. The layout choices below all follow from this.
    """
    import concourse.mybir as mybir
    import concourse.tile as tile
    from concourse import bacc
    from concourse.masks import make_identity
    from contextlib import ExitStack

    f32 = mybir.dt.float32
    mm_dt = getattr(mybir.dt, MM_DT_NAME)
    AF = mybir.ActivationFunctionType

    k_proj = n_h
    pos, n_mtiles = _pack_layout(n_h)
    dims_pad = n_mtiles * 128
    VW = 97  # v_aug slot width: 80 v dims + 16 zero pad + ones col at 96

    # global key-tile list: (seg_idx, t0, t1)
    t_tiles = []
    for si, (a, e) in enumerate(segs_local):
        t = a
        while t < e:
            t_tiles.append((si, t, min(t + 128, e)))
            t += 128
    n_tt = len(t_tiles)

    nc = bacc.Bacc("TRN2", target_bir_lowering=False, debug=False,
                   enable_asserts=False, num_devices=N_CORES)

    # host supplies hiddenT/wqkvT pre-tiled into 128-partition-major layout
    hiddenT = nc.dram_tensor("hiddenT", [128, NK * S_core], mm_dt,
                             kind="ExternalInput").ap()
    wqkvT = nc.dram_tensor("wqkvT", [128, NK * dims_pad], mm_dt,
                           kind="ExternalInput").ap()
    bias2d = nc.dram_tensor("bias2d", [128, n_mtiles], f32,
                            kind="ExternalInput").ap()
    # cosP/sin2P are host-packed [128, S]: rows 0:40 and 64:104 hold the
    # lo/hi rope coefficients, all other rows zero (zeroes the junk rows
    # of the rotated q/k so the K=104 score matmuls see exact zeros).
    cosP = nc.dram_tensor("cosP", [128, S_core], mm_dt,
                          kind="ExternalInput").ap()
    sin2P = nc.dram_tensor("sin2P", [128, S_core], mm_dt,
                           kind="ExternalInput").ap()
    wprojT = nc.dram_tensor("wprojT", [n_h * HD, D], mm_dt,
                            kind="ExternalInput").ap()
    # per-key-tile v_aug tail init: 16 zero pad cols + ones col (f32r memset
    # fails walrus codegen, so this comes in via DMA)
    vinit = nc.dram_tensor("vinit", [128, n_tt * (VW - HD)], mm_dt,
                           kind="ExternalInput").ap()
    outT = nc.dram_tensor("outT", [D, S_core], f32, kind="ExternalOutput").ap()
    if KERNEL_DEBUG:
        dbg_qkv = nc.dram_tensor("dbg_qkv", [128, n_mtiles * S_core], f32,
                                 kind="ExternalOutput").ap()
        dbg_rot = nc.dram_tensor("dbg_rot", [128, 2 * n_h * S_core], f32,
                                 kind="ExternalOutput").ap()
        dbg_vaug = nc.dram_tensor("dbg_vaug", [128, n_h * n_tt * VW], f32,
                                  kind="ExternalOutput").ap()
        dbg_attn = nc.dram_tensor("dbg_attn", [128, n_h * S_core], f32,
                                  kind="ExternalOutput").ap()

    def r_(ap):
        return ap.bitcast(mm_dt)

    BC = 1024  # psum tile width (2 banks); matmuls stream <=512
    big_chunks = [(c, min(c + BC, S_core)) for c in range(0, S_core, BC)]

    def halves(c0, c1):
        out = []
        q = c0
        while q < c1:
            out.append((q, min(q + 512, c1)))
            q = q + 512
        return out

    with tile.TileContext(nc) as tc, ExitStack() as ctx:
        persist = ctx.enter_context(tc.tile_pool(name="persist", bufs=1))
        ident = persist.tile([128, 128], f32, tag="ident", name="ident")
        make_identity(nc, ident[:])
        bias_sb = persist.tile([128, n_mtiles], f32, tag="bias", name="bias")
        nc.sync.dma_start(bias_sb[:], bias2d[:])

        # PSUM: two 2-bank slots (t0/t1) shared by qkv/scores/proj, two
        # 1-bank slots for v-transposes, one 2-bank slot for PV accumulate
        psum_all_cm = tc.tile_pool(name="psum_all", bufs=1, space="PSUM")
        psum_all = psum_all_cm.__enter__()
        # big pool: qkvT tiles (phases 1-3), slots reused by attn (phases 4-5)
        qkv_pool = ctx.enter_context(tc.tile_pool(name="big", bufs=1))
        qkv_sb = [qkv_pool.tile([128, S_core], mm_dt, tag=f"qkvT{j}",
                                name=f"qkvT{j}") for j in range(n_mtiles)]
        # rope output (rows 0:104 live, 40:64 zeroed via cosP/sin2P pads)
        rot_cm = tc.tile_pool(name="rot", bufs=1)
        rv = rot_cm.__enter__()
        rot_sb = {}
        for h in range(n_h):
            for sec in ("q", "k"):
                rot_sb[(sec, h)] = rv.tile([128, S_core], mm_dt,
                                           tag=f"rot_{sec}{h}",
                                           name=f"rot_{sec}{h}")
        RC = 1024
        rope_cm = tc.tile_pool(name="rope_scr", bufs=2)
        rope_scr = rope_cm.__enter__()

        # ------------ phase 1: qkvT = Wpack @ hidden.T --------------
        with ExitStack() as p1:
            hidden3 = hiddenT.rearrange("p (k s) -> p k s", k=NK)
            w3 = wqkvT.rearrange("p (k m) -> p k m", k=NK)
            if resident_hidden:
                hid_pool = p1.enter_context(tc.tile_pool(name="hid", bufs=1))
                w_pool = p1.enter_context(tc.tile_pool(name="wstream", bufs=3))
                hid_sb = [hid_pool.tile([128, S_core], mm_dt, tag=f"hid{k}",
                                        name=f"hid{k}") for k in range(NK)]
                wj0 = w_pool.tile([128, NK * 128], mm_dt, tag="wj", name="wj")
                nc.sync.dma_start(hid_sb[0][:], hidden3[:, 0, :])
                nc.sync.dma_start(
                    wj0.rearrange("p (k m) -> p k m", k=NK)[:, :, :],
                    w3[:, :, 0:128])
                for k in range(1, NK):
                    nc.sync.dma_start(hid_sb[k][:], hidden3[:, k, :])
                for j in range(n_mtiles):
                    if j == 0:
                        wj = wj0
                    else:
                        wj = w_pool.tile([128, NK * 128], mm_dt, tag="wj",
                                         name="wj")
                        nc.sync.dma_start(
                            wj.rearrange("p (k m) -> p k m", k=NK)[:, :, :],
                            w3[:, :, j * 128:(j + 1) * 128])
                    for (h0, h1) in halves(0, S_core):
                        hw = h1 - h0
                        ps = psum_all.tile([128, 512], f32,
                                           tag=f"t{(h0 // 512) % 2}",
                                           name="qkvp")
                        for k in range(NK):
                            nc.tensor.matmul(
                                ps[:, :hw],
                                r_(wj[:, k * 128:(k + 1) * 128]),
                                r_(hid_sb[k][:, h0:h1]),
                                start=(k == 0), stop=(k == NK - 1))
                        nc.scalar.activation(qkv_sb[j][:, h0:h1], ps[:, :hw],
                                             AF.Identity,
                                             bias=bias_sb[:, j:j + 1])
            else:
                # k-outer streaming: two psum slots hold four j-streams
                # (columns 0:512 and 512:1024), hidden tiles are tiny
                w_pool = p1.enter_context(tc.tile_pool(name="wres", bufs=1))
                w_sb = [w_pool.tile([128, dims_pad], mm_dt, tag=f"w{k}",
                                    name=f"w{k}") for k in range(NK)]
                for k in range(NK):
                    nc.sync.dma_start(w_sb[k][:], w3[:, k, :])
                assert n_mtiles == 4
                hid_pool = p1.enter_context(tc.tile_pool(name="hidstream",
                                                         bufs=3))
                for (h0, h1) in halves(0, S_core):
                    hw = h1 - h0
                    ps01 = psum_all.tile([128, BC], f32, tag="t0", name="ps01")
                    ps23 = psum_all.tile([128, BC], f32, tag="t1", name="ps23")
                    pj_of = lambda j: (ps01 if j < 2 else ps23,
                                       (j % 2) * 512)
                    for k in range(NK):
                        ht = hid_pool.tile([128, 512], mm_dt, tag="hidc",
                                           name="hidc")
                        nc.sync.dma_start(ht[:, :hw], hidden3[:, k, h0:h1])
                        for j in range(n_mtiles):
                            psj, co = pj_of(j)
                            nc.tensor.matmul(
                                psj[:, co:co + hw],
                                r_(w_sb[k][:, j * 128:(j + 1) * 128]),
                                r_(ht[:, :hw]),
                                start=(k == 0), stop=(k == NK - 1))
                    for j in range(n_mtiles):
                        psj, co = pj_of(j)
                        nc.scalar.activation(qkv_sb[j][:, h0:h1],
                                             psj[:, co:co + hw], AF.Identity,
                                             bias=bias_sb[:, j:j + 1])

        psum_all_cm.__exit__(None, None, None)
        ps_att = ctx.enter_context(tc.tile_pool(name="ps_att", bufs=1,
                                                space="PSUM"))

        # ------------ phase 2: RoPE --------------------------------
        # DMA-stage lo/hi into 0:40 / 64:104 (stgA) and swapped (stgB),
        # then rot = stgA*cosP + stgB*sin2P as three same-base wide ops.
        # double-buffered persistent staging tensors; rows 40:64 zeroed once
        # from cosP's zero rows so the [0:104) products read defined zeros
        stg = {}
        for nm in ("sa0", "sa1", "sb0", "sb1"):
            stg[nm] = rope_scr.tile([128, RC], mm_dt, tag=nm, name=nm, bufs=1)
        pair_i = 0
        for ci, f0 in enumerate(range(0, S_core, RC)):
            f1 = min(f0 + RC, S_core)
            fs = f1 - f0
            cos_sb = rope_scr.tile([128, RC], mm_dt, tag="cos", name="cos",
                                   bufs=1)
            sin_sb = rope_scr.tile([128, RC], mm_dt, tag="sin", name="sin",
                                   bufs=1)
            nc.scalar.dma_start(cos_sb[:, :fs], cosP[:, f0:f1])
            nc.scalar.dma_start(sin_sb[:, :fs], sin2P[:, f0:f1])
            if ci == 0:
                for nm in stg:
                    nc.scalar.dma_start(stg[nm][BLK:64, :], cos_sb[BLK:64, :])
            for h in range(n_h):
                for sec in ("q", "k"):
                    lo_t, lo_r = pos[(sec, h, 0)]
                    hi_t, hi_r = pos[(sec, h, 1)]
                    assert hi_t == lo_t and hi_r == lo_r + BLK
                    x = qkv_sb[lo_t]
                    dst = rot_sb[(sec, h)]
                    stga = stg[f"sa{pair_i % 2}"]
                    stgb = stg[f"sb{pair_i % 2}"]
                    nc.scalar.dma_start(stga[0:BLK, :fs],
                                        x[lo_r:lo_r + BLK, f0:f1])
                    nc.scalar.dma_start(stga[64:64 + BLK, :fs],
                                        x[hi_r:hi_r + BLK, f0:f1])
                    nc.scalar.dma_start(stgb[0:BLK, :fs],
                                        x[hi_r:hi_r + BLK, f0:f1])
                    nc.scalar.dma_start(stgb[64:64 + BLK, :fs],
                                        x[lo_r:lo_r + BLK, f0:f1])
                    nc.vector.tensor_mul(dst[0:104, f0:f1], stga[0:104, :fs],
                                         cos_sb[0:104, :fs])
                    eng = nc.gpsimd if pair_i % 2 == 0 else nc.vector
                    eng.tensor_mul(stgb[0:104, :fs], stgb[0:104, :fs],
                                   sin_sb[0:104, :fs])
                    nc.vector.tensor_add(dst[0:104, f0:f1], dst[0:104, f0:f1],
                                         stgb[0:104, :fs])
                    pair_i += 1
        rope_cm.__exit__(None, None, None)

        # v_aug tiles + per-head emitter (invoked right after each head's
        # rope so attention unblocks head by head)
        vaug_cm = tc.tile_pool(name="vaug", bufs=1)
        vaug_pool = vaug_cm.__enter__()
        vaug_sb = [vaug_pool.tile([128, n_tt * VW], mm_dt, tag=f"vaug{h}",
                                  name=f"vaug{h}") for h in range(n_h)]
        vinit3 = vinit.rearrange("p (t c) -> p t c", c=VW - HD)
        for h in range(n_h):
            nc.sync.dma_start(
                vaug_sb[h].rearrange("p (t c) -> p t c", c=VW)[:, :, HD:VW],
                vinit3[:, :, :])
        GRP = 4  # key tiles transposed per psum tile / copy (1 psum bank)

        def emit_vaug(h):
            gi = 0
            while gi < n_tt:
                hi_g = min(gi + GRP, n_tt)
                if all(t_tiles[g][2] - t_tiles[g][1] == 128
                       for g in range(gi, hi_g)):
                    grp = list(range(gi, hi_g))
                else:
                    grp = [gi]
                ng = len(grp)
                tp = ps_att.tile([128, GRP * HD], f32, tag="tp", name="tp")
                for x, g in enumerate(grp):
                    si, t0, t1 = t_tiles[g]
                    sz = t1 - t0
                    for half in (0, 1):
                        vt, vr = pos[("v", h, half)]
                        nc.tensor.transpose(
                            tp[:sz, x * HD + half * BLK:
                               x * HD + (half + 1) * BLK],
                            qkv_sb[vt][0:BLK, t0:t1].bitcast(f32),
                            ident[:BLK, :BLK])
                sz0 = t_tiles[grp[0]][2] - t_tiles[grp[0]][1]
                dst = vaug_sb[h].rearrange("p (t c) -> p t c", c=VW)
                src_ap = tp.rearrange("p (t c) -> p t c", c=HD)
                if h % 2 == 0:
                    nc.vector.tensor_copy(dst[:sz0, grp[0]:grp[0] + ng, 0:HD],
                                          src_ap[:sz0, 0:ng, :])
                else:
                    nc.scalar.activation(dst[:sz0, grp[0]:grp[0] + ng, 0:HD],
                                         src_ap[:sz0, 0:ng, :], AF.Identity)
                gi += ng




        if KERNEL_DEBUG:
            for j in range(n_mtiles):
                nc.sync.dma_start(
                    dbg_qkv[:, j * S_core:(j + 1) * S_core],
                    qkv_sb[j][:].bitcast(f32))
            i_ = 0
            for h in range(n_h):
                for sec in ("q", "k"):
                    nc.sync.dma_start(
                        dbg_rot[:, i_ * S_core:(i_ + 1) * S_core],
                        rot_sb[(sec, h)][:].bitcast(f32))
                    i_ += 1

        # ------------ phase 4: attention ----------------------------
        # one attn tile per head (rows 0:80) so every compute access is
        # partition-0 based; tiles reuse the dead qkvT slots
        attn_sb = [qkv_pool.tile([128, S_core], mm_dt, tag=f"qkvT{h}",
                                 name=f"attnT{h}") for h in range(n_h)]

        seg_ttiles = {}
        for ti, (si, t0, t1) in enumerate(t_tiles):
            seg_ttiles.setdefault(si, []).append((ti, t0, t1))

        BA = 512  # attention query-chunk width (1-bank psum slots)
        with ExitStack() as p4:
            pt_pool = p4.enter_context(tc.tile_pool(name="pt", bufs=3))
            nrm_pool = p4.enter_context(tc.tile_pool(name="nrm", bufs=2))
            unit_box = [0]

            def emit_attention(h, si, a, e):
                qT = rot_sb[("q", h)]
                kT = rot_sb[("k", h)]
                q = a
                while q < e:
                    q0, q1 = q, min(q + BA, e)
                    qs = q1 - q0
                    po = ps_att.tile([128, BA], f32,
                                     tag=f"po{unit_box[0] % 2}", name="pv")
                    tts = seg_ttiles[si]
                    for idx, (ti, t0, t1) in enumerate(tts):
                        sz = t1 - t0
                        ps = ps_att.tile([128, BA], f32, tag=f"st{idx % 2}",
                                         name="st")
                        nc.tensor.matmul(ps[:sz, :qs], r_(kT[0:104, t0:t1]),
                                         r_(qT[0:104, q0:q1]),
                                         start=True, stop=True)
                        pt = pt_pool.tile([128, BA], mm_dt, tag="pt", name="pt")
                        nc.scalar.activation(pt[:sz, :qs], ps[:sz, :qs], AF.Exp)
                        nc.tensor.matmul(
                            po[:VW, :qs],
                            r_(vaug_sb[h][:sz, ti * VW:(ti + 1) * VW]),
                            r_(pt[:sz, :qs]),
                            start=(idx == 0), stop=(idx == len(tts) - 1))
                    # partition_broadcast ucode reads physical partition 0,
                    # so shift the denominator row 96 -> 0 via DMA
                    rc = nrm_pool.tile([128, BA], f32, tag="rc", name="rc")
                    nc.vector.tensor_copy(rc[96:97, :qs], po[96:97, :qs])
                    nc.sync.dma_start(rc[0:1, :qs], rc[96:97, :qs])
                    nc.vector.reciprocal(rc[0:1, :qs], rc[0:1, :qs])
                    bc = nrm_pool.tile([128, BA], mm_dt, tag="bc", name="bc")
                    nc.gpsimd.partition_broadcast(
                        bc[0:HD, :qs], rc[0:1, :qs].bitcast(mm_dt))
                    nc.vector.tensor_mul(attn_sb[h][0:HD, q0:q1],
                                         po[0:HD, :qs], bc[0:HD, :qs])
                    unit_box[0] += 1
                    q = q1

            if len(segs_local) == 1:
                a, e = segs_local[0]
                for h in range(n_h):
                    emit_vaug(h)
                    emit_attention(h, 0, a, e)
            else:
                for h in range(n_h):
                    emit_vaug(h)
                for si, (a, e) in enumerate(segs_local):
                    for h in range(n_h):
                        emit_attention(h, si, a, e)

        vaug_cm.__exit__(None, None, None)
        rot_cm.__exit__(None, None, None)

        # ------------ phase 5: projection partial -------------------
        with ExitStack() as p5:
            wp_pool = p5.enter_context(tc.tile_pool(name="wp", bufs=1))
            wp_sb = []
            for kt in range(k_proj):
                t = wp_pool.tile([HD, D], mm_dt, tag=f"wp{kt}", name=f"wp{kt}")
                nc.sync.dma_start(t[:], wprojT[kt * HD:(kt + 1) * HD, :])
                wp_sb.append(t)
            out_pool = p5.enter_context(tc.tile_pool(name="outsb", bufs=3))
            for (c0, c1) in big_chunks:
                cs = c1 - c0
                for j in range(D // 128):
                    ob = out_pool.tile([128, BC], f32, tag="ob", name="ob")
                    for (h0, h1) in halves(c0, c1):
                        ps = ps_att.tile([128, 512], f32, tag=f"st{j % 2}",
                                         name="pj")
                        for kt in range(k_proj):
                            nc.tensor.matmul(
                                ps[:, :h1 - h0],
                                r_(wp_sb[kt][:, j * 128:(j + 1) * 128]),
                                r_(attn_sb[kt][0:HD, h0:h1]),
                                start=(kt == 0), stop=(kt == k_proj - 1))
                        if j % 2 == 0:
                            nc.vector.tensor_copy(ob[:, h0 - c0:h1 - c0],
                                                  ps[:, :h1 - h0])
                        else:
                            nc.scalar.activation(ob[:, h0 - c0:h1 - c0],
                                                 ps[:, :h1 - h0], AF.Identity)
                    nc.sync.dma_start(outT[j * 128:(j + 1) * 128, c0:c1],
                                      ob[:, :cs])

    nc.compile()
    return nc


def _pack_w(Wqkv, bqkv, heads, n_h):
    """Per-core packed qkv weights (q rows pre-scaled).

    Returns wqkvT_tiled [128, NK*dims_pad] (k-major blocks of [128, dims_pad])
    and bias2d [128, n_mtiles]."""
    pos, n_mtiles = _pack_layout(n_h)
    dims_pad = n_mtiles * 128
    W = np.zeros((dims_pad, D), np.float32)
    b = np.zeros((dims_pad,), np.float32)
    sec_off = {"q": 0, "k": D, "v": 2 * D}
    for i, h in enumerate(heads):
        for sec in ("q", "k", "v"):
            for half in (0, 1):
                t, r = pos[(sec, i, half)]
                src = sec_off[sec] + h * HD + half * BLK
                w = Wqkv[src:src + BLK, :]
                bb = bqkv[src:src + BLK]
                if sec == "q":
                    w = w * SCALE
                    bb = bb * SCALE
                W[t * 128 + r:t * 128 + r + BLK] = w
                b[t * 128 + r:t * 128 + r + BLK] = bb
    w_tiled = _tile_rows(np.ascontiguousarray(W.T))
    bias2d = np.ascontiguousarray(b.reshape(n_mtiles, 128).T)
    return w_tiled, bias2d


def _tile_rows(x):
    """[R, C] with R = nk*128 -> [128, nk*C] k-major tiling."""
    R, C = x.shape
    nk = R // 128
    return np.ascontiguousarray(
        x.reshape(nk, 128, C).transpose(1, 0, 2).reshape(128, nk * C))


def _pack_wproj(Wproj, heads):
    """Rows of Wproj.T for this core's head dims, stacked per head."""
    W = np.zeros((len(heads) * HD, Wproj.shape[0]), np.float32)
    for i, h in enumerate(heads):
        W[i * HD:(i + 1) * HD] = Wproj[:, h * HD:(h + 1) * HD].T
    return W


def _pack_cos_sin(cos, sin):
    """cosP/sin2P [128, S]: lo coeffs at rows 0:40, hi at 64:104, rest 0.

    sin2P row signs match rot = x*cosP + swap(x)*sin2P: lo rows hold
    -sin_lo (they multiply x_hi), hi rows hold +sin_hi (they multiply x_lo).
    """
    S = cos.shape[0]
    cosP = np.zeros((128, S), np.float32)
    sinP = np.zeros((128, S), np.float32)
    cosP[0:BLK] = cos.T[0:BLK]
    cosP[64:64 + BLK] = cos.T[BLK:HD]
    sinP[0:BLK] = -sin.T[0:BLK]
    sinP[64:64 + BLK] = sin.T[BLK:HD]
    return cosP, sinP


_CACHE = {}


def kernel(hidden_states, cos, sin, Wqkv, bqkv, Wproj, bproj, cu_seqlens):
    sys.path.insert(0, "/opt/trn_rl_repo")
    from concourse import bass_utils

    hidden_states = np.asarray(hidden_states, np.float32)
    cos = np.asarray(cos, np.float32)
    sin = np.asarray(sin, np.float32)
    Wqkv = np.asarray(Wqkv, np.float32)
    bqkv = np.asarray(bqkv, np.float32)
    Wproj = np.asarray(Wproj, np.float32)
    bproj = np.asarray(bproj, np.float32)

    S, D_ = hidden_states.shape
    assert D_ == D
    segs = _segments(cu_seqlens, S)
    uniform = (S % 4 == 0) and segs == [(i * S // 4, (i + 1) * S // 4)
                                        for i in range(4)]

    hiddenT = np.ascontiguousarray(hidden_states.T)
    cosP, sin2P = _pack_cos_sin(cos, sin)

    def _vinit(segs_local):
        n_tt = sum(-(-(e - a) // 128) for a, e in segs_local)
        v = np.zeros((128, n_tt, 17), np.float32)
        v[:, :, 16] = 1.0
        return np.ascontiguousarray(v.reshape(128, n_tt * 17))

    if uniform:
        # mode A: 2 head-groups x 4 segments
        n_h, S_core = H // 2, S // 4
        key = ("A", S)
        if key not in _CACHE:
            _CACHE[key] = _build_program(n_h, S_core, [(0, S_core)],
                                         resident_hidden=True)
        nc = _CACHE[key]
        vinit = _vinit([(0, S_core)])
        in_maps = []
        meta = []
        for g in range(2):
            heads = list(range(g * n_h, (g + 1) * n_h))
            wt, b2 = _pack_w(Wqkv, bqkv, heads, n_h)
            wprojT = _pack_wproj(Wproj, heads)
            for s in range(4):
                sl = slice(s * S_core, (s + 1) * S_core)
                in_maps.append({
                    "hiddenT": _tile_rows(hiddenT[:, sl]),
                    "wqkvT": wt,
                    "bias2d": b2,
                    "cosP": np.ascontiguousarray(cosP[:, sl]),
                    "sin2P": np.ascontiguousarray(sin2P[:, sl]),
                    "wprojT": wprojT,
                    "vinit": vinit,
                })
                meta.append((g, s))
        res = bass_utils.run_bass_kernel_spmd(nc, in_maps,
                                              core_ids=list(range(N_CORES)))
        out = np.zeros((D, S), np.float32)
        for c, (g, s) in enumerate(meta):
            out[:, s * S_core:(s + 1) * S_core] += res.results[c]["outT"]
    else:
        # mode C: 8-way head parallel, full sequence per core
        n_h, S_core = H // N_CORES, S
        key = ("C", S, tuple(np.asarray(cu_seqlens).tolist()))
        if key not in _CACHE:
            _CACHE[key] = _build_program(n_h, S_core, segs,
                                         resident_hidden=False)
        nc = _CACHE[key]
        vinit = _vinit(segs)
        hid_tiled = _tile_rows(hiddenT)
        in_maps = []
        for c in range(N_CORES):
            heads = list(range(c * n_h, (c + 1) * n_h))
            wt, b2 = _pack_w(Wqkv, bqkv, heads, n_h)
            in_maps.append({
                "hiddenT": hid_tiled,
                "wqkvT": wt,
                "bias2d": b2,
                "cosP": cosP,
                "sin2P": sin2P,
                "wprojT": _pack_wproj(Wproj, heads),
                "vinit": vinit,
            })
        res = bass_utils.run_bass_kernel_spmd(nc, in_maps,
                                              core_ids=list(range(N_CORES)))
        out = np.zeros((D, S), np.float32)
        for c in range(N_CORES):
            out += res.results[c]["outT"]

    return np.ascontiguousarray(out.T) + bproj[None, :]



# revision 7
# speedup vs baseline: 1.1663x; 1.1663x over previous
"""Trainium2 Bass kernel for Ernie4.5-VL vision attention (ragged segments).

Contract: kernel(**inputs) takes the FULL unsharded inputs (keyed as in
setup_inputs()) and returns the FULL [S, D] float32 output.

Mode A (uniform 4x1024 segments — the graded shape): 8 cores = 2 head
groups x 4 segments; per core 8 heads x 1024 tokens, everything in bf16
on the PE array (psum f32):

  qkvT = Wpack @ hidden.T     15 dense 128-row tiles (v 80-row blocks at
                              tile h rows 0:80, q/k packed tile-major)
  rope: dense [0:80] layout; the rotate-half operand is built with 2-4
        small SBUF DMAs per (q|k, head); rot = a*cos + b*sin on DVE/Pool
  per head: v transposes (PE) -> scoresT (PE) -> exp (ACT, 1024 wide)
        -> PV accumulate with ones column for the denominator ->
        reciprocal+broadcast+mul normalize
  attn heads DMA-repacked into 5 dense 128-row tiles; proj = 5 k-tiles
  Host does O(S*D) glue: packing, summing the 2 per-token partial
  projections, bias adds.

Engine budget per core (cost model): PE ~142us of matmul rows, ACT
~82us (exp + qkv bias copies), DVE ~40us, Pool ~30us, DMA ~19MB.
Emission interleaves attention per head into the qkv j-loop so every
engine streams; all DMAs avoid the ACT queue (exp lives there).

Mode C fallback (any other cu_seqlens): 8-way head parallel fp32r path
(unchanged from the earlier version of this kernel).
"""

import os
import sys

import numpy as np

H = 16
HD = 80
BLK = 40  # rotate_half half-width
SCALE = HD ** -0.5
N_CORES = 8
D = 1280
NK = D // 128  # contraction tiles for the qkv matmul
ATTN_STRIDE = 96  # head row pitch in the packed attention output (mode C)
MM_DT_NAME = os.environ.get("KERNEL_MM_DT", "float32r")  # mode C only
KERNEL_DEBUG = bool(int(os.environ.get("KERNEL_DEBUG", "0")))

# ---- mode A constants ----
NJ = 15          # dense qkv M tiles (1920 rows)
NTT = 8          # 128-row key tiles per 1024 segment
VW = 97          # vaug slot: 80 v dims + 16 pad + ones col at 96
SA_CORE = 1024   # tokens per core


def _segments(cu_seqlens, S):
    """Intervals matching reference's searchsorted(cu[1:], i, 'right')."""
    b = np.clip(np.sort(np.asarray(cu_seqlens, dtype=np.int64)[1:5]), 0, S)
    bounds = [0] + list(b) + [S]
    segs = []
    for a, e in zip(bounds[:-1], bounds[1:]):
        if e > a:
            segs.append((int(a), int(e)))
    return segs


# ---------------------------------------------------------------------------
# mode A: dense bf16 program
# ---------------------------------------------------------------------------

def _qk_phys(o):
    """q/k space row (0..1280) -> (tile, row). Tiles 0..7 rows 80:128 hold
    48 rows each (below the v block); tiles 8..14 hold 128 each."""
    if o < 384:
        return o // 48, 80 + o % 48
    o -= 384
    return 8 + o // 128, o % 128


def _qk_pieces(o0, n):
    """Contiguous (tile, row, len, rel_off) pieces covering [o0, o0+n)."""
    out = []
    o = o0
    while o < o0 + n:
        t, r = _qk_phys(o)
        if o < 384:
            run_end = (o // 48 + 1) * 48
        else:
            run_end = 384 + ((o - 384) // 128 + 1) * 128
        ln = min(run_end, o0 + n) - o
        out.append((t, r, ln, o - o0))
        o += ln
    return out


def _head_ready_j(h):
    """Last qkv j-tile needed before head h's rope can run."""
    tiles = [h]  # v tile
    for o0 in (160 * h, 160 * h + 80):
        tiles += [t for t, _, _, _ in _qk_pieces(o0, 80)]
    return max(tiles)


def _build_program_a2():
    """Mode A program: n_h=8 heads, S=1024 tokens per core, one segment."""
    import concourse.mybir as mybir
    import concourse.tile as tile
    from concourse import bacc
    from concourse.masks import make_identity
    from contextlib import ExitStack

    f32 = mybir.dt.float32
    bf16 = mybir.dt.bfloat16
    AF = mybir.ActivationFunctionType
    n_h, S = 8, SA_CORE

    nc = bacc.Bacc("TRN2", target_bir_lowering=False, debug=False,
                   enable_asserts=False, num_devices=N_CORES)

    hiddenT = nc.dram_tensor("hiddenT", [128, NK * S], bf16,
                             kind="ExternalInput").ap()
    wqkvT = nc.dram_tensor("wqkvT", [128, NJ * NK * 128], bf16,
                           kind="ExternalInput").ap()
    bias2d = nc.dram_tensor("bias2d", [128, NJ], f32,
                            kind="ExternalInput").ap()
    cosP = nc.dram_tensor("cosP", [HD, S], bf16, kind="ExternalInput").ap()
    sinP = nc.dram_tensor("sinP", [HD, S], bf16, kind="ExternalInput").ap()
    wprojT = nc.dram_tensor("wprojT", [n_h * HD, D], bf16,
                            kind="ExternalInput").ap()
    vinit = nc.dram_tensor("vinit", [128, NTT * VW], bf16,
                           kind="ExternalInput").ap()
    outT = nc.dram_tensor("outT", [D, S], f32, kind="ExternalOutput").ap()
    if KERNEL_DEBUG:
        dbg_qkv = nc.dram_tensor("dbg_qkv", [128, NJ * S], f32,
                                 kind="ExternalOutput").ap()
        dbg_rot = nc.dram_tensor("dbg_rot", [128, 2 * n_h * S], f32,
                                 kind="ExternalOutput").ap()
        dbg_attn = nc.dram_tensor("dbg_attn", [128, n_h * S], f32,
                                  kind="ExternalOutput").ap()

    heads_after = {j: [] for j in range(NJ)}
    for h in range(n_h):
        heads_after[min(_head_ready_j(h) + 1, NJ - 1)].append(h)

    with tile.TileContext(nc) as tc, ExitStack() as ctx:
        persist = ctx.enter_context(tc.tile_pool(name="persist", bufs=1))
        ident = persist.tile([128, 128], bf16, tag="ident", name="ident")
        make_identity(nc, ident[:])
        bias_sb = persist.tile([128, NJ], f32, tag="bias", name="bias")
        nc.sync.dma_start(bias_sb[:], bias2d[:])
        cos_sb = persist.tile([128, S], bf16, tag="cos", name="cos")
        sin_sb = persist.tile([128, S], bf16, tag="sin", name="sin")

        psum = ctx.enter_context(tc.tile_pool(name="psum", bufs=1,
                                              space="PSUM"))
        qkv_pool = ctx.enter_context(tc.tile_pool(name="qkv", bufs=1))
        qkv_sb = [qkv_pool.tile([128, S], bf16, tag=f"qkvT{j}",
                                name=f"qkvT{j}") for j in range(NJ)]
        rot_pool = ctx.enter_context(tc.tile_pool(name="rot", bufs=1))
        rot_sb = {}
        for h in range(n_h):
            for sec in ("q", "k"):
                rot_sb[(sec, h)] = rot_pool.tile(
                    [128, S], bf16, tag=f"rot_{sec}{h}", name=f"rot_{sec}{h}")
        vaug_pool = ctx.enter_context(tc.tile_pool(name="vaug", bufs=1))
        vaug_sb = [vaug_pool.tile([128, NTT * VW], bf16, tag=f"vaug{h}",
                                  name=f"vaug{h}") for h in range(n_h)]
        attn_pool = ctx.enter_context(tc.tile_pool(name="attn", bufs=1))
        attn_sb = [attn_pool.tile([128, S], bf16, tag=f"attn{h}",
                                  name=f"attn{h}") for h in range(n_h)]
        stg_pool = ctx.enter_context(tc.tile_pool(name="stg", bufs=2))
        pt_pool = ctx.enter_context(tc.tile_pool(name="pt", bufs=3))
        nrm_pool = ctx.enter_context(tc.tile_pool(name="nrm", bufs=2))
        pk_pool = ctx.enter_context(tc.tile_pool(name="pk", bufs=1))
        pk_sb = [pk_pool.tile([128, S], bf16, tag=f"pk{t}", name=f"pk{t}")
                 for t in range(5)]
        wp_pool = ctx.enter_context(tc.tile_pool(name="wp", bufs=1))
        wp_sb = [wp_pool.tile([128, D], bf16, tag=f"wp{t}", name=f"wp{t}")
                 for t in range(5)]

        ps_ctr = [0]

        def ps_tile():
            t = psum.tile([128, S], f32, tag=f"ps{ps_ctr[0] % 2}", name="ps")
            ps_ctr[0] += 1
            return t

        def emit_rope(h):
            # staging DMAs (DVE queue, never ACT) + a*cos + b*sin muls
            for sec, base in (("q", 160 * h), ("k", 160 * h + 80)):
                sa = stg_pool.tile([128, S], bf16, tag="sa", name=f"sa_{sec}{h}")
                sb = stg_pool.tile([128, S], bf16, tag="sb", name=f"sb_{sec}{h}")
                for t, r, ln, off in _qk_pieces(base, HD):
                    nc.scalar.dma_start(sa[off:off + ln, :],
                                        qkv_sb[t][r:r + ln, :])
                # swapped halves: [hi; lo]
                for t, r, ln, off in _qk_pieces(base + BLK, BLK):
                    nc.scalar.dma_start(sb[off:off + ln, :],
                                        qkv_sb[t][r:r + ln, :])
                for t, r, ln, off in _qk_pieces(base, BLK):
                    nc.scalar.dma_start(sb[BLK + off:BLK + off + ln, :],
                                        qkv_sb[t][r:r + ln, :])
                rot = rot_sb[(sec, h)]
                nc.vector.tensor_mul(rot[0:HD, :], sa[0:HD, :],
                                     cos_sb[0:HD, :])
                nc.gpsimd.tensor_mul(sb[0:HD, :], sb[0:HD, :],
                                     sin_sb[0:HD, :])
                nc.vector.tensor_add(rot[0:HD, :], rot[0:HD, :], sb[0:HD, :])

        def emit_attn(h):
            # ---- v transposes -> vaug ----
            for g in range(2):  # groups of 4 key tiles
                tp = psum.tile([128, 4 * HD], bf16, tag=f"tp{g % 2}",
                               name="tp")
                for x in range(4):
                    ti = 4 * g + x
                    nc.tensor.transpose(
                        tp[:, x * HD:(x + 1) * HD],
                        qkv_sb[h][0:HD, ti * 128:(ti + 1) * 128],
                        ident[0:HD, 0:HD])
                dst = vaug_sb[h].rearrange("p (t c) -> p t c", c=VW)
                nc.vector.tensor_copy(
                    dst[:, 4 * g:4 * g + 4, 0:HD],
                    tp.rearrange("p (t c) -> p t c", c=HD)[:, 0:4, :])

            # ---- scores -> exp -> PV ----
            qT = rot_sb[("q", h)]
            kT = rot_sb[("k", h)]
            po = psum.tile([128, S], f32, tag="po", name="po")
            for ti in range(NTT):
                st = ps_tile()
                for c in (0, 512):
                    nc.tensor.matmul(st[:, c:c + 512],
                                     kT[0:HD, ti * 128:(ti + 1) * 128],
                                     qT[0:HD, c:c + 512],
                                     start=True, stop=True)
                pt = pt_pool.tile([128, S], bf16, tag="pt", name="pt")
                nc.scalar.activation(pt[:, :], st[:, :], AF.Exp)
                for c in (0, 512):
                    nc.tensor.matmul(
                        po[0:VW, c:c + 512],
                        vaug_sb[h][:, ti * VW:(ti + 1) * VW],
                        pt[:, c:c + 512],
                        start=(ti == 0), stop=(ti == NTT - 1))

            # ---- normalize ----
            rc = nrm_pool.tile([128, S], f32, tag="rc", name="rc")
            nc.vector.tensor_copy(rc[96:97, :], po[96:97, :])
            nc.sync.dma_start(rc[0:1, :], rc[96:97, :])
            nc.vector.reciprocal(rc[0:1, :], rc[0:1, :])
            bc = nrm_pool.tile([128, S], f32, tag="bc", name="bc")
            nc.gpsimd.partition_broadcast(bc[0:HD, :], rc[0:1, :])
            nc.vector.tensor_mul(attn_sb[h][0:HD, :], po[0:HD, :],
                                 bc[0:HD, :])

        def emit_repack(h):
            # dense proj k-tiles; emitted post-loop so these DMAs never
            # head-of-line-block the weight stream on the sync queue
            r0 = HD * h
            while r0 < HD * (h + 1):
                t, r = r0 // 128, r0 % 128
                ln = min(128 - r, HD * (h + 1) - r0)
                off = r0 - HD * h
                nc.sync.dma_start(pk_sb[t][r:r + ln, :],
                                  attn_sb[h][off:off + ln, :])
                r0 += ln

        # ------------ phase 1: qkv + interleaved per-head attention ----
        with ExitStack() as p1:
            hid_pool = p1.enter_context(tc.tile_pool(name="hid", bufs=1))
            w_pool = p1.enter_context(tc.tile_pool(name="wstream", bufs=4))
            hid_sb = [hid_pool.tile([128, S], bf16, tag=f"hid{k}",
                                    name=f"hid{k}") for k in range(NK)]
            wj_tiles = {}

            def load_wj(j):
                wj = w_pool.tile([128, NK * 128], bf16, tag="wj", name=f"wj{j}")
                nc.sync.dma_start(wj[:],
                                  wqkvT[:, j * NK * 128:(j + 1) * NK * 128])
                wj_tiles[j] = wj

            # DMA order on the sync queue: hid + first wjs, then the
            # small persistent inputs, then the rest of the w stream.
            nc.sync.dma_start(hid_sb[0][:], hiddenT[:, 0:S])
            load_wj(0)
            for k in range(1, 5):
                nc.sync.dma_start(hid_sb[k][:],
                                  hiddenT[:, k * S:(k + 1) * S])
            load_wj(1)
            for k in range(5, NK):
                nc.sync.dma_start(hid_sb[k][:],
                                  hiddenT[:, k * S:(k + 1) * S])
            load_wj(2)
            load_wj(3)
            nc.sync.dma_start(cos_sb[0:HD, :], cosP[:])
            nc.sync.dma_start(sin_sb[0:HD, :], sinP[:])
            for h in range(n_h):
                nc.sync.dma_start(vaug_sb[h][:], vinit[:])
            for t in range(5):
                nc.sync.dma_start(wp_sb[t][:],
                                  wprojT[t * 128:(t + 1) * 128, :])

            pending = []
            for j in range(NJ):
                if j not in wj_tiles:
                    load_wj(j)
                ps = ps_tile()
                for c in (0, 512):
                    for k in range(NK):
                        nc.tensor.matmul(
                            ps[:, c:c + 512],
                            wj_tiles[j][:, k * 128:(k + 1) * 128],
                            hid_sb[k][:, c:c + 512],
                            start=(k == 0), stop=(k == NK - 1))
                nc.scalar.activation(qkv_sb[j][:, :], ps[:, :], AF.Identity,
                                     bias=bias_sb[:, j:j + 1])
                if j + 1 < NJ and (j + 1) not in wj_tiles:
                    load_wj(j + 1)
                for h in heads_after[j]:
                    emit_rope(h)
                    pending.append(h)
                    # keep one rope of lead so attention never waits on DVE
                    while len(pending) >= 2:
                        emit_attn(pending.pop(0))
            for h in pending:
                emit_attn(h)
            for h in range(n_h):
                emit_repack(h)

        if KERNEL_DEBUG:
            for j in range(NJ):
                nc.sync.dma_start(dbg_qkv[:, j * S:(j + 1) * S],
                                    qkv_sb[j][:])
            i_ = 0
            for h in range(n_h):
                for sec in ("q", "k"):
                    nc.sync.dma_start(dbg_rot[:, i_ * S:(i_ + 1) * S],
                                        rot_sb[(sec, h)][:])
                    i_ += 1
            for h in range(n_h):
                nc.sync.dma_start(dbg_attn[:, h * S:(h + 1) * S],
                                    attn_sb[h][:])

        # ------------ phase 2: projection ---------------------------
        with ExitStack() as p5:
            out_pool = p5.enter_context(tc.tile_pool(name="outsb", bufs=3))
            for j in range(D // 128):
                ps = ps_tile()
                for c in (0, 512):
                    for kt in range(5):
                        nc.tensor.matmul(
                            ps[:, c:c + 512],
                            wp_sb[kt][:, j * 128:(j + 1) * 128],
                            pk_sb[kt][:, c:c + 512],
                            start=(kt == 0), stop=(kt == 4))
                ob = out_pool.tile([128, S], f32, tag="ob", name="ob")
                if j % 2 == 0:
                    nc.vector.tensor_copy(ob[:, :], ps[:, :])
                else:
                    nc.scalar.activation(ob[:, :], ps[:, :], AF.Identity)
                nc.sync.dma_start(outT[j * 128:(j + 1) * 128, :], ob[:, :])

    nc.compile()
    return nc


def _pack_w_a2(Wqkv, bqkv, heads):
    """Dense 15-tile packing: v at tile h rows 0:80, q/k tile-major."""
    import ml_dtypes
    perm = np.zeros((NJ * 128,), np.int64)
    scl = np.ones((NJ * 128,), np.float32)
    used = np.zeros((NJ * 128,), bool)
    for i, h in enumerate(heads):
        for d in range(HD):
            perm[i * 128 + d] = 2 * D + h * HD + d  # v
            used[i * 128 + d] = True
        for sec, base in ((0, 160 * i), (1, 160 * i + 80)):
            src0 = sec * D + h * HD
            for d in range(HD):
                t, r = _qk_phys(base + d)
                perm[t * 128 + r] = src0 + d
                used[t * 128 + r] = True
                if sec == 0:
                    scl[t * 128 + r] = SCALE
    W = Wqkv[perm] * scl[:, None]
    W[~used] = 0.0
    b = bqkv[perm] * scl
    b[~used] = 0.0
    # wqkvT host layout: [128, j, k, 128]; [p, j, k, m] = W.T[k*128+p, j*128+m]
    WT = np.ascontiguousarray(W.T)  # [1280, 1920]
    wt = WT.reshape(NK, 128, NJ, 128).transpose(1, 2, 0, 3)
    wt = np.ascontiguousarray(wt.reshape(128, NJ * NK * 128))
    bias2d = np.ascontiguousarray(b.reshape(NJ, 128).T)
    return wt.astype(ml_dtypes.bfloat16), bias2d


def _pack_wproj(Wproj, heads):
    """Rows of Wproj.T for this core's head dims, stacked per head."""
    W = np.zeros((len(heads) * HD, Wproj.shape[0]), np.float32)
    for i, h in enumerate(heads):
        W[i * HD:(i + 1) * HD] = Wproj[:, h * HD:(h + 1) * HD].T
    return W


_CACHE = {}


def _kernel_mode_a(hidden_states, cos, sin, Wqkv, bqkv, Wproj, bproj, S):
    import ml_dtypes
    from concourse import bass_utils

    n_h, S_core = H // 2, S // 4
    if "A2" not in _CACHE:
        _CACHE["A2"] = _build_program_a2()
    nc = _CACHE["A2"]

    bf = ml_dtypes.bfloat16
    hiddenT = np.ascontiguousarray(hidden_states.T)  # [D, S]

    vinit = np.zeros((128, NTT, VW), np.float32)
    vinit[:, :, 96] = 1.0
    vinit = np.ascontiguousarray(vinit.reshape(128, NTT * VW)).astype(bf)

    in_maps = []
    meta = []
    for g in range(2):
        heads = list(range(g * n_h, (g + 1) * n_h))
        wt, b2 = _pack_w_a2(Wqkv, bqkv, heads)
        wprojT = _pack_wproj(Wproj, heads).astype(bf)
        for s in range(4):
            sl = slice(s * S_core, (s + 1) * S_core)
            hseg = hiddenT[:, sl]  # [1280, 1024]
            hid_t = np.ascontiguousarray(
                hseg.reshape(NK, 128, S_core).transpose(1, 0, 2)
                .reshape(128, NK * S_core)).astype(bf)
            cosP = np.ascontiguousarray(cos[sl].T).astype(bf)
            sinP = np.concatenate(
                [-sin[sl].T[0:BLK], sin[sl].T[BLK:HD]], axis=0)
            sinP = np.ascontiguousarray(sinP).astype(bf)
            in_maps.append({
                "hiddenT": hid_t,
                "wqkvT": wt,
                "bias2d": b2,
                "cosP": cosP,
                "sinP": sinP,
                "wprojT": wprojT,
                "vinit": vinit,
            })
            meta.append((g, s))
    res = bass_utils.run_bass_kernel_spmd(nc, in_maps,
                                          core_ids=list(range(N_CORES)))
    out = np.zeros((D, S), np.float32)
    for c, (g, s) in enumerate(meta):
        out[:, s * S_core:(s + 1) * S_core] += res.results[c]["outT"]
    return out


# ---------------------------------------------------------------------------
# mode C (non-uniform segments): fp32r 8-way head-parallel fallback
# ---------------------------------------------------------------------------

def _pack_layout(n_h):
    """Pack per-core qkv dims as 40-row blocks, 3 per 128-row tile (8 pad).

    Each tile holds one v-block at row 0 (PE transpose operands must start
    at a 32-aligned partition) and two q/k blocks at rows 40 and 80.
    Returns pos[(sec, h, half)] = (tile, row) and the number of tiles.
    """
    ntiles = 2 * n_h
    pos = {}
    for h in range(n_h):
        for half in (0, 1):
            pos[("v", h, half)] = (2 * h + half, 0)
    qk = [("q", h, half) for h in range(n_h) for half in (0, 1)]
    qk += [("k", h, half) for h in range(n_h) for half in (0, 1)]
    for j, blk in enumerate(qk):
        pos[blk] = (j // 2, BLK + BLK * (j % 2))
    return pos, ntiles


def _build_program(n_h, S_core, segs_local, resident_hidden):
    """Mode C SPMD program (fp32r)."""
    import concourse.mybir as mybir
    import concourse.tile as tile
    from concourse import bacc
    from concourse.masks import make_identity
    from contextlib import ExitStack

    f32 = mybir.dt.float32
    mm_dt = getattr(mybir.dt, MM_DT_NAME)
    AF = mybir.ActivationFunctionType

    k_proj = n_h
    pos, n_mtiles = _pack_layout(n_h)
    dims_pad = n_mtiles * 128
    VWc = 97

    t_tiles = []
    for si, (a, e) in enumerate(segs_local):
        t = a
        while t < e:
            t_tiles.append((si, t, min(t + 128, e)))
            t += 128
    n_tt = len(t_tiles)

    nc = bacc.Bacc("TRN2", target_bir_lowering=False, debug=False,
                   enable_asserts=False, num_devices=N_CORES)

    hiddenT = nc.dram_tensor("hiddenT", [128, NK * S_core], mm_dt,
                             kind="ExternalInput").ap()
    wqkvT = nc.dram_tensor("wqkvT", [128, NK * dims_pad], mm_dt,
                           kind="ExternalInput").ap()
    bias2d = nc.dram_tensor("bias2d", [128, n_mtiles], f32,
                            kind="ExternalInput").ap()
    cosP = nc.dram_tensor("cosP", [128, S_core], mm_dt,
                          kind="ExternalInput").ap()
    sin2P = nc.dram_tensor("sin2P", [128, S_core], mm_dt,
                           kind="ExternalInput").ap()
    wprojT = nc.dram_tensor("wprojT", [n_h * HD, D], mm_dt,
                            kind="ExternalInput").ap()
    vinit = nc.dram_tensor("vinit", [128, n_tt * (VWc - HD)], mm_dt,
                           kind="ExternalInput").ap()
    outT = nc.dram_tensor("outT", [D, S_core], f32, kind="ExternalOutput").ap()

    def r_(ap):
        return ap.bitcast(mm_dt)

    BC = 1024
    big_chunks = [(c, min(c + BC, S_core)) for c in range(0, S_core, BC)]

    def halves(c0, c1):
        out = []
        q = c0
        while q < c1:
            out.append((q, min(q + 512, c1)))
            q = q + 512
        return out

    with tile.TileContext(nc) as tc, ExitStack() as ctx:
        persist = ctx.enter_context(tc.tile_pool(name="persist", bufs=1))
        ident = persist.tile([128, 128], f32, tag="ident", name="ident")
        make_identity(nc, ident[:])
        bias_sb = persist.tile([128, n_mtiles], f32, tag="bias", name="bias")
        nc.sync.dma_start(bias_sb[:], bias2d[:])

        psum_all_cm = tc.tile_pool(name="psum_all", bufs=1, space="PSUM")
        psum_all = psum_all_cm.__enter__()
        qkv_pool = ctx.enter_context(tc.tile_pool(name="big", bufs=1))
        qkv_sb = [qkv_pool.tile([128, S_core], mm_dt, tag=f"qkvT{j}",
                                name=f"qkvT{j}") for j in range(n_mtiles)]
        rot_cm = tc.tile_pool(name="rot", bufs=1)
        rv = rot_cm.__enter__()
        rot_sb = {}
        for h in range(n_h):
            for sec in ("q", "k"):
                rot_sb[(sec, h)] = rv.tile([128, S_core], mm_dt,
                                           tag=f"rot_{sec}{h}",
                                           name=f"rot_{sec}{h}")
        RC = 1024
        rope_cm = tc.tile_pool(name="rope_scr", bufs=2)
        rope_scr = rope_cm.__enter__()

        with ExitStack() as p1:
            hidden3 = hiddenT.rearrange("p (k s) -> p k s", k=NK)
            w3 = wqkvT.rearrange("p (k m) -> p k m", k=NK)
            w_pool = p1.enter_context(tc.tile_pool(name="wres", bufs=1))
            w_sb = [w_pool.tile([128, dims_pad], mm_dt, tag=f"w{k}",
                                name=f"w{k}") for k in range(NK)]
            for k in range(NK):
                nc.sync.dma_start(w_sb[k][:], w3[:, k, :])
            hid_pool = p1.enter_context(tc.tile_pool(name="hidstream",
                                                     bufs=3))
            n4 = n_mtiles // 4
            for (h0, h1) in halves(0, S_core):
                hw = h1 - h0
                for q4 in range(n4):
                    ps01 = psum_all.tile([128, BC], f32, tag="t0",
                                         name="ps01")
                    ps23 = psum_all.tile([128, BC], f32, tag="t1",
                                         name="ps23")
                    pj_of = lambda j: (ps01 if j % 4 < 2 else ps23,
                                       (j % 2) * 512)
                    for k in range(NK):
                        ht = hid_pool.tile([128, 512], mm_dt, tag="hidc",
                                           name="hidc")
                        nc.sync.dma_start(ht[:, :hw], hidden3[:, k, h0:h1])
                        for j in range(q4 * 4, q4 * 4 + 4):
                            psj, co = pj_of(j)
                            nc.tensor.matmul(
                                psj[:, co:co + hw],
                                r_(w_sb[k][:, j * 128:(j + 1) * 128]),
                                r_(ht[:, :hw]),
                                start=(k == 0), stop=(k == NK - 1))
                    for j in range(q4 * 4, q4 * 4 + 4):
                        psj, co = pj_of(j)
                        nc.scalar.activation(qkv_sb[j][:, h0:h1],
                                             psj[:, co:co + hw], AF.Identity,
                                             bias=bias_sb[:, j:j + 1])

        psum_all_cm.__exit__(None, None, None)
        ps_att = ctx.enter_context(tc.tile_pool(name="ps_att", bufs=1,
                                                space="PSUM"))

        stg = {}
        for nm in ("sa0", "sa1", "sb0", "sb1"):
            stg[nm] = rope_scr.tile([128, RC], mm_dt, tag=nm, name=nm, bufs=1)
        pair_i = 0
        for ci, f0 in enumerate(range(0, S_core, RC)):
            f1 = min(f0 + RC, S_core)
            fs = f1 - f0
            cos_sb = rope_scr.tile([128, RC], mm_dt, tag="cos", name="cos",
                                   bufs=1)
            sin_sb = rope_scr.tile([128, RC], mm_dt, tag="sin", name="sin",
                                   bufs=1)
            nc.scalar.dma_start(cos_sb[:, :fs], cosP[:, f0:f1])
            nc.scalar.dma_start(sin_sb[:, :fs], sin2P[:, f0:f1])
            if ci == 0:
                for nm in stg:
                    nc.scalar.dma_start(stg[nm][BLK:64, :], cos_sb[BLK:64, :])
            for h in range(n_h):
                for sec in ("q", "k"):
                    lo_t, lo_r = pos[(sec, h, 0)]
                    hi_t, hi_r = pos[(sec, h, 1)]
                    x = qkv_sb[lo_t]
                    dst = rot_sb[(sec, h)]
                    stga = stg[f"sa{pair_i % 2}"]
                    stgb = stg[f"sb{pair_i % 2}"]
                    nc.scalar.dma_start(stga[0:BLK, :fs],
                                        x[lo_r:lo_r + BLK, f0:f1])
                    nc.scalar.dma_start(stga[64:64 + BLK, :fs],
                                        x[hi_r:hi_r + BLK, f0:f1])
                    nc.scalar.dma_start(stgb[0:BLK, :fs],
                                        x[hi_r:hi_r + BLK, f0:f1])
                    nc.scalar.dma_start(stgb[64:64 + BLK, :fs],
                                        x[lo_r:lo_r + BLK, f0:f1])
                    nc.vector.tensor_mul(dst[0:104, f0:f1], stga[0:104, :fs],
                                         cos_sb[0:104, :fs])
                    eng = nc.gpsimd if pair_i % 2 == 0 else nc.vector
                    eng.tensor_mul(stgb[0:104, :fs], stgb[0:104, :fs],
                                   sin_sb[0:104, :fs])
                    nc.vector.tensor_add(dst[0:104, f0:f1], dst[0:104, f0:f1],
                                         stgb[0:104, :fs])
                    pair_i += 1
        rope_cm.__exit__(None, None, None)

        vaug_cm = tc.tile_pool(name="vaug", bufs=1)
        vaug_pool = vaug_cm.__enter__()
        vaug_sb = [vaug_pool.tile([128, n_tt * VWc], mm_dt, tag=f"vaug{h}",
                                  name=f"vaug{h}") for h in range(n_h)]
        vinit3 = vinit.rearrange("p (t c) -> p t c", c=VWc - HD)
        for h in range(n_h):
            nc.sync.dma_start(
                vaug_sb[h].rearrange("p (t c) -> p t c", c=VWc)[:, :, HD:VWc],
                vinit3[:, :, :])
        GRP = 4

        def emit_vaug(h):
            gi = 0
            while gi < n_tt:
                hi_g = min(gi + GRP, n_tt)
                if all(t_tiles[g][2] - t_tiles[g][1] == 128
                       for g in range(gi, hi_g)):
                    grp = list(range(gi, hi_g))
                else:
                    grp = [gi]
                ng = len(grp)
                tp = ps_att.tile([128, GRP * HD], f32, tag="tp", name="tp")
                for x, g in enumerate(grp):
                    si, t0, t1 = t_tiles[g]
                    sz = t1 - t0
                    for half in (0, 1):
                        vt, vr = pos[("v", h, half)]
                        nc.tensor.transpose(
                            tp[:sz, x * HD + half * BLK:
                               x * HD + (half + 1) * BLK],
                            qkv_sb[vt][0:BLK, t0:t1].bitcast(f32),
                            ident[:BLK, :BLK])
                sz0 = t_tiles[grp[0]][2] - t_tiles[grp[0]][1]
                dst = vaug_sb[h].rearrange("p (t c) -> p t c", c=VWc)
                src_ap = tp.rearrange("p (t c) -> p t c", c=HD)
                nc.vector.tensor_copy(dst[:sz0, grp[0]:grp[0] + ng, 0:HD],
                                      src_ap[:sz0, 0:ng, :])
                gi += ng

        attn_sb = [qkv_pool.tile([128, S_core], mm_dt, tag=f"qkvT{hh}",
                                 name=f"attnT{hh}") for hh in range(n_h)]

        seg_ttiles = {}
        for ti, (si, t0, t1) in enumerate(t_tiles):
            seg_ttiles.setdefault(si, []).append((ti, t0, t1))

        BA = 512
        with ExitStack() as p4:
            pt_pool = p4.enter_context(tc.tile_pool(name="pt", bufs=3))
            nrm_pool = p4.enter_context(tc.tile_pool(name="nrm", bufs=2))
            unit_box = [0]

            def emit_attention(h, si, a, e):
                qT = rot_sb[("q", h)]
                kT = rot_sb[("k", h)]
                q = a
                while q < e:
                    q0, q1 = q, min(q + BA, e)
                    qs = q1 - q0
                    po = ps_att.tile([128, BA], f32,
                                     tag=f"po{unit_box[0] % 2}", name="pv")
                    tts = seg_ttiles[si]
                    for idx, (ti, t0, t1) in enumerate(tts):
                        sz = t1 - t0
                        ps = ps_att.tile([128, BA], f32, tag=f"st{idx % 2}",
                                         name="st")
                        nc.tensor.matmul(ps[:sz, :qs], r_(kT[0:104, t0:t1]),
                                         r_(qT[0:104, q0:q1]),
                                         start=True, stop=True)
                        pt = pt_pool.tile([128, BA], mm_dt, tag="pt",
                                          name="pt")
                        nc.scalar.activation(pt[:sz, :qs], ps[:sz, :qs],
                                             AF.Exp)
                        nc.tensor.matmul(
                            po[:VWc, :qs],
                            r_(vaug_sb[h][:sz, ti * VWc:(ti + 1) * VWc]),
                            r_(pt[:sz, :qs]),
                            start=(idx == 0), stop=(idx == len(tts) - 1))
                    rc = nrm_pool.tile([128, BA], f32, tag="rc", name="rc")
                    nc.vector.tensor_copy(rc[96:97, :qs], po[96:97, :qs])
                    nc.sync.dma_start(rc[0:1, :qs], rc[96:97, :qs])
                    nc.vector.reciprocal(rc[0:1, :qs], rc[0:1, :qs])
                    bc = nrm_pool.tile([128, BA], mm_dt, tag="bc", name="bc")
                    nc.gpsimd.partition_broadcast(
                        bc[0:HD, :qs], rc[0:1, :qs].bitcast(mm_dt))
                    nc.vector.tensor_mul(attn_sb[h][0:HD, q0:q1],
                                         po[0:HD, :qs], bc[0:HD, :qs])
                    unit_box[0] += 1
                    q = q1

            for h in range(n_h):
                emit_vaug(h)
            for si, (a, e) in enumerate(segs_local):
                for h in range(n_h):
                    emit_attention(h, si, a, e)

        vaug_cm.__exit__(None, None, None)
        rot_cm.__exit__(None, None, None)

        with ExitStack() as p5:
            wp_pool = p5.enter_context(tc.tile_pool(name="wp", bufs=1))
            wp_sb = []
            for kt in range(k_proj):
                t = wp_pool.tile([HD, D], mm_dt, tag=f"wp{kt}", name=f"wp{kt}")
                nc.sync.dma_start(t[:], wprojT[kt * HD:(kt + 1) * HD, :])
                wp_sb.append(t)
            out_pool = p5.enter_context(tc.tile_pool(name="outsb", bufs=3))
            for (c0, c1) in big_chunks:
                cs = c1 - c0
                for j in range(D // 128):
                    ob = out_pool.tile([128, BC], f32, tag="ob", name="ob")
                    for (h0, h1) in halves(c0, c1):
                        ps = ps_att.tile([128, 512], f32, tag=f"st{j % 2}",
                                         name="pj")
                        for kt in range(k_proj):
                            nc.tensor.matmul(
                                ps[:, :h1 - h0],
                                r_(wp_sb[kt][:, j * 128:(j + 1) * 128]),
                                r_(attn_sb[kt][0:HD, h0:h1]),
                                start=(kt == 0), stop=(kt == k_proj - 1))
                        if j % 2 == 0:
                            nc.vector.tensor_copy(ob[:, h0 - c0:h1 - c0],
                                                  ps[:, :h1 - h0])
                        else:
                            nc.scalar.activation(ob[:, h0 - c0:h1 - c0],
                                                 ps[:, :h1 - h0], AF.Identity)
                    nc.sync.dma_start(outT[j * 128:(j + 1) * 128, c0:c1],
                                      ob[:, :cs])

    nc.compile()
    return nc


def _pack_w(Wqkv, bqkv, heads, n_h):
    """Mode C packed qkv weights (q rows pre-scaled)."""
    pos, n_mtiles = _pack_layout(n_h)
    dims_pad = n_mtiles * 128
    W = np.zeros((dims_pad, D), np.float32)
    b = np.zeros((dims_pad,), np.float32)
    sec_off = {"q": 0, "k": D, "v": 2 * D}
    for i, h in enumerate(heads):
        for sec in ("q", "k", "v"):
            for half in (0, 1):
                t, r = pos[(sec, i, half)]
                src = sec_off[sec] + h * HD + half * BLK
                w = Wqkv[src:src + BLK, :]
                bb = bqkv[src:src + BLK]
                if sec == "q":
                    w = w * SCALE
                    bb = bb * SCALE
                W[t * 128 + r:t * 128 + r + BLK] = w
                b[t * 128 + r:t * 128 + r + BLK] = bb
    w_tiled = _tile_rows(np.ascontiguousarray(W.T))
    bias2d = np.ascontiguousarray(b.reshape(n_mtiles, 128).T)
    return w_tiled, bias2d


def _tile_rows(x):
    """[R, C] with R = nk*128 -> [128, nk*C] k-major tiling."""
    R, C = x.shape
    nk = R // 128
    return np.ascontiguousarray(
        x.reshape(nk, 128, C).transpose(1, 0, 2).reshape(128, nk * C))


def _pack_cos_sin(cos, sin):
    """Mode C cosP/sin2P [128, S]."""
    S = cos.shape[0]
    cosP = np.zeros((128, S), np.float32)
    sinP = np.zeros((128, S), np.float32)
    cosP[0:BLK] = cos.T[0:BLK]
    cosP[64:64 + BLK] = cos.T[BLK:HD]
    sinP[0:BLK] = -sin.T[0:BLK]
    sinP[64:64 + BLK] = sin.T[BLK:HD]
    return cosP, sinP


def kernel(hidden_states, cos, sin, Wqkv, bqkv, Wproj, bproj, cu_seqlens):
    sys.path.insert(0, "/opt/trn_rl_repo")
    from concourse import bass_utils

    hidden_states = np.asarray(hidden_states, np.float32)
    cos = np.asarray(cos, np.float32)
    sin = np.asarray(sin, np.float32)
    Wqkv = np.asarray(Wqkv, np.float32)
    bqkv = np.asarray(bqkv, np.float32)
    Wproj = np.asarray(Wproj, np.float32)
    bproj = np.asarray(bproj, np.float32)

    S, D_ = hidden_states.shape
    assert D_ == D
    segs = _segments(cu_seqlens, S)
    uniform = (S == 4096) and segs == [(i * S // 4, (i + 1) * S // 4)
                                       for i in range(4)]

    if uniform:
        out = _kernel_mode_a(hidden_states, cos, sin, Wqkv, bqkv, Wproj,
                             bproj, S)
    else:
        hiddenT = np.ascontiguousarray(hidden_states.T)
        cosP, sin2P = _pack_cos_sin(cos, sin)
        n_h, S_core = H // N_CORES, S
        key = ("C", S, tuple(np.asarray(cu_seqlens).tolist()))
        if key not in _CACHE:
            _CACHE[key] = _build_program(n_h, S_core, segs,
                                         resident_hidden=False)
        nc = _CACHE[key]
        n_tt = sum(-(-(e - a) // 128) for a, e in segs)
        vinit = np.zeros((128, n_tt, 17), np.float32)
        vinit[:, :, 16] = 1.0
        vinit = np.ascontiguousarray(vinit.reshape(128, n_tt * 17))
        hid_tiled = _tile_rows(hiddenT)
        in_maps = []
        for c in range(N_CORES):
            heads = list(range(c * n_h, (c + 1) * n_h))
            wt, b2 = _pack_w(Wqkv, bqkv, heads, n_h)
            in_maps.append({
                "hiddenT": hid_tiled,
                "wqkvT": wt,
                "bias2d": b2,
                "cosP": cosP,
                "sin2P": sin2P,
                "wprojT": _pack_wproj(Wproj, heads).astype(np.float32),
                "vinit": vinit,
            })
        res = bass_utils.run_bass_kernel_spmd(nc, in_maps,
                                              core_ids=list(range(N_CORES)))
        out = np.zeros((D, S), np.float32)
        for c in range(N_CORES):
            out += res.results[c]["outT"]

    return np.ascontiguousarray(out.T) + bproj[None, :]


# revision 36
# speedup vs baseline: 1.3624x; 1.1681x over previous
"""Trainium2 Bass kernel for Ernie4.5-VL vision attention (ragged segments).

Contract: kernel(**inputs) takes the FULL unsharded inputs (keyed as in
setup_inputs()) and returns the FULL [S, D] float32 output.

Mode A (uniform 4x1024 segments — the graded shape): 8 cores = 2 head
groups x 4 segments; per core 8 heads x 1024 tokens, everything in bf16
on the PE array (psum f32):

  qkvT = Wpack @ hidden.T     15 dense 128-row tiles (v 80-row blocks at
                              tile h rows 0:80, q/k packed tile-major)
  rope: dense [0:80] layout; the rotate-half operand is built with 2-4
        small SBUF DMAs per (q|k, head); rot = a*cos + b*sin on DVE/Pool
  per head: v transposes (PE) -> scoresT (PE) -> exp (ACT, 1024 wide)
        -> PV accumulate with ones column for the denominator ->
        reciprocal+broadcast+mul normalize
  attn heads DMA-repacked into 5 dense 128-row tiles; proj = 5 k-tiles
  Host does O(S*D) glue: packing, summing the 2 per-token partial
  projections, bias adds.

Engine budget per core (cost model): PE ~142us of matmul rows, ACT
~82us (exp + qkv bias copies), DVE ~40us, Pool ~30us, DMA ~19MB.
Emission interleaves attention per head into the qkv j-loop so every
engine streams; all DMAs avoid the ACT queue (exp lives there).

Mode C fallback (any other cu_seqlens): 8-way head parallel fp32r path
(unchanged from the earlier version of this kernel).
"""

import os
import sys

import numpy as np

H = 16
HD = 80
BLK = 40  # rotate_half half-width
SCALE = HD ** -0.5
N_CORES = 8
D = 1280
NK = D // 128  # contraction tiles for the qkv matmul
ATTN_STRIDE = 96  # head row pitch in the packed attention output (mode C)
MM_DT_NAME = os.environ.get("KERNEL_MM_DT", "float32r")  # mode C only
KERNEL_DEBUG = bool(int(os.environ.get("KERNEL_DEBUG", "0")))

# ---- mode A constants ----
NJ = 15          # dense qkv M tiles (1920 rows)
NTT = 8          # 128-row key tiles per 1024 segment
VW = 97          # vaug slot: 80 v dims + 16 pad + ones col at 96
SA_CORE = 1024   # tokens per core


def _segments(cu_seqlens, S):
    """Intervals matching reference's searchsorted(cu[1:], i, 'right')."""
    b = np.clip(np.sort(np.asarray(cu_seqlens, dtype=np.int64)[1:5]), 0, S)
    bounds = [0] + list(b) + [S]
    segs = []
    for a, e in zip(bounds[:-1], bounds[1:]):
        if e > a:
            segs.append((int(a), int(e)))
    return segs


# ---------------------------------------------------------------------------
# mode A: dense bf16 program
# ---------------------------------------------------------------------------

def _layout_a2():
    """Per-head-contiguous packing: head h owns global rows [240h, 240h+240).
    v sits at rows 0:80 of tile ceil(240h/128) (PE transpose needs a
    32-aligned non-crossing 80-row read); q then k fill the remaining
    window rows in ascending order (read via DMA, placement free).

    Returns (v_tile[h], q_rows[h], k_rows[h], ready_j[h]) where
    q_rows/k_rows are the 80 global rows of each section in dim order.
    """
    v_tile, q_rows, k_rows, ready = [], [], [], []
    for h in range(8):
        w0, w1 = 240 * h, 240 * (h + 1)
        th = -(-w0 // 128)
        vg0 = 128 * th
        qk = [g for g in range(w0, w1) if not (vg0 <= g < vg0 + HD)]
        v_tile.append(th)
        q_rows.append(qk[0:HD])
        k_rows.append(qk[HD:2 * HD])
        ready.append(max(th, qk[-1] // 128))
    return v_tile, q_rows, k_rows, ready


def _row_pieces(rows):
    """Split a list of global rows into (tile, row, len, rel_off) runs that
    are consecutive and stay within one 128-row tile."""
    out = []
    i = 0
    while i < len(rows):
        g = rows[i]
        n = 1
        while (i + n < len(rows) and rows[i + n] == g + n
               and (g + n) // 128 == g // 128):
            n += 1
        out.append((g // 128, g % 128, n, i))
        i += n
    return out


def _build_program_a2(bias_zero=True):
    """Mode A program: n_h=8 heads, S=1024 tokens per core, one segment."""
    import concourse.mybir as mybir
    import concourse.tile as tile
    from concourse import bacc
    from concourse.masks import make_identity
    from contextlib import ExitStack

    f32 = mybir.dt.float32
    bf16 = mybir.dt.bfloat16
    AF = mybir.ActivationFunctionType
    n_h, S = 8, SA_CORE

    nc = bacc.Bacc("TRN2", target_bir_lowering=False, debug=False,
                   enable_asserts=False, num_devices=N_CORES)

    hiddenT = nc.dram_tensor("hiddenT", [128, NK * S], bf16,
                             kind="ExternalInput").ap()
    wqkvT = nc.dram_tensor("wqkvT", [128, NJ * NK * 128], bf16,
                           kind="ExternalInput").ap()
    bias2d = nc.dram_tensor("bias2d", [128, NJ], f32,
                            kind="ExternalInput").ap()
    cosP = nc.dram_tensor("cosP", [HD, S], bf16, kind="ExternalInput").ap()
    sinP = nc.dram_tensor("sinP", [HD, S], bf16, kind="ExternalInput").ap()
    wprojT = nc.dram_tensor("wprojT", [n_h * HD, D], bf16,
                            kind="ExternalInput").ap()
    vinit = nc.dram_tensor("vinit", [128, 8 * NTT * VW], bf16,
                           kind="ExternalInput").ap()
    outT = nc.dram_tensor("outT", [D, S], bf16, kind="ExternalOutput").ap()
    if KERNEL_DEBUG:
        dbg_qkv = nc.dram_tensor("dbg_qkv", [128, NJ * S], f32,
                                 kind="ExternalOutput").ap()
        dbg_rot = nc.dram_tensor("dbg_rot", [128, 2 * n_h * S], f32,
                                 kind="ExternalOutput").ap()
        dbg_attn = nc.dram_tensor("dbg_attn", [128, n_h * S], f32,
                                  kind="ExternalOutput").ap()

    v_tile, q_rows, k_rows, ready = _layout_a2()
    rope_at = {j: [] for j in range(NJ)}   # (h, sec) at section readiness
    vaug_at = {j: [] for j in range(NJ)}   # h at v-tile readiness
    vaug_deferred = []
    for h in range(n_h):
        for sec, rows in (("q", q_rows[h]), ("k", k_rows[h])):
            rope_at[max(g // 128 for g in rows)].append((h, sec))
        if v_tile[h] >= NJ - 1:
            vaug_deferred.append(h)
        else:
            vaug_at[v_tile[h]].append(h)

    with tile.TileContext(nc) as tc, ExitStack() as ctx:
        persist = ctx.enter_context(tc.tile_pool(name="persist", bufs=1))
        ident = persist.tile([128, 128], bf16, tag="ident", name="ident")
        make_identity(nc, ident[:])
        bias_sb = persist.tile([128, NJ], f32, tag="bias", name="bias")
        cos_sb = persist.tile([128, S], bf16, tag="cos", name="cos")
        sin_sb = persist.tile([128, S], bf16, tag="sin", name="sin")

        psum = ctx.enter_context(tc.tile_pool(name="psum", bufs=1,
                                              space="PSUM"))
        qkv_pool = ctx.enter_context(tc.tile_pool(name="qkv", bufs=1))
        qkv_sb = [qkv_pool.tile([128, S], bf16, tag=f"qkvT{j}",
                                name=f"qkvT{j}") for j in range(NJ)]
        rot_pool = ctx.enter_context(tc.tile_pool(name="rot", bufs=1))
        rot_sb = {}
        for h in range(n_h):
            for sec in ("q", "k"):
                rot_sb[(sec, h)] = rot_pool.tile(
                    [128, S], bf16, tag=f"rot_{sec}{h}", name=f"rot_{sec}{h}")
        vaug_pool = ctx.enter_context(tc.tile_pool(name="vaug", bufs=1))
        vaug_all = vaug_pool.tile([128, n_h * NTT * VW], bf16, tag="vaug",
                                  name="vaug")
        vaug_sb = [vaug_all[:, h * NTT * VW:(h + 1) * NTT * VW]
                   for h in range(n_h)]
        attn_pool = ctx.enter_context(tc.tile_pool(name="attn", bufs=1))
        attn_sb = [attn_pool.tile([128, S], bf16, tag=f"attn{h}",
                                  name=f"attn{h}") for h in range(n_h)]
        stg_pool = ctx.enter_context(tc.tile_pool(name="stg", bufs=2))
        pt_pool = ctx.enter_context(tc.tile_pool(name="pt", bufs=4))
        nrm_pool = ctx.enter_context(tc.tile_pool(name="nrm", bufs=2))
        pk_pool = ctx.enter_context(tc.tile_pool(name="pk", bufs=1))
        pk_sb = [pk_pool.tile([128, S], bf16, tag=f"pk{t}", name=f"pk{t}")
                 for t in range(5)]
        wp_pool = ctx.enter_context(tc.tile_pool(name="wp", bufs=1))
        wp_sb = [wp_pool.tile([128, D], bf16, tag=f"wp{t}", name=f"wp{t}")
                 for t in range(5)]


        def emit_rope(h, sec, late=False):
            # sa (x in dim order) staged from qkv tiles on the SP queue;
            # sb = rotate_half(sa) built from sa with exactly two Pool
            # (SWDGE) DMAs. DMA count is precious: each dma_start holds its
            # queue for wait+transfer+sem (~1.1us fixed).
            rows = q_rows[h] if sec == "q" else k_rows[h]
            sa = stg_pool.tile([128, S], bf16, tag="sa", name=f"sa_{sec}{h}")
            sb = stg_pool.tile([128, S], bf16, tag="sb", name=f"sb_{sec}{h}")
            for t, r, ln, off in _row_pieces(rows):
                nc.sync.dma_start(sa[off:off + ln, :],
                                  qkv_sb[t][r:r + ln, :])
            if late:
                # endgame: stage sb straight from qkv tiles (parallel with
                # sa) so the last ropes don't pay the serial sa->sb hop
                for t, r, ln, off in _row_pieces(rows[BLK:]):
                    nc.sync.dma_start(sb[off:off + ln, :],
                                      qkv_sb[t][r:r + ln, :])
                for t, r, ln, off in _row_pieces(rows[:BLK]):
                    nc.sync.dma_start(sb[BLK + off:BLK + off + ln, :],
                                      qkv_sb[t][r:r + ln, :])
            else:
                nc.gpsimd.dma_start(sb[0:BLK, :], sa[BLK:HD, :])
                nc.gpsimd.dma_start(sb[BLK:HD, :], sa[0:BLK, :])
            rot = rot_sb[(sec, h)]
            nc.vector.tensor_mul(rot[0:HD, :], sa[0:HD, :], cos_sb[0:HD, :])
            nc.vector.tensor_mul(sb[0:HD, :], sb[0:HD, :], sin_sb[0:HD, :])
            nc.vector.tensor_add(rot[0:HD, :], rot[0:HD, :], sb[0:HD, :])

        def emit_vaug(h):
            # v transposes -> vaug; emitted as soon as the v tile's
            # activation exists so the DVE copies never crowd the endgame
            for g in range(2):  # groups of 4 key tiles
                tp = psum.tile([128, 1024], bf16, tag=f"tp{g % 2}",
                               name="tp")
                for x in range(4):
                    ti = 4 * g + x
                    nc.tensor.transpose(
                        tp[:, x * HD:(x + 1) * HD],
                        qkv_sb[v_tile[h]][0:HD, ti * 128:(ti + 1) * 128],
                        ident[0:HD, 0:HD])
                dst = vaug_sb[h].rearrange("p (t c) -> p t c", c=VW)
                nc.scalar.activation(
                    dst[:, 4 * g:4 * g + 4, 0:HD],
                    tp[:, 0:4 * HD].rearrange("p (t c) -> p t c",
                                              c=HD)[:, 0:4, :], AF.Identity)

        def emit_attn(h, weave=None, po_tag="po"):
            # ---- scores -> exp -> PV over 16 (key tile, half) units ----
            # PV lags one unit so exp latency is hidden; the woven qkv
            # j-tile's matmuls fill the remaining PE slack.
            qT = rot_sb[("q", h)]
            kT = rot_sb[("k", h)]
            po = psum.tile([128, S], f32, tag=po_tag, name="po")
            if weave is not None:
                wv_ps = psum.tile([128, S], f32, tag="wv", name="wv")
                wv_mm = [(c, k) for c in (0, 512) for k in range(NK)]
                wv_done = 0
            units = [(ti, c) for ti in range(NTT) for c in (0, 512)]
            pend = {}

            def emit_pv(u):
                pt, ti, c = pend.pop(u)
                nc.tensor.matmul(
                    po[0:VW, c:c + 512],
                    vaug_sb[h][:, ti * VW:(ti + 1) * VW],
                    pt[:, :],
                    start=(ti == 0), stop=(ti == NTT - 1))

            for u, (ti, c) in enumerate(units):
                st = psum.tile([128, 512], f32, tag=f"st{u % 2}", name="st")
                nc.tensor.matmul(st[:, :], kT[0:HD, ti * 128:(ti + 1) * 128],
                                 qT[0:HD, c:c + 512], start=True, stop=True)
                pt = pt_pool.tile([128, 512], bf16, tag="pt", name="pt")
                nc.scalar.activation(pt[:, :], st[:, :], AF.Exp)
                pend[u] = (pt, ti, c)
                if weave is not None:
                    take = 2 if u % 4 == 0 else 1
                    for cc, k in wv_mm[wv_done:wv_done + take]:
                        nc.tensor.matmul(
                            wv_ps[:, cc:cc + 512],
                            wj_tiles[weave][:, k * 128:(k + 1) * 128],
                            hid_of(k)[:, cc:cc + 512],
                            start=(k == 0), stop=(k == NK - 1))
                    wv_done += take
                if u >= 1:
                    emit_pv(u - 1)
            if weave is not None and wv_done < len(wv_mm):
                for cc, k in wv_mm[wv_done:]:
                    nc.tensor.matmul(
                        wv_ps[:, cc:cc + 512],
                        wj_tiles[weave][:, k * 128:(k + 1) * 128],
                        hid_of(k)[:, cc:cc + 512],
                        start=(k == 0), stop=(k == NK - 1))
            emit_pv(len(units) - 1)
            if weave is not None:
                if bias_zero:
                    nc.vector.tensor_copy(qkv_sb[weave][:, :], wv_ps[:, :])
                else:
                    nc.scalar.activation(qkv_sb[weave][:, :], wv_ps[:, :],
                                         AF.Identity,
                                         bias=bias_sb[:, weave:weave + 1])

            # ---- normalize: recip straight off PSUM row 96, row shift
            # on the ACT queue (lands right after this head's exps) ----
            rc = nrm_pool.tile([128, S], f32, tag="rc", name="rc")
            nc.vector.reciprocal(rc[96:97, :], po[96:97, :])
            nc.gpsimd.dma_start(rc[0:1, :], rc[96:97, :])
            bc = nrm_pool.tile([128, S], f32, tag="bc", name="bc")
            nc.gpsimd.partition_broadcast(bc[0:HD, :], rc[0:1, :])
            # the last head lands directly in the packed proj tile (rows
            # 0:80 of pk4) so proj is gated only by this normalize, not by
            # an extra repack DMA
            dst = pk_sb[4][0:HD, :] if h == n_h - 1 else attn_sb[h][0:HD, :]
            nc.vector.tensor_mul(dst, po[0:HD, :], bc[0:HD, :])

        def emit_repack(h):
            # dense proj k-tiles; emitted post-loop so these DMAs never
            # head-of-line-block the weight stream on the sync queue.
            # proj-row map: heads 0-5 at 80h; h6 split 480:512 + 592:640;
            # h7 occupies 512:592 (written in place by its normalize).
            if h == n_h - 1:
                return
            if h == 6:
                spans = [(480, 0, 32), (592, 32, 48)]
            else:
                spans = [(HD * h, 0, HD)]
            for g0, off, ln in spans:
                while ln > 0:
                    t, r = g0 // 128, g0 % 128
                    n = min(128 - r, ln)
                    nc.sync.dma_start(pk_sb[t][r:r + n, :],
                                      attn_sb[h][off:off + n, :])
                    g0 += n
                    off += n
                    ln -= n

        # ------------ phase 1: qkv + interleaved per-head attention ----
        with ExitStack() as p1:
            hid_pool = p1.enter_context(tc.tile_pool(name="hid", bufs=1))
            w_pool = p1.enter_context(tc.tile_pool(name="wstream", bufs=3))
            # hid loaded in k-pairs (halves the DMA count)
            hid_pairs = [hid_pool.tile([128, 2 * S], bf16, tag=f"hid{p}",
                                       name=f"hid{p}") for p in range(NK // 2)]
            hid_of = lambda k: hid_pairs[k // 2][:, (k % 2) * S:
                                                 (k % 2) * S + S]
            wj_tiles = {}

            def load_wj(j):
                # j-pair granularity: one DMA covers tiles j, j+1
                if j in wj_tiles:
                    return
                j0 = j - j % 2
                wp2 = w_pool.tile([128, 2 * NK * 128], bf16, tag="wj",
                                  name=f"wj{j0}")
                nn = min(2, NJ - j0)
                nc.sync.dma_start(
                    wp2[:, 0:nn * NK * 128],
                    wqkvT[:, j0 * NK * 128:(j0 + nn) * NK * 128])
                for jj in range(j0, j0 + nn):
                    wj_tiles[jj] = wp2[:, (jj - j0) * NK * 128:
                                       (jj - j0 + 1) * NK * 128]

            # DMA order on the sync queue: hid k0, wj0, hid k1, wj1 — the
            # fused j0/j1 pair consumes hid tiles as they arrive.
            wp01 = w_pool.tile([128, 2 * NK * 128], bf16, tag="wj",
                               name="wj0")
            nc.scalar.dma_start(wp01[:, 0:NK * 128], wqkvT[:, 0:NK * 128])
            nc.sync.dma_start(hid_pairs[0][:, 0:S], hiddenT[:, 0:S])
            nc.scalar.dma_start(wp01[:, NK * 128:], wqkvT[:, NK * 128:
                                                          2 * NK * 128])
            nc.sync.dma_start(hid_pairs[0][:, S:2 * S], hiddenT[:, S:2 * S])
            wj_tiles[0] = wp01[:, 0:NK * 128]
            wj_tiles[1] = wp01[:, NK * 128:2 * NK * 128]
            for p in range(1, NK // 2):
                nc.sync.dma_start(hid_pairs[p][:],
                                  hiddenT[:, 2 * p * S:(2 * p + 2) * S])
            load_wj(2)
            nc.sync.dma_start(bias_sb[:], bias2d[:])
            nc.sync.dma_start(cos_sb[0:HD, :], cosP[:])
            nc.sync.dma_start(sin_sb[0:HD, :], sinP[:])
            nc.sync.dma_start(vaug_all[:], vinit[:])

            def after_j(j):
                if j + 1 < NJ and (j + 1) not in wj_tiles:
                    load_wj(j + 1)
                if j == 10:
                    for t in range(5):
                        nc.sync.dma_start(wp_sb[t][:],
                                          wprojT[t * 128:(t + 1) * 128, :])
                for h in vaug_at[j]:
                    emit_vaug(h)
                for h, sec in rope_at[j]:
                    emit_rope(h, sec, late=(j >= 13))

            # fused j0/j1: k-outer so the PE starts as soon as hid0 lands
            ps0 = psum.tile([128, S], f32, tag="wv", name="ps0")
            ps1 = psum.tile([128, S], f32, tag="po", name="ps1")
            for k in range(NK):
                for ps, j in ((ps0, 0), (ps1, 1)):
                    for c in (0, 512):
                        nc.tensor.matmul(
                            ps[:, c:c + 512],
                            wj_tiles[j][:, k * 128:(k + 1) * 128],
                            hid_of(k)[:, c:c + 512],
                            start=(k == 0), stop=(k == NK - 1))
            for ps, j in ((ps0, 0), (ps1, 1)):
                if bias_zero:
                    nc.vector.tensor_copy(qkv_sb[j][:, :], ps[:, :])
                else:
                    nc.scalar.activation(qkv_sb[j][:, :], ps[:, :],
                                         AF.Identity,
                                         bias=bias_sb[:, j:j + 1])
                after_j(j)

            # attention blocks due at `ready+2` consume (weave) that j's
            # matmuls; j14 stays plain so act14 lands before the last ropes
            emitted = set()
            j = 2
            while j < NJ:
                due_now = [h for h in range(n_h) if h not in emitted
                           and min(ready[h] + 2, NJ - 1) <= j]
                if due_now and j < NJ - 1:
                    h = due_now[0]
                    if j not in wj_tiles:
                        load_wj(j)
                    emit_attn(h, weave=j)
                    emitted.add(h)
                    after_j(j)
                    j += 1
                    continue
                if j not in wj_tiles:
                    load_wj(j)
                sts = [psum.tile([128, 512], f32, tag=f"st{i}", name="qs")
                       for i in range(2)]
                for ci, c in enumerate((0, 512)):
                    for k in range(NK):
                        nc.tensor.matmul(
                            sts[ci][:, :],
                            wj_tiles[j][:, k * 128:(k + 1) * 128],
                            hid_of(k)[:, c:c + 512],
                            start=(k == 0), stop=(k == NK - 1))
                    if bias_zero:
                        nc.vector.tensor_copy(qkv_sb[j][:, c:c + 512],
                                              sts[ci][:, :])
                    else:
                        nc.scalar.activation(qkv_sb[j][:, c:c + 512],
                                             sts[ci][:, :], AF.Identity,
                                             bias=bias_sb[:, j:j + 1])
                after_j(j)
                j += 1
        # post-loop attention/repack sits OUTSIDE the p1 pools' scope: the
        # pool-exit engine drains must not wait on the attention tail
        unemitted = [h for h in range(n_h) if h not in emitted]
        for i, h in enumerate(unemitted):
            if h in vaug_deferred:
                emit_vaug(h)   # after the previous attn block so the
                # in-order PE isn't blocked waiting on the last act
            emit_attn(h, po_tag="wv" if i % 2 == 0 else "po")
        for h in range(n_h):
            emit_repack(h)

        if KERNEL_DEBUG:
            for j in range(NJ):
                nc.sync.dma_start(dbg_qkv[:, j * S:(j + 1) * S],
                                    qkv_sb[j][:])
            i_ = 0
            for h in range(n_h):
                for sec in ("q", "k"):
                    nc.sync.dma_start(dbg_rot[:, i_ * S:(i_ + 1) * S],
                                        rot_sb[(sec, h)][:])
                    i_ += 1
            for h in range(n_h):
                nc.sync.dma_start(dbg_attn[:, h * S:(h + 1) * S],
                                    attn_sb[h][:])

        # ------------ phase 2: projection ---------------------------
        with ExitStack() as p5:
            out_pool = p5.enter_context(tc.tile_pool(name="outsb", bufs=1))
            wv_sl = psum.tile([128, S], f32, tag="wv", name="pjwv")
            po_sl = psum.tile([128, S], f32, tag="po", name="pjpo")
            slots = []

            def slot(i):
                i = i % 8
                if i < 4:
                    return psum.tile([128, 512], f32,
                                     tag=["st0", "st1", "tp0", "tp1"][i],
                                     name="pj")
                if i < 6:
                    return wv_sl[:, (i - 4) * 512:(i - 3) * 512]
                return po_sl[:, (i - 6) * 512:(i - 5) * 512]

            chains = [(j, c) for j in range(D // 128) for c in (0, 512)]
            slot_of = {}

            def open_partA(u):
                j, c = chains[u]
                ps = slot(u)
                slot_of[u] = ps
                for kt in range(4):
                    nc.tensor.matmul(ps[:, 0:512],
                                     wp_sb[kt][:, j * 128:(j + 1) * 128],
                                     pk_sb[kt][:, c:c + 512],
                                     start=(kt == 0), stop=False)

            for u in range(8):
                open_partA(u)
            for u, (j, c) in enumerate(chains):
                if c == 0:
                    ob = out_pool.tile([128, S], bf16, tag=f"ob{j % 4}",
                                       name="ob")
                ps = slot_of.pop(u)
                nc.tensor.matmul(ps[:, 0:512],
                                 wp_sb[4][:, j * 128:(j + 1) * 128],
                                 pk_sb[4][:, c:c + 512],
                                 start=False, stop=True)
                nc.scalar.activation(ob[:, c:c + 512], ps[:, 0:512],
                                     AF.Identity)
                if u + 8 < len(chains):
                    open_partA(u + 8)
                if c == 512:
                    eng = (nc.sync, nc.gpsimd)[j % 2]
                    eng.dma_start(outT[j * 128:(j + 1) * 128, :], ob[:, :])

    nc.compile()
    return nc


def _pack_w_a2(Wqkv, bqkv, heads):
    """Dense 15-tile per-head-contiguous packing (see _layout_a2)."""
    import ml_dtypes
    v_tile, q_rows, k_rows, _ = _layout_a2()
    perm = np.zeros((NJ * 128,), np.int64)
    scl = np.ones((NJ * 128,), np.float32)
    used = np.zeros((NJ * 128,), bool)
    for i, h in enumerate(heads):
        for d in range(HD):
            g = 128 * v_tile[i] + d
            perm[g] = 2 * D + h * HD + d  # v
            used[g] = True
        for d, g in enumerate(q_rows[i]):
            perm[g] = h * HD + d
            scl[g] = SCALE
            used[g] = True
        for d, g in enumerate(k_rows[i]):
            perm[g] = D + h * HD + d
            used[g] = True
    W = Wqkv[perm] * scl[:, None]
    W[~used] = 0.0
    b = bqkv[perm] * scl
    b[~used] = 0.0
    # wqkvT host layout: [128, j, k, 128]; [p, j, k, m] = W.T[k*128+p, j*128+m]
    WT = np.ascontiguousarray(W.T)  # [1280, 1920]
    wt = WT.reshape(NK, 128, NJ, 128).transpose(1, 2, 0, 3)
    wt = np.ascontiguousarray(wt.reshape(128, NJ * NK * 128))
    bias2d = np.ascontiguousarray(b.reshape(NJ, 128).T)
    return wt.astype(ml_dtypes.bfloat16), bias2d


def _pack_wproj(Wproj, heads):
    """Rows of Wproj.T for this core's head dims, stacked per head."""
    W = np.zeros((len(heads) * HD, Wproj.shape[0]), np.float32)
    for i, h in enumerate(heads):
        W[i * HD:(i + 1) * HD] = Wproj[:, h * HD:(h + 1) * HD].T
    return W


def _pack_wproj_a2(Wproj, heads):
    """Mode A proj rows match the device pk layout: heads 0-5 at 80h,
    h6 split 480:512 (d0:32) + 592:640 (d32:80), h7 at 512:592."""
    W = np.zeros((640, Wproj.shape[0]), np.float32)
    wt = lambda h, d0, d1: Wproj[:, heads[h] * HD + d0:
                                 heads[h] * HD + d1].T
    for i in range(6):
        W[i * HD:(i + 1) * HD] = wt(i, 0, HD)
    W[480:512] = wt(6, 0, 32)
    W[512:592] = wt(7, 0, HD)
    W[592:640] = wt(6, 32, HD)
    return W


_CACHE = {}


def _kernel_mode_a(hidden_states, cos, sin, Wqkv, bqkv, Wproj, bproj, S):
    import ml_dtypes
    from concourse import bass_utils

    n_h, S_core = H // 2, S // 4
    bz = not np.any(bqkv)
    key = ("A2", bz)
    if key not in _CACHE:
        _CACHE[key] = _build_program_a2(bias_zero=bz)
    nc = _CACHE[key]

    bf = ml_dtypes.bfloat16
    hiddenT = np.ascontiguousarray(hidden_states.T)  # [D, S]

    vinit = np.zeros((128, 8 * NTT, VW), np.float32)
    vinit[:, :, 96] = 1.0
    vinit = np.ascontiguousarray(vinit.reshape(128, 8 * NTT * VW)).astype(bf)

    in_maps = []
    meta = []
    for g in range(2):
        heads = list(range(g * n_h, (g + 1) * n_h))
        wt, b2 = _pack_w_a2(Wqkv, bqkv, heads)
        wprojT = _pack_wproj_a2(Wproj, heads).astype(bf)
        for s in range(4):
            sl = slice(s * S_core, (s + 1) * S_core)
            hseg = hiddenT[:, sl]  # [1280, 1024]
            hid_t = np.ascontiguousarray(
                hseg.reshape(NK, 128, S_core).transpose(1, 0, 2)
                .reshape(128, NK * S_core)).astype(bf)
            cosP = np.ascontiguousarray(cos[sl].T).astype(bf)
            sinP = np.concatenate(
                [-sin[sl].T[0:BLK], sin[sl].T[BLK:HD]], axis=0)
            sinP = np.ascontiguousarray(sinP).astype(bf)
            in_maps.append({
                "hiddenT": hid_t,
                "wqkvT": wt,
                "bias2d": b2,
                "cosP": cosP,
                "sinP": sinP,
                "wprojT": wprojT,
                "vinit": vinit,
            })
            meta.append((g, s))
    res = bass_utils.run_bass_kernel_spmd(nc, in_maps,
                                          core_ids=list(range(N_CORES)))
    out = np.zeros((D, S), np.float32)
    for c, (g, s) in enumerate(meta):
        out[:, s * S_core:(s + 1) * S_core] += \
            res.results[c]["outT"].astype(np.float32)
    return out


# ---------------------------------------------------------------------------
# mode C (non-uniform segments): fp32r 8-way head-parallel fallback
# ---------------------------------------------------------------------------

def _pack_layout(n_h):
    """Pack per-core qkv dims as 40-row blocks, 3 per 128-row tile (8 pad).

    Each tile holds one v-block at row 0 (PE transpose operands must start
    at a 32-aligned partition) and two q/k blocks at rows 40 and 80.
    Returns pos[(sec, h, half)] = (tile, row) and the number of tiles.
    """
    ntiles = 2 * n_h
    pos = {}
    for h in range(n_h):
        for half in (0, 1):
            pos[("v", h, half)] = (2 * h + half, 0)
    qk = [("q", h, half) for h in range(n_h) for half in (0, 1)]
    qk += [("k", h, half) for h in range(n_h) for half in (0, 1)]
    for j, blk in enumerate(qk):
        pos[blk] = (j // 2, BLK + BLK * (j % 2))
    return pos, ntiles


def _build_program(n_h, S_core, segs_local, resident_hidden):
    """Mode C SPMD program (fp32r)."""
    import concourse.mybir as mybir
    import concourse.tile as tile
    from concourse import bacc
    from concourse.masks import make_identity
    from contextlib import ExitStack

    f32 = mybir.dt.float32
    mm_dt = getattr(mybir.dt, MM_DT_NAME)
    AF = mybir.ActivationFunctionType

    k_proj = n_h
    pos, n_mtiles = _pack_layout(n_h)
    dims_pad = n_mtiles * 128
    VWc = 97

    t_tiles = []
    for si, (a, e) in enumerate(segs_local):
        t = a
        while t < e:
            t_tiles.append((si, t, min(t + 128, e)))
            t += 128
    n_tt = len(t_tiles)

    nc = bacc.Bacc("TRN2", target_bir_lowering=False, debug=False,
                   enable_asserts=False, num_devices=N_CORES)

    hiddenT = nc.dram_tensor("hiddenT", [128, NK * S_core], mm_dt,
                             kind="ExternalInput").ap()
    wqkvT = nc.dram_tensor("wqkvT", [128, NK * dims_pad], mm_dt,
                           kind="ExternalInput").ap()
    bias2d = nc.dram_tensor("bias2d", [128, n_mtiles], f32,
                            kind="ExternalInput").ap()
    cosP = nc.dram_tensor("cosP", [128, S_core], mm_dt,
                          kind="ExternalInput").ap()
    sin2P = nc.dram_tensor("sin2P", [128, S_core], mm_dt,
                           kind="ExternalInput").ap()
    wprojT = nc.dram_tensor("wprojT", [n_h * HD, D], mm_dt,
                            kind="ExternalInput").ap()
    vinit = nc.dram_tensor("vinit", [128, n_tt * (VWc - HD)], mm_dt,
                           kind="ExternalInput").ap()
    outT = nc.dram_tensor("outT", [D, S_core], f32, kind="ExternalOutput").ap()

    def r_(ap):
        return ap.bitcast(mm_dt)

    BC = 1024
    big_chunks = [(c, min(c + BC, S_core)) for c in range(0, S_core, BC)]

    def halves(c0, c1):
        out = []
        q = c0
        while q < c1:
            out.append((q, min(q + 512, c1)))
            q = q + 512
        return out

    with tile.TileContext(nc) as tc, ExitStack() as ctx:
        persist = ctx.enter_context(tc.tile_pool(name="persist", bufs=1))
        ident = persist.tile([128, 128], f32, tag="ident", name="ident")
        make_identity(nc, ident[:])
        bias_sb = persist.tile([128, n_mtiles], f32, tag="bias", name="bias")
        nc.sync.dma_start(bias_sb[:], bias2d[:])

        psum_all_cm = tc.tile_pool(name="psum_all", bufs=1, space="PSUM")
        psum_all = psum_all_cm.__enter__()
        qkv_pool = ctx.enter_context(tc.tile_pool(name="big", bufs=1))
        qkv_sb = [qkv_pool.tile([128, S_core], mm_dt, tag=f"qkvT{j}",
                                name=f"qkvT{j}") for j in range(n_mtiles)]
        rot_cm = tc.tile_pool(name="rot", bufs=1)
        rv = rot_cm.__enter__()
        rot_sb = {}
        for h in range(n_h):
            for sec in ("q", "k"):
                rot_sb[(sec, h)] = rv.tile([128, S_core], mm_dt,
                                           tag=f"rot_{sec}{h}",
                                           name=f"rot_{sec}{h}")
        RC = 1024
        rope_cm = tc.tile_pool(name="rope_scr", bufs=2)
        rope_scr = rope_cm.__enter__()

        with ExitStack() as p1:
            hidden3 = hiddenT.rearrange("p (k s) -> p k s", k=NK)
            w3 = wqkvT.rearrange("p (k m) -> p k m", k=NK)
            w_pool = p1.enter_context(tc.tile_pool(name="wres", bufs=1))
            w_sb = [w_pool.tile([128, dims_pad], mm_dt, tag=f"w{k}",
                                name=f"w{k}") for k in range(NK)]
            for k in range(NK):
                nc.sync.dma_start(w_sb[k][:], w3[:, k, :])
            hid_pool = p1.enter_context(tc.tile_pool(name="hidstream",
                                                     bufs=3))
            n4 = n_mtiles // 4
            for (h0, h1) in halves(0, S_core):
                hw = h1 - h0
                for q4 in range(n4):
                    ps01 = psum_all.tile([128, BC], f32, tag="t0",
                                         name="ps01")
                    ps23 = psum_all.tile([128, BC], f32, tag="t1",
                                         name="ps23")
                    pj_of = lambda j: (ps01 if j % 4 < 2 else ps23,
                                       (j % 2) * 512)
                    for k in range(NK):
                        ht = hid_pool.tile([128, 512], mm_dt, tag="hidc",
                                           name="hidc")
                        nc.sync.dma_start(ht[:, :hw], hidden3[:, k, h0:h1])
                        for j in range(q4 * 4, q4 * 4 + 4):
                            psj, co = pj_of(j)
                            nc.tensor.matmul(
                                psj[:, co:co + hw],
                                r_(w_sb[k][:, j * 128:(j + 1) * 128]),
                                r_(ht[:, :hw]),
                                start=(k == 0), stop=(k == NK - 1))
                    for j in range(q4 * 4, q4 * 4 + 4):
                        psj, co = pj_of(j)
                        nc.scalar.activation(qkv_sb[j][:, h0:h1],
                                             psj[:, co:co + hw], AF.Identity,
                                             bias=bias_sb[:, j:j + 1])

        psum_all_cm.__exit__(None, None, None)
        ps_att = ctx.enter_context(tc.tile_pool(name="ps_att", bufs=1,
                                                space="PSUM"))

        stg = {}
        for nm in ("sa0", "sa1", "sb0", "sb1"):
            stg[nm] = rope_scr.tile([128, RC], mm_dt, tag=nm, name=nm, bufs=1)
        pair_i = 0
        for ci, f0 in enumerate(range(0, S_core, RC)):
            f1 = min(f0 + RC, S_core)
            fs = f1 - f0
            cos_sb = rope_scr.tile([128, RC], mm_dt, tag="cos", name="cos",
                                   bufs=1)
            sin_sb = rope_scr.tile([128, RC], mm_dt, tag="sin", name="sin",
                                   bufs=1)
            nc.scalar.dma_start(cos_sb[:, :fs], cosP[:, f0:f1])
            nc.scalar.dma_start(sin_sb[:, :fs], sin2P[:, f0:f1])
            if ci == 0:
                for nm in stg:
                    nc.scalar.dma_start(stg[nm][BLK:64, :], cos_sb[BLK:64, :])
            for h in range(n_h):
                for sec in ("q", "k"):
                    lo_t, lo_r = pos[(sec, h, 0)]
                    hi_t, hi_r = pos[(sec, h, 1)]
                    x = qkv_sb[lo_t]
                    dst = rot_sb[(sec, h)]
                    stga = stg[f"sa{pair_i % 2}"]
                    stgb = stg[f"sb{pair_i % 2}"]
                    nc.scalar.dma_start(stga[0:BLK, :fs],
                                        x[lo_r:lo_r + BLK, f0:f1])
                    nc.scalar.dma_start(stga[64:64 + BLK, :fs],
                                        x[hi_r:hi_r + BLK, f0:f1])
                    nc.scalar.dma_start(stgb[0:BLK, :fs],
                                        x[hi_r:hi_r + BLK, f0:f1])
                    nc.scalar.dma_start(stgb[64:64 + BLK, :fs],
                                        x[lo_r:lo_r + BLK, f0:f1])
                    nc.vector.tensor_mul(dst[0:104, f0:f1], stga[0:104, :fs],
                                         cos_sb[0:104, :fs])
                    eng = nc.gpsimd if pair_i % 2 == 0 else nc.vector
                    eng.tensor_mul(stgb[0:104, :fs], stgb[0:104, :fs],
                                   sin_sb[0:104, :fs])
                    nc.vector.tensor_add(dst[0:104, f0:f1], dst[0:104, f0:f1],
                                         stgb[0:104, :fs])
                    pair_i += 1
        rope_cm.__exit__(None, None, None)

        vaug_cm = tc.tile_pool(name="vaug", bufs=1)
        vaug_pool = vaug_cm.__enter__()
        vaug_sb = [vaug_pool.tile([128, n_tt * VWc], mm_dt, tag=f"vaug{h}",
                                  name=f"vaug{h}") for h in range(n_h)]
        vinit3 = vinit.rearrange("p (t c) -> p t c", c=VWc - HD)
        for h in range(n_h):
            nc.sync.dma_start(
                vaug_sb[h].rearrange("p (t c) -> p t c", c=VWc)[:, :, HD:VWc],
                vinit3[:, :, :])
        GRP = 4

        def emit_vaug(h):
            gi = 0
            while gi < n_tt:
                hi_g = min(gi + GRP, n_tt)
                if all(t_tiles[g][2] - t_tiles[g][1] == 128
                       for g in range(gi, hi_g)):
                    grp = list(range(gi, hi_g))
                else:
                    grp = [gi]
                ng = len(grp)
                tp = ps_att.tile([128, GRP * HD], f32, tag="tp", name="tp")
                for x, g in enumerate(grp):
                    si, t0, t1 = t_tiles[g]
                    sz = t1 - t0
                    for half in (0, 1):
                        vt, vr = pos[("v", h, half)]
                        nc.tensor.transpose(
                            tp[:sz, x * HD + half * BLK:
                               x * HD + (half + 1) * BLK],
                            qkv_sb[vt][0:BLK, t0:t1].bitcast(f32),
                            ident[:BLK, :BLK])
                sz0 = t_tiles[grp[0]][2] - t_tiles[grp[0]][1]
                dst = vaug_sb[h].rearrange("p (t c) -> p t c", c=VWc)
                src_ap = tp.rearrange("p (t c) -> p t c", c=HD)
                nc.vector.tensor_copy(dst[:sz0, grp[0]:grp[0] + ng, 0:HD],
                                      src_ap[:sz0, 0:ng, :])
                gi += ng

        attn_sb = [qkv_pool.tile([128, S_core], mm_dt, tag=f"qkvT{hh}",
                                 name=f"attnT{hh}") for hh in range(n_h)]

        seg_ttiles = {}
        for ti, (si, t0, t1) in enumerate(t_tiles):
            seg_ttiles.setdefault(si, []).append((ti, t0, t1))

        BA = 512
        with ExitStack() as p4:
            pt_pool = p4.enter_context(tc.tile_pool(name="pt", bufs=3))
            nrm_pool = p4.enter_context(tc.tile_pool(name="nrm", bufs=2))
            unit_box = [0]

            def emit_attention(h, si, a, e):
                qT = rot_sb[("q", h)]
                kT = rot_sb[("k", h)]
                q = a
                while q < e:
                    q0, q1 = q, min(q + BA, e)
                    qs = q1 - q0
                    po = ps_att.tile([128, BA], f32,
                                     tag=f"po{unit_box[0] % 2}", name="pv")
                    tts = seg_ttiles[si]
                    for idx, (ti, t0, t1) in enumerate(tts):
                        sz = t1 - t0
                        ps = ps_att.tile([128, BA], f32, tag=f"st{idx % 2}",
                                         name="st")
                        nc.tensor.matmul(ps[:sz, :qs], r_(kT[0:104, t0:t1]),
                                         r_(qT[0:104, q0:q1]),
                                         start=True, stop=True)
                        pt = pt_pool.tile([128, BA], mm_dt, tag="pt",
                                          name="pt")
                        nc.scalar.activation(pt[:sz, :qs], ps[:sz, :qs],
                                             AF.Exp)
                        nc.tensor.matmul(
                            po[:VWc, :qs],
                            r_(vaug_sb[h][:sz, ti * VWc:(ti + 1) * VWc]),
                            r_(pt[:sz, :qs]),
                            start=(idx == 0), stop=(idx == len(tts) - 1))
                    rc = nrm_pool.tile([128, BA], f32, tag="rc", name="rc")
                    nc.vector.tensor_copy(rc[96:97, :qs], po[96:97, :qs])
                    nc.sync.dma_start(rc[0:1, :qs], rc[96:97, :qs])
                    nc.vector.reciprocal(rc[0:1, :qs], rc[0:1, :qs])
                    bc = nrm_pool.tile([128, BA], mm_dt, tag="bc", name="bc")
                    nc.gpsimd.partition_broadcast(
                        bc[0:HD, :qs], rc[0:1, :qs].bitcast(mm_dt))
                    nc.vector.tensor_mul(attn_sb[h][0:HD, q0:q1],
                                         po[0:HD, :qs], bc[0:HD, :qs])
                    unit_box[0] += 1
                    q = q1

            for h in range(n_h):
                emit_vaug(h)
            for si, (a, e) in enumerate(segs_local):
                for h in range(n_h):
                    emit_attention(h, si, a, e)

        vaug_cm.__exit__(None, None, None)
        rot_cm.__exit__(None, None, None)

        with ExitStack() as p5:
            wp_pool = p5.enter_context(tc.tile_pool(name="wp", bufs=1))
            wp_sb = []
            for kt in range(k_proj):
                t = wp_pool.tile([HD, D], mm_dt, tag=f"wp{kt}", name=f"wp{kt}")
                nc.sync.dma_start(t[:], wprojT[kt * HD:(kt + 1) * HD, :])
                wp_sb.append(t)
            out_pool = p5.enter_context(tc.tile_pool(name="outsb", bufs=3))
            for (c0, c1) in big_chunks:
                cs = c1 - c0
                for j in range(D // 128):
                    ob = out_pool.tile([128, BC], f32, tag="ob", name="ob")
                    for (h0, h1) in halves(c0, c1):
                        ps = ps_att.tile([128, 512], f32, tag=f"st{j % 2}",
                                         name="pj")
                        for kt in range(k_proj):
                            nc.tensor.matmul(
                                ps[:, :h1 - h0],
                                r_(wp_sb[kt][:, j * 128:(j + 1) * 128]),
                                r_(attn_sb[kt][0:HD, h0:h1]),
                                start=(kt == 0), stop=(kt == k_proj - 1))
                        if j % 2 == 0:
                            nc.vector.tensor_copy(ob[:, h0 - c0:h1 - c0],
                                                  ps[:, :h1 - h0])
                        else:
                            nc.scalar.activation(ob[:, h0 - c0:h1 - c0],
                                                 ps[:, :h1 - h0], AF.Identity)
                    nc.sync.dma_start(outT[j * 128:(j + 1) * 128, c0:c1],
                                      ob[:, :cs])

    nc.compile()
    return nc


def _pack_w(Wqkv, bqkv, heads, n_h):
    """Mode C packed qkv weights (q rows pre-scaled)."""
    pos, n_mtiles = _pack_layout(n_h)
    dims_pad = n_mtiles * 128
    W = np.zeros((dims_pad, D), np.float32)
    b = np.zeros((dims_pad,), np.float32)
    sec_off = {"q": 0, "k": D, "v": 2 * D}
    for i, h in enumerate(heads):
        for sec in ("q", "k", "v"):
            for half in (0, 1):
                t, r = pos[(sec, i, half)]
                src = sec_off[sec] + h * HD + half * BLK
                w = Wqkv[src:src + BLK, :]
                bb = bqkv[src:src + BLK]
                if sec == "q":
                    w = w * SCALE
                    bb = bb * SCALE
                W[t * 128 + r:t * 128 + r + BLK] = w
                b[t * 128 + r:t * 128 + r + BLK] = bb
    w_tiled = _tile_rows(np.ascontiguousarray(W.T))
    bias2d = np.ascontiguousarray(b.reshape(n_mtiles, 128).T)
    return w_tiled, bias2d


def _tile_rows(x):
    """[R, C] with R = nk*128 -> [128, nk*C] k-major tiling."""
    R, C = x.shape
    nk = R // 128
    return np.ascontiguousarray(
        x.reshape(nk, 128, C).transpose(1, 0, 2).reshape(128, nk * C))


def _pack_cos_sin(cos, sin):
    """Mode C cosP/sin2P [128, S]."""
    S = cos.shape[0]
    cosP = np.zeros((128, S), np.float32)
    sinP = np.zeros((128, S), np.float32)
    cosP[0:BLK] = cos.T[0:BLK]
    cosP[64:64 + BLK] = cos.T[BLK:HD]
    sinP[0:BLK] = -sin.T[0:BLK]
    sinP[64:64 + BLK] = sin.T[BLK:HD]
    return cosP, sinP


def kernel(hidden_states, cos, sin, Wqkv, bqkv, Wproj, bproj, cu_seqlens):
    sys.path.insert(0, "/opt/trn_rl_repo")
    from concourse import bass_utils

    hidden_states = np.asarray(hidden_states, np.float32)
    cos = np.asarray(cos, np.float32)
    sin = np.asarray(sin, np.float32)
    Wqkv = np.asarray(Wqkv, np.float32)
    bqkv = np.asarray(bqkv, np.float32)
    Wproj = np.asarray(Wproj, np.float32)
    bproj = np.asarray(bproj, np.float32)

    S, D_ = hidden_states.shape
    assert D_ == D
    segs = _segments(cu_seqlens, S)
    uniform = (S == 4096) and segs == [(i * S // 4, (i + 1) * S // 4)
                                       for i in range(4)]

    if uniform:
        out = _kernel_mode_a(hidden_states, cos, sin, Wqkv, bqkv, Wproj,
                             bproj, S)
    else:
        hiddenT = np.ascontiguousarray(hidden_states.T)
        cosP, sin2P = _pack_cos_sin(cos, sin)
        n_h, S_core = H // N_CORES, S
        key = ("C", S, tuple(np.asarray(cu_seqlens).tolist()))
        if key not in _CACHE:
            _CACHE[key] = _build_program(n_h, S_core, segs,
                                         resident_hidden=False)
        nc = _CACHE[key]
        n_tt = sum(-(-(e - a) // 128) for a, e in segs)
        vinit = np.zeros((128, n_tt, 17), np.float32)
        vinit[:, :, 16] = 1.0
        vinit = np.ascontiguousarray(vinit.reshape(128, n_tt * 17))
        hid_tiled = _tile_rows(hiddenT)
        in_maps = []
        for c in range(N_CORES):
            heads = list(range(c * n_h, (c + 1) * n_h))
            wt, b2 = _pack_w(Wqkv, bqkv, heads, n_h)
            in_maps.append({
                "hiddenT": hid_tiled,
                "wqkvT": wt,
                "bias2d": b2,
                "cosP": cosP,
                "sin2P": sin2P,
                "wprojT": _pack_wproj(Wproj, heads).astype(np.float32),
                "vinit": vinit,
            })
        res = bass_utils.run_bass_kernel_spmd(nc, in_maps,
                                              core_ids=list(range(N_CORES)))
        out = np.zeros((D, S), np.float32)
        for c in range(N_CORES):
            out += res.results[c]["outT"]

    return np.ascontiguousarray(out.T) + bproj[None, :]


# revision 38
# speedup vs baseline: 1.3676x; 1.0038x over previous
"""Trainium2 Bass kernel for Ernie4.5-VL vision attention (ragged segments).

Contract: kernel(**inputs) takes the FULL unsharded inputs (keyed as in
setup_inputs()) and returns the FULL [S, D] float32 output.

Mode A (uniform 4x1024 segments — the graded shape): 8 cores = 2 head
groups x 4 segments; per core 8 heads x 1024 tokens, everything in bf16
on the PE array (psum f32):

  qkvT = Wpack @ hidden.T     15 dense 128-row tiles (v 80-row blocks at
                              tile h rows 0:80, q/k packed tile-major)
  rope: dense [0:80] layout; the rotate-half operand is built with 2-4
        small SBUF DMAs per (q|k, head); rot = a*cos + b*sin on DVE/Pool
  per head: v transposes (PE) -> scoresT (PE) -> exp (ACT, 1024 wide)
        -> PV accumulate with ones column for the denominator ->
        reciprocal+broadcast+mul normalize
  attn heads DMA-repacked into 5 dense 128-row tiles; proj = 5 k-tiles
  Host does O(S*D) glue: packing, summing the 2 per-token partial
  projections, bias adds.

Engine budget per core (cost model): PE ~142us of matmul rows, ACT
~82us (exp + qkv bias copies), DVE ~40us, Pool ~30us, DMA ~19MB.
Emission interleaves attention per head into the qkv j-loop so every
engine streams; all DMAs avoid the ACT queue (exp lives there).

Mode C fallback (any other cu_seqlens): 8-way head parallel fp32r path
(unchanged from the earlier version of this kernel).
"""

import os
import sys

import numpy as np

H = 16
HD = 80
BLK = 40  # rotate_half half-width
SCALE = HD ** -0.5
N_CORES = 8
D = 1280
NK = D // 128  # contraction tiles for the qkv matmul
ATTN_STRIDE = 96  # head row pitch in the packed attention output (mode C)
MM_DT_NAME = os.environ.get("KERNEL_MM_DT", "float32r")  # mode C only
KERNEL_DEBUG = bool(int(os.environ.get("KERNEL_DEBUG", "0")))

# ---- mode A constants ----
NJ = 15          # dense qkv M tiles (1920 rows)
NTT = 8          # 128-row key tiles per 1024 segment
VW = 97          # vaug slot: 80 v dims + 16 pad + ones col at 96
SA_CORE = 1024   # tokens per core


def _segments(cu_seqlens, S):
    """Intervals matching reference's searchsorted(cu[1:], i, 'right')."""
    b = np.clip(np.sort(np.asarray(cu_seqlens, dtype=np.int64)[1:5]), 0, S)
    bounds = [0] + list(b) + [S]
    segs = []
    for a, e in zip(bounds[:-1], bounds[1:]):
        if e > a:
            segs.append((int(a), int(e)))
    return segs


# ---------------------------------------------------------------------------
# mode A: dense bf16 program
# ---------------------------------------------------------------------------

def _layout_a2():
    """Per-head-contiguous packing: head h owns global rows [240h, 240h+240).
    v sits at rows 0:80 of tile ceil(240h/128) (PE transpose needs a
    32-aligned non-crossing 80-row read); q then k fill the remaining
    window rows in ascending order (read via DMA, placement free).

    Returns (v_tile[h], q_rows[h], k_rows[h], ready_j[h]) where
    q_rows/k_rows are the 80 global rows of each section in dim order.
    """
    v_tile, q_rows, k_rows, ready = [], [], [], []
    for h in range(8):
        w0, w1 = 240 * h, 240 * (h + 1)
        th = -(-w0 // 128)
        vg0 = 128 * th
        qk = [g for g in range(w0, w1) if not (vg0 <= g < vg0 + HD)]
        v_tile.append(th)
        q_rows.append(qk[0:HD])
        k_rows.append(qk[HD:2 * HD])
        ready.append(max(th, qk[-1] // 128))
    return v_tile, q_rows, k_rows, ready


def _row_pieces(rows):
    """Split a list of global rows into (tile, row, len, rel_off) runs that
    are consecutive and stay within one 128-row tile."""
    out = []
    i = 0
    while i < len(rows):
        g = rows[i]
        n = 1
        while (i + n < len(rows) and rows[i + n] == g + n
               and (g + n) // 128 == g // 128):
            n += 1
        out.append((g // 128, g % 128, n, i))
        i += n
    return out


def _build_program_a2(bias_zero=True):
    """Mode A program: n_h=8 heads, S=1024 tokens per core, one segment."""
    import concourse.mybir as mybir
    import concourse.tile as tile
    from concourse import bacc
    from concourse.masks import make_identity
    from contextlib import ExitStack

    f32 = mybir.dt.float32
    bf16 = mybir.dt.bfloat16
    AF = mybir.ActivationFunctionType
    n_h, S = 8, SA_CORE

    nc = bacc.Bacc("TRN2", target_bir_lowering=False, debug=False,
                   enable_asserts=False, num_devices=N_CORES)

    hiddenT = nc.dram_tensor("hiddenT", [128, NK * S], bf16,
                             kind="ExternalInput").ap()
    wqkvT = nc.dram_tensor("wqkvT", [128, NJ * NK * 128], bf16,
                           kind="ExternalInput").ap()
    bias2d = nc.dram_tensor("bias2d", [128, NJ], f32,
                            kind="ExternalInput").ap()
    cosP = nc.dram_tensor("cosP", [HD, S], bf16, kind="ExternalInput").ap()
    sinP = nc.dram_tensor("sinP", [HD, S], bf16, kind="ExternalInput").ap()
    wprojT = nc.dram_tensor("wprojT", [n_h * HD, D], bf16,
                            kind="ExternalInput").ap()
    vinit = nc.dram_tensor("vinit", [128, 8 * NTT * VW], bf16,
                           kind="ExternalInput").ap()
    outT = nc.dram_tensor("outT", [D, S], bf16, kind="ExternalOutput").ap()
    if KERNEL_DEBUG:
        dbg_qkv = nc.dram_tensor("dbg_qkv", [128, NJ * S], f32,
                                 kind="ExternalOutput").ap()
        dbg_rot = nc.dram_tensor("dbg_rot", [128, 2 * n_h * S], f32,
                                 kind="ExternalOutput").ap()
        dbg_attn = nc.dram_tensor("dbg_attn", [128, n_h * S], f32,
                                  kind="ExternalOutput").ap()

    v_tile, q_rows, k_rows, ready = _layout_a2()
    rope_at = {j: [] for j in range(NJ)}   # (h, sec) at section readiness
    vaug_at = {j: [] for j in range(NJ)}   # h at v-tile readiness
    vaug_deferred = []
    for h in range(n_h):
        for sec, rows in (("q", q_rows[h]), ("k", k_rows[h])):
            rope_at[max(g // 128 for g in rows)].append((h, sec))
        if v_tile[h] >= NJ - 1:
            vaug_deferred.append(h)
        else:
            vaug_at[v_tile[h]].append(h)

    with tile.TileContext(nc) as tc, ExitStack() as ctx:
        persist = ctx.enter_context(tc.tile_pool(name="persist", bufs=1))
        ident = persist.tile([128, 128], bf16, tag="ident", name="ident")
        make_identity(nc, ident[:])
        bias_sb = persist.tile([128, NJ], f32, tag="bias", name="bias")
        cos_sb = persist.tile([128, S], bf16, tag="cos", name="cos")
        sin_sb = persist.tile([128, S], bf16, tag="sin", name="sin")

        psum = ctx.enter_context(tc.tile_pool(name="psum", bufs=1,
                                              space="PSUM"))
        qkv_pool = ctx.enter_context(tc.tile_pool(name="qkv", bufs=1))
        qkv_sb = [qkv_pool.tile([128, S], bf16, tag=f"qkvT{j}",
                                name=f"qkvT{j}") for j in range(NJ)]
        rot_pool = ctx.enter_context(tc.tile_pool(name="rot", bufs=1))
        rot_sb = {}
        for h in range(n_h):
            for sec in ("q", "k"):
                rot_sb[(sec, h)] = rot_pool.tile(
                    [128, S], bf16, tag=f"rot_{sec}{h}", name=f"rot_{sec}{h}")
        vaug_pool = ctx.enter_context(tc.tile_pool(name="vaug", bufs=1))
        vaug_all = vaug_pool.tile([128, n_h * NTT * VW], bf16, tag="vaug",
                                  name="vaug")
        vaug_sb = [vaug_all[:, h * NTT * VW:(h + 1) * NTT * VW]
                   for h in range(n_h)]
        attn_pool = ctx.enter_context(tc.tile_pool(name="attn", bufs=1))
        attn_sb = [attn_pool.tile([128, S], bf16, tag=f"attn{h}",
                                  name=f"attn{h}") for h in range(n_h)]
        stg_pool = ctx.enter_context(tc.tile_pool(name="stg", bufs=2))
        pt_pool = ctx.enter_context(tc.tile_pool(name="pt", bufs=4))
        nrm_pool = ctx.enter_context(tc.tile_pool(name="nrm", bufs=2))
        pk_pool = ctx.enter_context(tc.tile_pool(name="pk", bufs=1))
        pk_sb = [pk_pool.tile([128, S], bf16, tag=f"pk{t}", name=f"pk{t}")
                 for t in range(5)]
        wp_pool = ctx.enter_context(tc.tile_pool(name="wp", bufs=1))
        wp_sb = [wp_pool.tile([128, D], bf16, tag=f"wp{t}", name=f"wp{t}")
                 for t in range(5)]


        def emit_rope(h, sec, late=False):
            # sa (x in dim order) staged from qkv tiles on the SP queue;
            # sb = rotate_half(sa) built from sa with exactly two Pool
            # (SWDGE) DMAs. DMA count is precious: each dma_start holds its
            # queue for wait+transfer+sem (~1.1us fixed).
            rows = q_rows[h] if sec == "q" else k_rows[h]
            sa = stg_pool.tile([128, S], bf16, tag="sa", name=f"sa_{sec}{h}")
            sb = stg_pool.tile([128, S], bf16, tag="sb", name=f"sb_{sec}{h}")
            for t, r, ln, off in _row_pieces(rows):
                nc.sync.dma_start(sa[off:off + ln, :],
                                  qkv_sb[t][r:r + ln, :])
            if late:
                # endgame: stage sb straight from qkv tiles (parallel with
                # sa) so the last ropes don't pay the serial sa->sb hop
                for t, r, ln, off in _row_pieces(rows[BLK:]):
                    nc.sync.dma_start(sb[off:off + ln, :],
                                      qkv_sb[t][r:r + ln, :])
                for t, r, ln, off in _row_pieces(rows[:BLK]):
                    nc.sync.dma_start(sb[BLK + off:BLK + off + ln, :],
                                      qkv_sb[t][r:r + ln, :])
            else:
                nc.gpsimd.dma_start(sb[0:BLK, :], sa[BLK:HD, :])
                nc.gpsimd.dma_start(sb[BLK:HD, :], sa[0:BLK, :])
            rot = rot_sb[(sec, h)]
            nc.vector.tensor_mul(rot[0:HD, :], sa[0:HD, :], cos_sb[0:HD, :])
            nc.vector.tensor_mul(sb[0:HD, :], sb[0:HD, :], sin_sb[0:HD, :])
            nc.vector.tensor_add(rot[0:HD, :], rot[0:HD, :], sb[0:HD, :])

        def emit_vaug(h):
            # v transposes -> vaug; emitted as soon as the v tile's
            # activation exists so the DVE copies never crowd the endgame
            for g in range(2):  # groups of 4 key tiles
                tp = psum.tile([128, 1024], bf16, tag=f"tp{g % 2}",
                               name="tp")
                for x in range(4):
                    ti = 4 * g + x
                    nc.tensor.transpose(
                        tp[:, x * HD:(x + 1) * HD],
                        qkv_sb[v_tile[h]][0:HD, ti * 128:(ti + 1) * 128],
                        ident[0:HD, 0:HD])
                dst = vaug_sb[h].rearrange("p (t c) -> p t c", c=VW)
                nc.scalar.activation(
                    dst[:, 4 * g:4 * g + 4, 0:HD],
                    tp[:, 0:4 * HD].rearrange("p (t c) -> p t c",
                                              c=HD)[:, 0:4, :], AF.Identity)

        def emit_attn(h, weave=None, po_tag="po", shift_eng=None):
            # ---- scores -> exp -> PV over 16 (key tile, half) units ----
            # PV lags one unit so exp latency is hidden; the woven qkv
            # j-tile's matmuls fill the remaining PE slack.
            qT = rot_sb[("q", h)]
            kT = rot_sb[("k", h)]
            po = psum.tile([128, S], f32, tag=po_tag, name="po")
            if weave is not None:
                wv_ps = psum.tile([128, S], f32, tag="wv", name="wv")
                wv_mm = [(c, k) for c in (0, 512) for k in range(NK)]
                wv_done = 0
            units = [(ti, c) for ti in range(NTT) for c in (0, 512)]
            pend = {}

            def emit_pv(u):
                pt, ti, c = pend.pop(u)
                nc.tensor.matmul(
                    po[0:VW, c:c + 512],
                    vaug_sb[h][:, ti * VW:(ti + 1) * VW],
                    pt[:, :],
                    start=(ti == 0), stop=(ti == NTT - 1))

            for u, (ti, c) in enumerate(units):
                st = psum.tile([128, 512], f32, tag=f"st{u % 2}", name="st")
                nc.tensor.matmul(st[:, :], kT[0:HD, ti * 128:(ti + 1) * 128],
                                 qT[0:HD, c:c + 512], start=True, stop=True)
                pt = pt_pool.tile([128, 512], bf16, tag="pt", name="pt")
                nc.scalar.activation(pt[:, :], st[:, :], AF.Exp)
                pend[u] = (pt, ti, c)
                if weave is not None:
                    take = 2 if u % 4 == 0 else 1
                    for cc, k in wv_mm[wv_done:wv_done + take]:
                        nc.tensor.matmul(
                            wv_ps[:, cc:cc + 512],
                            wj_tiles[weave][:, k * 128:(k + 1) * 128],
                            hid_of(k)[:, cc:cc + 512],
                            start=(k == 0), stop=(k == NK - 1))
                    wv_done += take
                if u >= 1:
                    emit_pv(u - 1)
            if weave is not None and wv_done < len(wv_mm):
                for cc, k in wv_mm[wv_done:]:
                    nc.tensor.matmul(
                        wv_ps[:, cc:cc + 512],
                        wj_tiles[weave][:, k * 128:(k + 1) * 128],
                        hid_of(k)[:, cc:cc + 512],
                        start=(k == 0), stop=(k == NK - 1))
            emit_pv(len(units) - 1)
            if weave is not None:
                if bias_zero:
                    nc.vector.tensor_copy(qkv_sb[weave][:, :], wv_ps[:, :])
                else:
                    nc.scalar.activation(qkv_sb[weave][:, :], wv_ps[:, :],
                                         AF.Identity,
                                         bias=bias_sb[:, weave:weave + 1])

            # ---- normalize: recip straight off PSUM row 96, row shift
            # on the ACT queue (lands right after this head's exps) ----
            rc = nrm_pool.tile([128, S], f32, tag="rc", name="rc")
            nc.vector.reciprocal(rc[96:97, :], po[96:97, :])
            (shift_eng or nc.gpsimd).dma_start(rc[0:1, :], rc[96:97, :])
            bc = nrm_pool.tile([128, S], f32, tag="bc", name="bc")
            nc.gpsimd.partition_broadcast(bc[0:HD, :], rc[0:1, :])
            # the last head lands directly in the packed proj tile (rows
            # 0:80 of pk4) so proj is gated only by this normalize, not by
            # an extra repack DMA
            dst = pk_sb[4][0:HD, :] if h == n_h - 1 else attn_sb[h][0:HD, :]
            nc.vector.tensor_mul(dst, po[0:HD, :], bc[0:HD, :])

        def emit_repack(h):
            # dense proj k-tiles; emitted post-loop so these DMAs never
            # head-of-line-block the weight stream on the sync queue.
            # proj-row map: heads 0-5 at 80h; h6 split 480:512 + 592:640;
            # h7 occupies 512:592 (written in place by its normalize).
            if h == n_h - 1:
                return
            if h == 6:
                spans = [(480, 0, 32), (592, 32, 48)]
            else:
                spans = [(HD * h, 0, HD)]
            for g0, off, ln in spans:
                while ln > 0:
                    t, r = g0 // 128, g0 % 128
                    n = min(128 - r, ln)
                    nc.sync.dma_start(pk_sb[t][r:r + n, :],
                                      attn_sb[h][off:off + n, :])
                    g0 += n
                    off += n
                    ln -= n

        # ------------ phase 1: qkv + interleaved per-head attention ----
        with ExitStack() as p1:
            hid_pool = p1.enter_context(tc.tile_pool(name="hid", bufs=1))
            w_pool = p1.enter_context(tc.tile_pool(name="wstream", bufs=3))
            # hid loaded in k-pairs (halves the DMA count)
            hid_pairs = [hid_pool.tile([128, 2 * S], bf16, tag=f"hid{p}",
                                       name=f"hid{p}") for p in range(NK // 2)]
            hid_of = lambda k: hid_pairs[k // 2][:, (k % 2) * S:
                                                 (k % 2) * S + S]
            wj_tiles = {}

            def load_wj(j):
                # j-pair granularity: one DMA covers tiles j, j+1
                if j in wj_tiles:
                    return
                j0 = j - j % 2
                wp2 = w_pool.tile([128, 2 * NK * 128], bf16, tag="wj",
                                  name=f"wj{j0}")
                nn = min(2, NJ - j0)
                nc.sync.dma_start(
                    wp2[:, 0:nn * NK * 128],
                    wqkvT[:, j0 * NK * 128:(j0 + nn) * NK * 128])
                for jj in range(j0, j0 + nn):
                    wj_tiles[jj] = wp2[:, (jj - j0) * NK * 128:
                                       (jj - j0 + 1) * NK * 128]

            # DMA order on the sync queue: hid k0, wj0, hid k1, wj1 — the
            # fused j0/j1 pair consumes hid tiles as they arrive.
            wp01 = w_pool.tile([128, 2 * NK * 128], bf16, tag="wj",
                               name="wj0")
            nc.scalar.dma_start(wp01[:, 0:NK * 128], wqkvT[:, 0:NK * 128])
            nc.sync.dma_start(hid_pairs[0][:, 0:S], hiddenT[:, 0:S])
            nc.scalar.dma_start(wp01[:, NK * 128:], wqkvT[:, NK * 128:
                                                          2 * NK * 128])
            nc.sync.dma_start(hid_pairs[0][:, S:2 * S], hiddenT[:, S:2 * S])
            wj_tiles[0] = wp01[:, 0:NK * 128]
            wj_tiles[1] = wp01[:, NK * 128:2 * NK * 128]
            for p in range(1, NK // 2):
                nc.sync.dma_start(hid_pairs[p][:],
                                  hiddenT[:, 2 * p * S:(2 * p + 2) * S])
            load_wj(2)
            nc.sync.dma_start(bias_sb[:], bias2d[:])
            nc.sync.dma_start(cos_sb[0:HD, :], cosP[:])
            nc.sync.dma_start(sin_sb[0:HD, :], sinP[:])
            nc.sync.dma_start(vaug_all[:], vinit[:])

            def after_j(j):
                if j + 1 < NJ and (j + 1) not in wj_tiles:
                    load_wj(j + 1)
                if j == 10:
                    for t in range(5):
                        nc.sync.dma_start(wp_sb[t][:],
                                          wprojT[t * 128:(t + 1) * 128, :])
                for h in vaug_at[j]:
                    emit_vaug(h)
                for h, sec in rope_at[j]:
                    emit_rope(h, sec, late=(j >= 13))

            # fused j0/j1: k-outer so the PE starts as soon as hid0 lands
            ps0 = psum.tile([128, S], f32, tag="wv", name="ps0")
            ps1 = psum.tile([128, S], f32, tag="po", name="ps1")
            for k in range(NK):
                for ps, j in ((ps0, 0), (ps1, 1)):
                    for c in (0, 512):
                        nc.tensor.matmul(
                            ps[:, c:c + 512],
                            wj_tiles[j][:, k * 128:(k + 1) * 128],
                            hid_of(k)[:, c:c + 512],
                            start=(k == 0), stop=(k == NK - 1))
            for ps, j in ((ps0, 0), (ps1, 1)):
                if bias_zero:
                    nc.vector.tensor_copy(qkv_sb[j][:, :], ps[:, :])
                else:
                    nc.scalar.activation(qkv_sb[j][:, :], ps[:, :],
                                         AF.Identity,
                                         bias=bias_sb[:, j:j + 1])
                after_j(j)

            # attention blocks due at `ready+2` consume (weave) that j's
            # matmuls; j14 stays plain so act14 lands before the last ropes
            emitted = set()
            j = 2
            while j < NJ:
                due_now = [h for h in range(n_h) if h not in emitted
                           and min(ready[h] + 2, NJ - 1) <= j]
                if due_now and j < NJ - 1:
                    h = due_now[0]
                    if j not in wj_tiles:
                        load_wj(j)
                    emit_attn(h, weave=j)
                    emitted.add(h)
                    after_j(j)
                    j += 1
                    continue
                if j not in wj_tiles:
                    load_wj(j)
                sts = [psum.tile([128, 512], f32, tag=f"st{i}", name="qs")
                       for i in range(2)]
                for ci, c in enumerate((0, 512)):
                    for k in range(NK):
                        nc.tensor.matmul(
                            sts[ci][:, :],
                            wj_tiles[j][:, k * 128:(k + 1) * 128],
                            hid_of(k)[:, c:c + 512],
                            start=(k == 0), stop=(k == NK - 1))
                    if bias_zero:
                        nc.vector.tensor_copy(qkv_sb[j][:, c:c + 512],
                                              sts[ci][:, :])
                    else:
                        nc.scalar.activation(qkv_sb[j][:, c:c + 512],
                                             sts[ci][:, :], AF.Identity,
                                             bias=bias_sb[:, j:j + 1])
                after_j(j)
                j += 1
        # post-loop attention/repack sits OUTSIDE the p1 pools' scope: the
        # pool-exit engine drains must not wait on the attention tail
        unemitted = [h for h in range(n_h) if h not in emitted]
        for i, h in enumerate(unemitted):
            if h in vaug_deferred:
                emit_vaug(h)   # after the previous attn block so the
                # in-order PE isn't blocked waiting on the last act
            emit_attn(h, po_tag="wv" if i % 2 == 0 else "po")
        for h in range(n_h):
            emit_repack(h)

        if KERNEL_DEBUG:
            for j in range(NJ):
                nc.sync.dma_start(dbg_qkv[:, j * S:(j + 1) * S],
                                    qkv_sb[j][:])
            i_ = 0
            for h in range(n_h):
                for sec in ("q", "k"):
                    nc.sync.dma_start(dbg_rot[:, i_ * S:(i_ + 1) * S],
                                        rot_sb[(sec, h)][:])
                    i_ += 1
            for h in range(n_h):
                nc.sync.dma_start(dbg_attn[:, h * S:(h + 1) * S],
                                    attn_sb[h][:])

        # ------------ phase 2: projection ---------------------------
        with ExitStack() as p5:
            out_pool = p5.enter_context(tc.tile_pool(name="outsb", bufs=1))
            wv_sl = psum.tile([128, S], f32, tag="wv", name="pjwv")
            po_sl = psum.tile([128, S], f32, tag="po", name="pjpo")
            slots = []

            def slot(i):
                i = i % 8
                if i < 4:
                    return psum.tile([128, 512], f32,
                                     tag=["st0", "st1", "tp0", "tp1"][i],
                                     name="pj")
                if i < 6:
                    return wv_sl[:, (i - 4) * 512:(i - 3) * 512]
                return po_sl[:, (i - 6) * 512:(i - 5) * 512]

            chains = [(j, c) for j in range(D // 128) for c in (0, 512)]
            slot_of = {}

            def open_partA(u):
                j, c = chains[u]
                ps = slot(u)
                slot_of[u] = ps
                for kt in range(4):
                    nc.tensor.matmul(ps[:, 0:512],
                                     wp_sb[kt][:, j * 128:(j + 1) * 128],
                                     pk_sb[kt][:, c:c + 512],
                                     start=(kt == 0), stop=False)

            for u in range(8):
                open_partA(u)
            for u, (j, c) in enumerate(chains):
                if c == 0:
                    ob = out_pool.tile([128, S], bf16, tag=f"ob{j % 4}",
                                       name="ob")
                ps = slot_of.pop(u)
                nc.tensor.matmul(ps[:, 0:512],
                                 wp_sb[4][:, j * 128:(j + 1) * 128],
                                 pk_sb[4][:, c:c + 512],
                                 start=False, stop=True)
                if u % 2 == 0:
                    nc.scalar.activation(ob[:, c:c + 512], ps[:, 0:512],
                                         AF.Identity)
                else:
                    nc.vector.tensor_copy(ob[:, c:c + 512], ps[:, 0:512])
                if u + 8 < len(chains):
                    open_partA(u + 8)
                if c == 512:
                    eng = (nc.sync, nc.gpsimd)[j % 2]
                    eng.dma_start(outT[j * 128:(j + 1) * 128, :], ob[:, :])

    nc.compile()
    return nc


def _pack_w_a2(Wqkv, bqkv, heads):
    """Dense 15-tile per-head-contiguous packing (see _layout_a2)."""
    import ml_dtypes
    v_tile, q_rows, k_rows, _ = _layout_a2()
    perm = np.zeros((NJ * 128,), np.int64)
    scl = np.ones((NJ * 128,), np.float32)
    used = np.zeros((NJ * 128,), bool)
    for i, h in enumerate(heads):
        for d in range(HD):
            g = 128 * v_tile[i] + d
            perm[g] = 2 * D + h * HD + d  # v
            used[g] = True
        for d, g in enumerate(q_rows[i]):
            perm[g] = h * HD + d
            scl[g] = SCALE
            used[g] = True
        for d, g in enumerate(k_rows[i]):
            perm[g] = D + h * HD + d
            used[g] = True
    W = Wqkv[perm] * scl[:, None]
    W[~used] = 0.0
    b = bqkv[perm] * scl
    b[~used] = 0.0
    # wqkvT host layout: [128, j, k, 128]; [p, j, k, m] = W.T[k*128+p, j*128+m]
    WT = np.ascontiguousarray(W.T)  # [1280, 1920]
    wt = WT.reshape(NK, 128, NJ, 128).transpose(1, 2, 0, 3)
    wt = np.ascontiguousarray(wt.reshape(128, NJ * NK * 128))
    bias2d = np.ascontiguousarray(b.reshape(NJ, 128).T)
    return wt.astype(ml_dtypes.bfloat16), bias2d


def _pack_wproj(Wproj, heads):
    """Rows of Wproj.T for this core's head dims, stacked per head."""
    W = np.zeros((len(heads) * HD, Wproj.shape[0]), np.float32)
    for i, h in enumerate(heads):
        W[i * HD:(i + 1) * HD] = Wproj[:, h * HD:(h + 1) * HD].T
    return W


def _pack_wproj_a2(Wproj, heads):
    """Mode A proj rows match the device pk layout: heads 0-5 at 80h,
    h6 split 480:512 (d0:32) + 592:640 (d32:80), h7 at 512:592."""
    W = np.zeros((640, Wproj.shape[0]), np.float32)
    wt = lambda h, d0, d1: Wproj[:, heads[h] * HD + d0:
                                 heads[h] * HD + d1].T
    for i in range(6):
        W[i * HD:(i + 1) * HD] = wt(i, 0, HD)
    W[480:512] = wt(6, 0, 32)
    W[512:592] = wt(7, 0, HD)
    W[592:640] = wt(6, 32, HD)
    return W


_CACHE = {}


def _kernel_mode_a(hidden_states, cos, sin, Wqkv, bqkv, Wproj, bproj, S):
    import ml_dtypes
    from concourse import bass_utils

    n_h, S_core = H // 2, S // 4
    bz = not np.any(bqkv)
    key = ("A2", bz)
    if key not in _CACHE:
        _CACHE[key] = _build_program_a2(bias_zero=bz)
    nc = _CACHE[key]

    bf = ml_dtypes.bfloat16
    hiddenT = np.ascontiguousarray(hidden_states.T)  # [D, S]

    vinit = np.zeros((128, 8 * NTT, VW), np.float32)
    vinit[:, :, 96] = 1.0
    vinit = np.ascontiguousarray(vinit.reshape(128, 8 * NTT * VW)).astype(bf)

    in_maps = []
    meta = []
    for g in range(2):
        heads = list(range(g * n_h, (g + 1) * n_h))
        wt, b2 = _pack_w_a2(Wqkv, bqkv, heads)
        wprojT = _pack_wproj_a2(Wproj, heads).astype(bf)
        for s in range(4):
            sl = slice(s * S_core, (s + 1) * S_core)
            hseg = hiddenT[:, sl]  # [1280, 1024]
            hid_t = np.ascontiguousarray(
                hseg.reshape(NK, 128, S_core).transpose(1, 0, 2)
                .reshape(128, NK * S_core)).astype(bf)
            cosP = np.ascontiguousarray(cos[sl].T).astype(bf)
            sinP = np.concatenate(
                [-sin[sl].T[0:BLK], sin[sl].T[BLK:HD]], axis=0)
            sinP = np.ascontiguousarray(sinP).astype(bf)
            in_maps.append({
                "hiddenT": hid_t,
                "wqkvT": wt,
                "bias2d": b2,
                "cosP": cosP,
                "sinP": sinP,
                "wprojT": wprojT,
                "vinit": vinit,
            })
            meta.append((g, s))
    res = bass_utils.run_bass_kernel_spmd(nc, in_maps,
                                          core_ids=list(range(N_CORES)))
    out = np.zeros((D, S), np.float32)
    for c, (g, s) in enumerate(meta):
        out[:, s * S_core:(s + 1) * S_core] += \
            res.results[c]["outT"].astype(np.float32)
    return out


# ---------------------------------------------------------------------------
# mode C (non-uniform segments): fp32r 8-way head-parallel fallback
# ---------------------------------------------------------------------------

def _pack_layout(n_h):
    """Pack per-core qkv dims as 40-row blocks, 3 per 128-row tile (8 pad).

    Each tile holds one v-block at row 0 (PE transpose operands must start
    at a 32-aligned partition) and two q/k blocks at rows 40 and 80.
    Returns pos[(sec, h, half)] = (tile, row) and the number of tiles.
    """
    ntiles = 2 * n_h
    pos = {}
    for h in range(n_h):
        for half in (0, 1):
            pos[("v", h, half)] = (2 * h + half, 0)
    qk = [("q", h, half) for h in range(n_h) for half in (0, 1)]
    qk += [("k", h, half) for h in range(n_h) for half in (0, 1)]
    for j, blk in enumerate(qk):
        pos[blk] = (j // 2, BLK + BLK * (j % 2))
    return pos, ntiles


def _build_program(n_h, S_core, segs_local, resident_hidden):
    """Mode C SPMD program (fp32r)."""
    import concourse.mybir as mybir
    import concourse.tile as tile
    from concourse import bacc
    from concourse.masks import make_identity
    from contextlib import ExitStack

    f32 = mybir.dt.float32
    mm_dt = getattr(mybir.dt, MM_DT_NAME)
    AF = mybir.ActivationFunctionType

    k_proj = n_h
    pos, n_mtiles = _pack_layout(n_h)
    dims_pad = n_mtiles * 128
    VWc = 97

    t_tiles = []
    for si, (a, e) in enumerate(segs_local):
        t = a
        while t < e:
            t_tiles.append((si, t, min(t + 128, e)))
            t += 128
    n_tt = len(t_tiles)

    nc = bacc.Bacc("TRN2", target_bir_lowering=False, debug=False,
                   enable_asserts=False, num_devices=N_CORES)

    hiddenT = nc.dram_tensor("hiddenT", [128, NK * S_core], mm_dt,
                             kind="ExternalInput").ap()
    wqkvT = nc.dram_tensor("wqkvT", [128, NK * dims_pad], mm_dt,
                           kind="ExternalInput").ap()
    bias2d = nc.dram_tensor("bias2d", [128, n_mtiles], f32,
                            kind="ExternalInput").ap()
    cosP = nc.dram_tensor("cosP", [128, S_core], mm_dt,
                          kind="ExternalInput").ap()
    sin2P = nc.dram_tensor("sin2P", [128, S_core], mm_dt,
                           kind="ExternalInput").ap()
    wprojT = nc.dram_tensor("wprojT", [n_h * HD, D], mm_dt,
                            kind="ExternalInput").ap()
    vinit = nc.dram_tensor("vinit", [128, n_tt * (VWc - HD)], mm_dt,
                           kind="ExternalInput").ap()
    outT = nc.dram_tensor("outT", [D, S_core], f32, kind="ExternalOutput").ap()

    def r_(ap):
        return ap.bitcast(mm_dt)

    BC = 1024
    big_chunks = [(c, min(c + BC, S_core)) for c in range(0, S_core, BC)]

    def halves(c0, c1):
        out = []
        q = c0
        while q < c1:
            out.append((q, min(q + 512, c1)))
            q = q + 512
        return out

    with tile.TileContext(nc) as tc, ExitStack() as ctx:
        persist = ctx.enter_context(tc.tile_pool(name="persist", bufs=1))
        ident = persist.tile([128, 128], f32, tag="ident", name="ident")
        make_identity(nc, ident[:])
        bias_sb = persist.tile([128, n_mtiles], f32, tag="bias", name="bias")
        nc.sync.dma_start(bias_sb[:], bias2d[:])

        psum_all_cm = tc.tile_pool(name="psum_all", bufs=1, space="PSUM")
        psum_all = psum_all_cm.__enter__()
        qkv_pool = ctx.enter_context(tc.tile_pool(name="big", bufs=1))
        qkv_sb = [qkv_pool.tile([128, S_core], mm_dt, tag=f"qkvT{j}",
                                name=f"qkvT{j}") for j in range(n_mtiles)]
        rot_cm = tc.tile_pool(name="rot", bufs=1)
        rv = rot_cm.__enter__()
        rot_sb = {}
        for h in range(n_h):
            for sec in ("q", "k"):
                rot_sb[(sec, h)] = rv.tile([128, S_core], mm_dt,
                                           tag=f"rot_{sec}{h}",
                                           name=f"rot_{sec}{h}")
        RC = 1024
        rope_cm = tc.tile_pool(name="rope_scr", bufs=2)
        rope_scr = rope_cm.__enter__()

        with ExitStack() as p1:
            hidden3 = hiddenT.rearrange("p (k s) -> p k s", k=NK)
            w3 = wqkvT.rearrange("p (k m) -> p k m", k=NK)
            w_pool = p1.enter_context(tc.tile_pool(name="wres", bufs=1))
            w_sb = [w_pool.tile([128, dims_pad], mm_dt, tag=f"w{k}",
                                name=f"w{k}") for k in range(NK)]
            for k in range(NK):
                nc.sync.dma_start(w_sb[k][:], w3[:, k, :])
            hid_pool = p1.enter_context(tc.tile_pool(name="hidstream",
                                                     bufs=3))
            n4 = n_mtiles // 4
            for (h0, h1) in halves(0, S_core):
                hw = h1 - h0
                for q4 in range(n4):
                    ps01 = psum_all.tile([128, BC], f32, tag="t0",
                                         name="ps01")
                    ps23 = psum_all.tile([128, BC], f32, tag="t1",
                                         name="ps23")
                    pj_of = lambda j: (ps01 if j % 4 < 2 else ps23,
                                       (j % 2) * 512)
                    for k in range(NK):
                        ht = hid_pool.tile([128, 512], mm_dt, tag="hidc",
                                           name="hidc")
                        nc.sync.dma_start(ht[:, :hw], hidden3[:, k, h0:h1])
                        for j in range(q4 * 4, q4 * 4 + 4):
                            psj, co = pj_of(j)
                            nc.tensor.matmul(
                                psj[:, co:co + hw],
                                r_(w_sb[k][:, j * 128:(j + 1) * 128]),
                                r_(ht[:, :hw]),
                                start=(k == 0), stop=(k == NK - 1))
                    for j in range(q4 * 4, q4 * 4 + 4):
                        psj, co = pj_of(j)
                        nc.scalar.activation(qkv_sb[j][:, h0:h1],
                                             psj[:, co:co + hw], AF.Identity,
                                             bias=bias_sb[:, j:j + 1])

        psum_all_cm.__exit__(None, None, None)
        ps_att = ctx.enter_context(tc.tile_pool(name="ps_att", bufs=1,
                                                space="PSUM"))

        stg = {}
        for nm in ("sa0", "sa1", "sb0", "sb1"):
            stg[nm] = rope_scr.tile([128, RC], mm_dt, tag=nm, name=nm, bufs=1)
        pair_i = 0
        for ci, f0 in enumerate(range(0, S_core, RC)):
            f1 = min(f0 + RC, S_core)
            fs = f1 - f0
            cos_sb = rope_scr.tile([128, RC], mm_dt, tag="cos", name="cos",
                                   bufs=1)
            sin_sb = rope_scr.tile([128, RC], mm_dt, tag="sin", name="sin",
                                   bufs=1)
            nc.scalar.dma_start(cos_sb[:, :fs], cosP[:, f0:f1])
            nc.scalar.dma_start(sin_sb[:, :fs], sin2P[:, f0:f1])
            if ci == 0:
                for nm in stg:
                    nc.scalar.dma_start(stg[nm][BLK:64, :], cos_sb[BLK:64, :])
            for h in range(n_h):
                for sec in ("q", "k"):
                    lo_t, lo_r = pos[(sec, h, 0)]
                    hi_t, hi_r = pos[(sec, h, 1)]
                    x = qkv_sb[lo_t]
                    dst = rot_sb[(sec, h)]
                    stga = stg[f"sa{pair_i % 2}"]
                    stgb = stg[f"sb{pair_i % 2}"]
                    nc.scalar.dma_start(stga[0:BLK, :fs],
                                        x[lo_r:lo_r + BLK, f0:f1])
                    nc.scalar.dma_start(stga[64:64 + BLK, :fs],
                                        x[hi_r:hi_r + BLK, f0:f1])
                    nc.scalar.dma_start(stgb[0:BLK, :fs],
                                        x[hi_r:hi_r + BLK, f0:f1])
                    nc.scalar.dma_start(stgb[64:64 + BLK, :fs],
                                        x[lo_r:lo_r + BLK, f0:f1])
                    nc.vector.tensor_mul(dst[0:104, f0:f1], stga[0:104, :fs],
                                         cos_sb[0:104, :fs])
                    eng = nc.gpsimd if pair_i % 2 == 0 else nc.vector
                    eng.tensor_mul(stgb[0:104, :fs], stgb[0:104, :fs],
                                   sin_sb[0:104, :fs])
                    nc.vector.tensor_add(dst[0:104, f0:f1], dst[0:104, f0:f1],
                                         stgb[0:104, :fs])
                    pair_i += 1
        rope_cm.__exit__(None, None, None)

        vaug_cm = tc.tile_pool(name="vaug", bufs=1)
        vaug_pool = vaug_cm.__enter__()
        vaug_sb = [vaug_pool.tile([128, n_tt * VWc], mm_dt, tag=f"vaug{h}",
                                  name=f"vaug{h}") for h in range(n_h)]
        vinit3 = vinit.rearrange("p (t c) -> p t c", c=VWc - HD)
        for h in range(n_h):
            nc.sync.dma_start(
                vaug_sb[h].rearrange("p (t c) -> p t c", c=VWc)[:, :, HD:VWc],
                vinit3[:, :, :])
        GRP = 4

        def emit_vaug(h):
            gi = 0
            while gi < n_tt:
                hi_g = min(gi + GRP, n_tt)
                if all(t_tiles[g][2] - t_tiles[g][1] == 128
                       for g in range(gi, hi_g)):
                    grp = list(range(gi, hi_g))
                else:
                    grp = [gi]
                ng = len(grp)
                tp = ps_att.tile([128, GRP * HD], f32, tag="tp", name="tp")
                for x, g in enumerate(grp):
                    si, t0, t1 = t_tiles[g]
                    sz = t1 - t0
                    for half in (0, 1):
                        vt, vr = pos[("v", h, half)]
                        nc.tensor.transpose(
                            tp[:sz, x * HD + half * BLK:
                               x * HD + (half + 1) * BLK],
                            qkv_sb[vt][0:BLK, t0:t1].bitcast(f32),
                            ident[:BLK, :BLK])
                sz0 = t_tiles[grp[0]][2] - t_tiles[grp[0]][1]
                dst = vaug_sb[h].rearrange("p (t c) -> p t c", c=VWc)
                src_ap = tp.rearrange("p (t c) -> p t c", c=HD)
                nc.vector.tensor_copy(dst[:sz0, grp[0]:grp[0] + ng, 0:HD],
                                      src_ap[:sz0, 0:ng, :])
                gi += ng

        attn_sb = [qkv_pool.tile([128, S_core], mm_dt, tag=f"qkvT{hh}",
                                 name=f"attnT{hh}") for hh in range(n_h)]

        seg_ttiles = {}
        for ti, (si, t0, t1) in enumerate(t_tiles):
            seg_ttiles.setdefault(si, []).append((ti, t0, t1))

        BA = 512
        with ExitStack() as p4:
            pt_pool = p4.enter_context(tc.tile_pool(name="pt", bufs=3))
            nrm_pool = p4.enter_context(tc.tile_pool(name="nrm", bufs=2))
            unit_box = [0]

            def emit_attention(h, si, a, e):
                qT = rot_sb[("q", h)]
                kT = rot_sb[("k", h)]
                q = a
                while q < e:
                    q0, q1 = q, min(q + BA, e)
                    qs = q1 - q0
                    po = ps_att.tile([128, BA], f32,
                                     tag=f"po{unit_box[0] % 2}", name="pv")
                    tts = seg_ttiles[si]
                    for idx, (ti, t0, t1) in enumerate(tts):
                        sz = t1 - t0
                        ps = ps_att.tile([128, BA], f32, tag=f"st{idx % 2}",
                                         name="st")
                        nc.tensor.matmul(ps[:sz, :qs], r_(kT[0:104, t0:t1]),
                                         r_(qT[0:104, q0:q1]),
                                         start=True, stop=True)
                        pt = pt_pool.tile([128, BA], mm_dt, tag="pt",
                                          name="pt")
                        nc.scalar.activation(pt[:sz, :qs], ps[:sz, :qs],
                                             AF.Exp)
                        nc.tensor.matmul(
                            po[:VWc, :qs],
                            r_(vaug_sb[h][:sz, ti * VWc:(ti + 1) * VWc]),
                            r_(pt[:sz, :qs]),
                            start=(idx == 0), stop=(idx == len(tts) - 1))
                    rc = nrm_pool.tile([128, BA], f32, tag="rc", name="rc")
                    nc.vector.tensor_copy(rc[96:97, :qs], po[96:97, :qs])
                    nc.sync.dma_start(rc[0:1, :qs], rc[96:97, :qs])
                    nc.vector.reciprocal(rc[0:1, :qs], rc[0:1, :qs])
                    bc = nrm_pool.tile([128, BA], mm_dt, tag="bc", name="bc")
                    nc.gpsimd.partition_broadcast(
                        bc[0:HD, :qs], rc[0:1, :qs].bitcast(mm_dt))
                    nc.vector.tensor_mul(attn_sb[h][0:HD, q0:q1],
                                         po[0:HD, :qs], bc[0:HD, :qs])
                    unit_box[0] += 1
                    q = q1

            for h in range(n_h):
                emit_vaug(h)
            for si, (a, e) in enumerate(segs_local):
                for h in range(n_h):
                    emit_attention(h, si, a, e)

        vaug_cm.__exit__(None, None, None)
        rot_cm.__exit__(None, None, None)

        with ExitStack() as p5:
            wp_pool = p5.enter_context(tc.tile_pool(name="wp", bufs=1))
            wp_sb = []
            for kt in range(k_proj):
                t = wp_pool.tile([HD, D], mm_dt, tag=f"wp{kt}", name=f"wp{kt}")
                nc.sync.dma_start(t[:], wprojT[kt * HD:(kt + 1) * HD, :])
                wp_sb.append(t)
            out_pool = p5.enter_context(tc.tile_pool(name="outsb", bufs=3))
            for (c0, c1) in big_chunks:
                cs = c1 - c0
                for j in range(D // 128):
                    ob = out_pool.tile([128, BC], f32, tag="ob", name="ob")
                    for (h0, h1) in halves(c0, c1):
                        ps = ps_att.tile([128, 512], f32, tag=f"st{j % 2}",
                                         name="pj")
                        for kt in range(k_proj):
                            nc.tensor.matmul(
                                ps[:, :h1 - h0],
                                r_(wp_sb[kt][:, j * 128:(j + 1) * 128]),
                                r_(attn_sb[kt][0:HD, h0:h1]),
                                start=(kt == 0), stop=(kt == k_proj - 1))
                        if j % 2 == 0:
                            nc.vector.tensor_copy(ob[:, h0 - c0:h1 - c0],
                                                  ps[:, :h1 - h0])
                        else:
                            nc.scalar.activation(ob[:, h0 - c0:h1 - c0],
                                                 ps[:, :h1 - h0], AF.Identity)
                    nc.sync.dma_start(outT[j * 128:(j + 1) * 128, c0:c1],
                                      ob[:, :cs])

    nc.compile()
    return nc


def _pack_w(Wqkv, bqkv, heads, n_h):
    """Mode C packed qkv weights (q rows pre-scaled)."""
    pos, n_mtiles = _pack_layout(n_h)
    dims_pad = n_mtiles * 128
    W = np.zeros((dims_pad, D), np.float32)
    b = np.zeros((dims_pad,), np.float32)
    sec_off = {"q": 0, "k": D, "v": 2 * D}
    for i, h in enumerate(heads):
        for sec in ("q", "k", "v"):
            for half in (0, 1):
                t, r = pos[(sec, i, half)]
                src = sec_off[sec] + h * HD + half * BLK
                w = Wqkv[src:src + BLK, :]
                bb = bqkv[src:src + BLK]
                if sec == "q":
                    w = w * SCALE
                    bb = bb * SCALE
                W[t * 128 + r:t * 128 + r + BLK] = w
                b[t * 128 + r:t * 128 + r + BLK] = bb
    w_tiled = _tile_rows(np.ascontiguousarray(W.T))
    bias2d = np.ascontiguousarray(b.reshape(n_mtiles, 128).T)
    return w_tiled, bias2d


def _tile_rows(x):
    """[R, C] with R = nk*128 -> [128, nk*C] k-major tiling."""
    R, C = x.shape
    nk = R // 128
    return np.ascontiguousarray(
        x.reshape(nk, 128, C).transpose(1, 0, 2).reshape(128, nk * C))


def _pack_cos_sin(cos, sin):
    """Mode C cosP/sin2P [128, S]."""
    S = cos.shape[0]
    cosP = np.zeros((128, S), np.float32)
    sinP = np.zeros((128, S), np.float32)
    cosP[0:BLK] = cos.T[0:BLK]
    cosP[64:64 + BLK] = cos.T[BLK:HD]
    sinP[0:BLK] = -sin.T[0:BLK]
    sinP[64:64 + BLK] = sin.T[BLK:HD]
    return cosP, sinP


def kernel(hidden_states, cos, sin, Wqkv, bqkv, Wproj, bproj, cu_seqlens):
    sys.path.insert(0, "/opt/trn_rl_repo")
    from concourse import bass_utils

    hidden_states = np.asarray(hidden_states, np.float32)
    cos = np.asarray(cos, np.float32)
    sin = np.asarray(sin, np.float32)
    Wqkv = np.asarray(Wqkv, np.float32)
    bqkv = np.asarray(bqkv, np.float32)
    Wproj = np.asarray(Wproj, np.float32)
    bproj = np.asarray(bproj, np.float32)

    S, D_ = hidden_states.shape
    assert D_ == D
    segs = _segments(cu_seqlens, S)
    uniform = (S == 4096) and segs == [(i * S // 4, (i + 1) * S // 4)
                                       for i in range(4)]

    if uniform:
        out = _kernel_mode_a(hidden_states, cos, sin, Wqkv, bqkv, Wproj,
                             bproj, S)
    else:
        hiddenT = np.ascontiguousarray(hidden_states.T)
        cosP, sin2P = _pack_cos_sin(cos, sin)
        n_h, S_core = H // N_CORES, S
        key = ("C", S, tuple(np.asarray(cu_seqlens).tolist()))
        if key not in _CACHE:
            _CACHE[key] = _build_program(n_h, S_core, segs,
                                         resident_hidden=False)
        nc = _CACHE[key]
        n_tt = sum(-(-(e - a) // 128) for a, e in segs)
        vinit = np.zeros((128, n_tt, 17), np.float32)
        vinit[:, :, 16] = 1.0
        vinit = np.ascontiguousarray(vinit.reshape(128, n_tt * 17))
        hid_tiled = _tile_rows(hiddenT)
        in_maps = []
        for c in range(N_CORES):
            heads = list(range(c * n_h, (c + 1) * n_h))
            wt, b2 = _pack_w(Wqkv, bqkv, heads, n_h)
            in_maps.append({
                "hiddenT": hid_tiled,
                "wqkvT": wt,
                "bias2d": b2,
                "cosP": cosP,
                "sin2P": sin2P,
                "wprojT": _pack_wproj(Wproj, heads).astype(np.float32),
                "vinit": vinit,
            })
        res = bass_utils.run_bass_kernel_spmd(nc, in_maps,
                                              core_ids=list(range(N_CORES)))
        out = np.zeros((D, S), np.float32)
        for c in range(N_CORES):
            out += res.results[c]["outT"]

    return np.ascontiguousarray(out.T) + bproj[None, :]


# revision 39
# speedup vs baseline: 1.3679x; 1.0002x over previous
"""Trainium2 Bass kernel for Ernie4.5-VL vision attention (ragged segments).

Contract: kernel(**inputs) takes the FULL unsharded inputs (keyed as in
setup_inputs()) and returns the FULL [S, D] float32 output.

Mode A (uniform 4x1024 segments — the graded shape): 8 cores = 2 head
groups x 4 segments; per core 8 heads x 1024 tokens, everything in bf16
on the PE array (psum f32):

  qkvT = Wpack @ hidden.T     15 dense 128-row tiles (v 80-row blocks at
                              tile h rows 0:80, q/k packed tile-major)
  rope: dense [0:80] layout; the rotate-half operand is built with 2-4
        small SBUF DMAs per (q|k, head); rot = a*cos + b*sin on DVE/Pool
  per head: v transposes (PE) -> scoresT (PE) -> exp (ACT, 1024 wide)
        -> PV accumulate with ones column for the denominator ->
        reciprocal+broadcast+mul normalize
  attn heads DMA-repacked into 5 dense 128-row tiles; proj = 5 k-tiles
  Host does O(S*D) glue: packing, summing the 2 per-token partial
  projections, bias adds.

Engine budget per core (cost model): PE ~142us of matmul rows, ACT
~82us (exp + qkv bias copies), DVE ~40us, Pool ~30us, DMA ~19MB.
Emission interleaves attention per head into the qkv j-loop so every
engine streams; all DMAs avoid the ACT queue (exp lives there).

Mode C fallback (any other cu_seqlens): 8-way head parallel fp32r path
(unchanged from the earlier version of this kernel).
"""

import os
import sys

import numpy as np

H = 16
HD = 80
BLK = 40  # rotate_half half-width
SCALE = HD ** -0.5
N_CORES = 8
D = 1280
NK = D // 128  # contraction tiles for the qkv matmul
ATTN_STRIDE = 96  # head row pitch in the packed attention output (mode C)
MM_DT_NAME = os.environ.get("KERNEL_MM_DT", "float32r")  # mode C only
KERNEL_DEBUG = bool(int(os.environ.get("KERNEL_DEBUG", "0")))

# ---- mode A constants ----
NJ = 15          # dense qkv M tiles (1920 rows)
NTT = 8          # 128-row key tiles per 1024 segment
VW = 97          # vaug slot: 80 v dims + 16 pad + ones col at 96
SA_CORE = 1024   # tokens per core


def _segments(cu_seqlens, S):
    """Intervals matching reference's searchsorted(cu[1:], i, 'right')."""
    b = np.clip(np.sort(np.asarray(cu_seqlens, dtype=np.int64)[1:5]), 0, S)
    bounds = [0] + list(b) + [S]
    segs = []
    for a, e in zip(bounds[:-1], bounds[1:]):
        if e > a:
            segs.append((int(a), int(e)))
    return segs


# ---------------------------------------------------------------------------
# mode A: dense bf16 program
# ---------------------------------------------------------------------------

def _layout_a2():
    """Per-head-contiguous packing: head h owns global rows [240h, 240h+240).
    v sits at rows 0:80 of tile ceil(240h/128) (PE transpose needs a
    32-aligned non-crossing 80-row read); q then k fill the remaining
    window rows in ascending order (read via DMA, placement free).

    Returns (v_tile[h], q_rows[h], k_rows[h], ready_j[h]) where
    q_rows/k_rows are the 80 global rows of each section in dim order.
    """
    v_tile, q_rows, k_rows, ready = [], [], [], []
    for h in range(8):
        w0, w1 = 240 * h, 240 * (h + 1)
        th = -(-w0 // 128)
        vg0 = 128 * th
        qk = [g for g in range(w0, w1) if not (vg0 <= g < vg0 + HD)]
        v_tile.append(th)
        q_rows.append(qk[0:HD])
        k_rows.append(qk[HD:2 * HD])
        ready.append(max(th, qk[-1] // 128))
    return v_tile, q_rows, k_rows, ready


def _row_pieces(rows):
    """Split a list of global rows into (tile, row, len, rel_off) runs that
    are consecutive and stay within one 128-row tile."""
    out = []
    i = 0
    while i < len(rows):
        g = rows[i]
        n = 1
        while (i + n < len(rows) and rows[i + n] == g + n
               and (g + n) // 128 == g // 128):
            n += 1
        out.append((g // 128, g % 128, n, i))
        i += n
    return out


def _build_program_a2(bias_zero=True):
    """Mode A program: n_h=8 heads, S=1024 tokens per core, one segment."""
    import concourse.mybir as mybir
    import concourse.tile as tile
    from concourse import bacc
    from concourse.masks import make_identity
    from contextlib import ExitStack

    f32 = mybir.dt.float32
    bf16 = mybir.dt.bfloat16
    AF = mybir.ActivationFunctionType
    n_h, S = 8, SA_CORE

    nc = bacc.Bacc("TRN2", target_bir_lowering=False, debug=False,
                   enable_asserts=False, num_devices=N_CORES)

    hiddenT = nc.dram_tensor("hiddenT", [128, NK * S], bf16,
                             kind="ExternalInput").ap()
    wqkvT = nc.dram_tensor("wqkvT", [128, NJ * NK * 128], bf16,
                           kind="ExternalInput").ap()
    bias2d = nc.dram_tensor("bias2d", [128, NJ], f32,
                            kind="ExternalInput").ap()
    cosP = nc.dram_tensor("cosP", [HD, S], bf16, kind="ExternalInput").ap()
    sinP = nc.dram_tensor("sinP", [HD, S], bf16, kind="ExternalInput").ap()
    wprojT = nc.dram_tensor("wprojT", [n_h * HD, D], bf16,
                            kind="ExternalInput").ap()
    vinit = nc.dram_tensor("vinit", [128, 8 * NTT * VW], bf16,
                           kind="ExternalInput").ap()
    outT = nc.dram_tensor("outT", [D, S], bf16, kind="ExternalOutput").ap()
    if KERNEL_DEBUG:
        dbg_qkv = nc.dram_tensor("dbg_qkv", [128, NJ * S], f32,
                                 kind="ExternalOutput").ap()
        dbg_rot = nc.dram_tensor("dbg_rot", [128, 2 * n_h * S], f32,
                                 kind="ExternalOutput").ap()
        dbg_attn = nc.dram_tensor("dbg_attn", [128, n_h * S], f32,
                                  kind="ExternalOutput").ap()

    v_tile, q_rows, k_rows, ready = _layout_a2()
    rope_at = {j: [] for j in range(NJ)}   # (h, sec) at section readiness
    vaug_at = {j: [] for j in range(NJ)}   # h at v-tile readiness
    vaug_deferred = []
    for h in range(n_h):
        for sec, rows in (("q", q_rows[h]), ("k", k_rows[h])):
            rope_at[max(g // 128 for g in rows)].append((h, sec))
        if v_tile[h] >= NJ - 1:
            vaug_deferred.append(h)
        else:
            vaug_at[v_tile[h]].append(h)

    with tile.TileContext(nc) as tc, ExitStack() as ctx:
        persist = ctx.enter_context(tc.tile_pool(name="persist", bufs=1))
        ident = persist.tile([128, 128], bf16, tag="ident", name="ident")
        make_identity(nc, ident[:])
        bias_sb = persist.tile([128, NJ], f32, tag="bias", name="bias")
        cos_sb = persist.tile([128, S], bf16, tag="cos", name="cos")
        sin_sb = persist.tile([128, S], bf16, tag="sin", name="sin")

        psum = ctx.enter_context(tc.tile_pool(name="psum", bufs=1,
                                              space="PSUM"))
        qkv_pool = ctx.enter_context(tc.tile_pool(name="qkv", bufs=1))
        qkv_sb = [qkv_pool.tile([128, S], bf16, tag=f"qkvT{j}",
                                name=f"qkvT{j}") for j in range(NJ)]
        rot_pool = ctx.enter_context(tc.tile_pool(name="rot", bufs=1))
        rot_sb = {}
        for h in range(n_h):
            for sec in ("q", "k"):
                rot_sb[(sec, h)] = rot_pool.tile(
                    [128, S], bf16, tag=f"rot_{sec}{h}", name=f"rot_{sec}{h}")
        vaug_pool = ctx.enter_context(tc.tile_pool(name="vaug", bufs=1))
        vaug_all = vaug_pool.tile([128, n_h * NTT * VW], bf16, tag="vaug",
                                  name="vaug")
        vaug_sb = [vaug_all[:, h * NTT * VW:(h + 1) * NTT * VW]
                   for h in range(n_h)]
        attn_pool = ctx.enter_context(tc.tile_pool(name="attn", bufs=1))
        attn_sb = [attn_pool.tile([128, S], bf16, tag=f"attn{h}",
                                  name=f"attn{h}") for h in range(n_h)]
        stg_pool = ctx.enter_context(tc.tile_pool(name="stg", bufs=2))
        pt_pool = ctx.enter_context(tc.tile_pool(name="pt", bufs=4))
        nrm_pool = ctx.enter_context(tc.tile_pool(name="nrm", bufs=2))
        pk_pool = ctx.enter_context(tc.tile_pool(name="pk", bufs=1))
        pk_sb = [pk_pool.tile([128, S], bf16, tag=f"pk{t}", name=f"pk{t}")
                 for t in range(5)]
        wp_pool = ctx.enter_context(tc.tile_pool(name="wp", bufs=1))
        wp_sb = [wp_pool.tile([128, D], bf16, tag=f"wp{t}", name=f"wp{t}")
                 for t in range(5)]


        def emit_rope(h, sec, late=False):
            # sa (x in dim order) staged from qkv tiles on the SP queue;
            # sb = rotate_half(sa) built from sa with exactly two Pool
            # (SWDGE) DMAs. DMA count is precious: each dma_start holds its
            # queue for wait+transfer+sem (~1.1us fixed).
            rows = q_rows[h] if sec == "q" else k_rows[h]
            sa = stg_pool.tile([128, S], bf16, tag="sa", name=f"sa_{sec}{h}")
            sb = stg_pool.tile([128, S], bf16, tag="sb", name=f"sb_{sec}{h}")
            for t, r, ln, off in _row_pieces(rows):
                nc.sync.dma_start(sa[off:off + ln, :],
                                  qkv_sb[t][r:r + ln, :])
            if late:
                # endgame: stage sb straight from qkv tiles (parallel with
                # sa) so the last ropes don't pay the serial sa->sb hop
                for t, r, ln, off in _row_pieces(rows[BLK:]):
                    nc.sync.dma_start(sb[off:off + ln, :],
                                      qkv_sb[t][r:r + ln, :])
                for t, r, ln, off in _row_pieces(rows[:BLK]):
                    nc.sync.dma_start(sb[BLK + off:BLK + off + ln, :],
                                      qkv_sb[t][r:r + ln, :])
            else:
                nc.gpsimd.dma_start(sb[0:BLK, :], sa[BLK:HD, :])
                nc.gpsimd.dma_start(sb[BLK:HD, :], sa[0:BLK, :])
            rot = rot_sb[(sec, h)]
            nc.vector.tensor_mul(rot[0:HD, :], sa[0:HD, :], cos_sb[0:HD, :])
            nc.vector.tensor_mul(sb[0:HD, :], sb[0:HD, :], sin_sb[0:HD, :])
            nc.vector.tensor_add(rot[0:HD, :], rot[0:HD, :], sb[0:HD, :])

        def emit_vaug(h, on_act=False):
            # v transposes -> vaug; copies on DVE mid-run (ACT paces the
            # attention blocks), on ACT for the deferred last head (DVE is
            # serialized behind the previous norm chain there)
            for g in range(2):  # groups of 4 key tiles
                tp = psum.tile([128, 1024], bf16, tag=f"tp{g % 2}",
                               name="tp")
                for x in range(4):
                    ti = 4 * g + x
                    nc.tensor.transpose(
                        tp[:, x * HD:(x + 1) * HD],
                        qkv_sb[v_tile[h]][0:HD, ti * 128:(ti + 1) * 128],
                        ident[0:HD, 0:HD])
                dst = vaug_sb[h].rearrange("p (t c) -> p t c", c=VW)
                src_ap = tp[:, 0:4 * HD].rearrange("p (t c) -> p t c",
                                                   c=HD)[:, 0:4, :]
                if on_act:
                    nc.scalar.activation(dst[:, 4 * g:4 * g + 4, 0:HD],
                                         src_ap, AF.Identity)
                else:
                    nc.vector.tensor_copy(dst[:, 4 * g:4 * g + 4, 0:HD],
                                          src_ap)

        def emit_attn(h, weave=None, po_tag="po", shift_eng=None):
            # ---- scores -> exp -> PV over 16 (key tile, half) units ----
            # PV lags one unit so exp latency is hidden; the woven qkv
            # j-tile's matmuls fill the remaining PE slack.
            qT = rot_sb[("q", h)]
            kT = rot_sb[("k", h)]
            po = psum.tile([128, S], f32, tag=po_tag, name="po")
            if weave is not None:
                wv_ps = psum.tile([128, S], f32, tag="wv", name="wv")
                wv_mm = [(c, k) for c in (0, 512) for k in range(NK)]
                wv_done = 0
            units = [(ti, c) for ti in range(NTT) for c in (0, 512)]
            pend = {}

            def emit_pv(u):
                pt, ti, c = pend.pop(u)
                nc.tensor.matmul(
                    po[0:VW, c:c + 512],
                    vaug_sb[h][:, ti * VW:(ti + 1) * VW],
                    pt[:, :],
                    start=(ti == 0), stop=(ti == NTT - 1))

            for u, (ti, c) in enumerate(units):
                st = psum.tile([128, 512], f32, tag=f"st{u % 2}", name="st")
                nc.tensor.matmul(st[:, :], kT[0:HD, ti * 128:(ti + 1) * 128],
                                 qT[0:HD, c:c + 512], start=True, stop=True)
                pt = pt_pool.tile([128, 512], bf16, tag="pt", name="pt")
                nc.scalar.activation(pt[:, :], st[:, :], AF.Exp)
                pend[u] = (pt, ti, c)
                if weave is not None:
                    take = 2 if u % 4 == 0 else 1
                    for cc, k in wv_mm[wv_done:wv_done + take]:
                        nc.tensor.matmul(
                            wv_ps[:, cc:cc + 512],
                            wj_tiles[weave][:, k * 128:(k + 1) * 128],
                            hid_of(k)[:, cc:cc + 512],
                            start=(k == 0), stop=(k == NK - 1))
                    wv_done += take
                if u >= 1:
                    emit_pv(u - 1)
            if weave is not None and wv_done < len(wv_mm):
                for cc, k in wv_mm[wv_done:]:
                    nc.tensor.matmul(
                        wv_ps[:, cc:cc + 512],
                        wj_tiles[weave][:, k * 128:(k + 1) * 128],
                        hid_of(k)[:, cc:cc + 512],
                        start=(k == 0), stop=(k == NK - 1))
            emit_pv(len(units) - 1)
            if weave is not None:
                if bias_zero:
                    nc.vector.tensor_copy(qkv_sb[weave][:, :], wv_ps[:, :])
                else:
                    nc.scalar.activation(qkv_sb[weave][:, :], wv_ps[:, :],
                                         AF.Identity,
                                         bias=bias_sb[:, weave:weave + 1])

            # ---- normalize: recip straight off PSUM row 96, row shift
            # on the ACT queue (lands right after this head's exps) ----
            rc = nrm_pool.tile([128, S], f32, tag="rc", name="rc")
            nc.vector.reciprocal(rc[96:97, :], po[96:97, :])
            (shift_eng or nc.gpsimd).dma_start(rc[0:1, :], rc[96:97, :])
            bc = nrm_pool.tile([128, S], f32, tag="bc", name="bc")
            nc.gpsimd.partition_broadcast(bc[0:HD, :], rc[0:1, :])
            # the last head lands directly in the packed proj tile (rows
            # 0:80 of pk4) so proj is gated only by this normalize, not by
            # an extra repack DMA
            dst = pk_sb[4][0:HD, :] if h == n_h - 1 else attn_sb[h][0:HD, :]
            nc.vector.tensor_mul(dst, po[0:HD, :], bc[0:HD, :])

        def emit_repack(h):
            # dense proj k-tiles; emitted post-loop so these DMAs never
            # head-of-line-block the weight stream on the sync queue.
            # proj-row map: heads 0-5 at 80h; h6 split 480:512 + 592:640;
            # h7 occupies 512:592 (written in place by its normalize).
            if h == n_h - 1:
                return
            if h == 6:
                spans = [(480, 0, 32), (592, 32, 48)]
            else:
                spans = [(HD * h, 0, HD)]
            for g0, off, ln in spans:
                while ln > 0:
                    t, r = g0 // 128, g0 % 128
                    n = min(128 - r, ln)
                    nc.sync.dma_start(pk_sb[t][r:r + n, :],
                                      attn_sb[h][off:off + n, :])
                    g0 += n
                    off += n
                    ln -= n

        # ------------ phase 1: qkv + interleaved per-head attention ----
        with ExitStack() as p1:
            hid_pool = p1.enter_context(tc.tile_pool(name="hid", bufs=1))
            w_pool = p1.enter_context(tc.tile_pool(name="wstream", bufs=3))
            # hid loaded in k-pairs (halves the DMA count)
            hid_pairs = [hid_pool.tile([128, 2 * S], bf16, tag=f"hid{p}",
                                       name=f"hid{p}") for p in range(NK // 2)]
            hid_of = lambda k: hid_pairs[k // 2][:, (k % 2) * S:
                                                 (k % 2) * S + S]
            wj_tiles = {}

            def load_wj(j):
                # j-pair granularity: one DMA covers tiles j, j+1
                if j in wj_tiles:
                    return
                j0 = j - j % 2
                wp2 = w_pool.tile([128, 2 * NK * 128], bf16, tag="wj",
                                  name=f"wj{j0}")
                nn = min(2, NJ - j0)
                nc.sync.dma_start(
                    wp2[:, 0:nn * NK * 128],
                    wqkvT[:, j0 * NK * 128:(j0 + nn) * NK * 128])
                for jj in range(j0, j0 + nn):
                    wj_tiles[jj] = wp2[:, (jj - j0) * NK * 128:
                                       (jj - j0 + 1) * NK * 128]

            # DMA order on the sync queue: hid k0, wj0, hid k1, wj1 — the
            # fused j0/j1 pair consumes hid tiles as they arrive.
            wp01 = w_pool.tile([128, 2 * NK * 128], bf16, tag="wj",
                               name="wj0")
            nc.scalar.dma_start(wp01[:, 0:NK * 128], wqkvT[:, 0:NK * 128])
            nc.sync.dma_start(hid_pairs[0][:, 0:S], hiddenT[:, 0:S])
            nc.scalar.dma_start(wp01[:, NK * 128:], wqkvT[:, NK * 128:
                                                          2 * NK * 128])
            nc.sync.dma_start(hid_pairs[0][:, S:2 * S], hiddenT[:, S:2 * S])
            wj_tiles[0] = wp01[:, 0:NK * 128]
            wj_tiles[1] = wp01[:, NK * 128:2 * NK * 128]
            for p in range(1, NK // 2):
                nc.sync.dma_start(hid_pairs[p][:],
                                  hiddenT[:, 2 * p * S:(2 * p + 2) * S])
            load_wj(2)
            nc.sync.dma_start(bias_sb[:], bias2d[:])
            nc.sync.dma_start(cos_sb[0:HD, :], cosP[:])
            nc.sync.dma_start(sin_sb[0:HD, :], sinP[:])
            nc.sync.dma_start(vaug_all[:], vinit[:])

            def after_j(j):
                if j + 1 < NJ and (j + 1) not in wj_tiles:
                    load_wj(j + 1)
                if j == 10:
                    for t in range(5):
                        nc.sync.dma_start(wp_sb[t][:],
                                          wprojT[t * 128:(t + 1) * 128, :])
                for h in vaug_at[j]:
                    emit_vaug(h)
                for h, sec in rope_at[j]:
                    emit_rope(h, sec, late=(j >= 13))

            # fused j0/j1: k-outer so the PE starts as soon as hid0 lands
            ps0 = psum.tile([128, S], f32, tag="wv", name="ps0")
            ps1 = psum.tile([128, S], f32, tag="po", name="ps1")
            for k in range(NK):
                for ps, j in ((ps0, 0), (ps1, 1)):
                    for c in (0, 512):
                        nc.tensor.matmul(
                            ps[:, c:c + 512],
                            wj_tiles[j][:, k * 128:(k + 1) * 128],
                            hid_of(k)[:, c:c + 512],
                            start=(k == 0), stop=(k == NK - 1))
            for ps, j in ((ps0, 0), (ps1, 1)):
                if bias_zero:
                    nc.vector.tensor_copy(qkv_sb[j][:, :], ps[:, :])
                else:
                    nc.scalar.activation(qkv_sb[j][:, :], ps[:, :],
                                         AF.Identity,
                                         bias=bias_sb[:, j:j + 1])
                after_j(j)

            # attention blocks due at `ready+2` consume (weave) that j's
            # matmuls; j14 stays plain so act14 lands before the last ropes
            emitted = set()
            j = 2
            while j < NJ:
                due_now = [h for h in range(n_h) if h not in emitted
                           and min(ready[h] + 2, NJ - 1) <= j]
                if due_now and j < NJ - 1:
                    h = due_now[0]
                    if j not in wj_tiles:
                        load_wj(j)
                    emit_attn(h, weave=j)
                    emitted.add(h)
                    after_j(j)
                    j += 1
                    continue
                if j not in wj_tiles:
                    load_wj(j)
                sts = [psum.tile([128, 512], f32, tag=f"st{i}", name="qs")
                       for i in range(2)]
                for ci, c in enumerate((0, 512)):
                    for k in range(NK):
                        nc.tensor.matmul(
                            sts[ci][:, :],
                            wj_tiles[j][:, k * 128:(k + 1) * 128],
                            hid_of(k)[:, c:c + 512],
                            start=(k == 0), stop=(k == NK - 1))
                    if bias_zero:
                        nc.vector.tensor_copy(qkv_sb[j][:, c:c + 512],
                                              sts[ci][:, :])
                    else:
                        nc.scalar.activation(qkv_sb[j][:, c:c + 512],
                                             sts[ci][:, :], AF.Identity,
                                             bias=bias_sb[:, j:j + 1])
                after_j(j)
                j += 1
        # post-loop attention/repack sits OUTSIDE the p1 pools' scope: the
        # pool-exit engine drains must not wait on the attention tail
        unemitted = [h for h in range(n_h) if h not in emitted]
        for i, h in enumerate(unemitted):
            if h in vaug_deferred:
                emit_vaug(h)   # after the previous attn block so the
                # in-order PE isn't blocked waiting on the last act
            emit_attn(h, po_tag="wv" if i % 2 == 0 else "po")
        for h in range(n_h):
            emit_repack(h)

        if KERNEL_DEBUG:
            for j in range(NJ):
                nc.sync.dma_start(dbg_qkv[:, j * S:(j + 1) * S],
                                    qkv_sb[j][:])
            i_ = 0
            for h in range(n_h):
                for sec in ("q", "k"):
                    nc.sync.dma_start(dbg_rot[:, i_ * S:(i_ + 1) * S],
                                        rot_sb[(sec, h)][:])
                    i_ += 1
            for h in range(n_h):
                nc.sync.dma_start(dbg_attn[:, h * S:(h + 1) * S],
                                    attn_sb[h][:])

        # ------------ phase 2: projection ---------------------------
        with ExitStack() as p5:
            out_pool = p5.enter_context(tc.tile_pool(name="outsb", bufs=1))
            wv_sl = psum.tile([128, S], f32, tag="wv", name="pjwv")
            po_sl = psum.tile([128, S], f32, tag="po", name="pjpo")
            slots = []

            def slot(i):
                i = i % 8
                if i < 4:
                    return psum.tile([128, 512], f32,
                                     tag=["st0", "st1", "tp0", "tp1"][i],
                                     name="pj")
                if i < 6:
                    return wv_sl[:, (i - 4) * 512:(i - 3) * 512]
                return po_sl[:, (i - 6) * 512:(i - 5) * 512]

            chains = [(j, c) for j in range(D // 128) for c in (0, 512)]
            slot_of = {}

            def open_partA(u):
                j, c = chains[u]
                ps = slot(u)
                slot_of[u] = ps
                for kt in range(4):
                    nc.tensor.matmul(ps[:, 0:512],
                                     wp_sb[kt][:, j * 128:(j + 1) * 128],
                                     pk_sb[kt][:, c:c + 512],
                                     start=(kt == 0), stop=False)

            for u in range(8):
                open_partA(u)
            for u, (j, c) in enumerate(chains):
                if c == 0:
                    ob = out_pool.tile([128, S], bf16, tag=f"ob{j % 4}",
                                       name="ob")
                ps = slot_of.pop(u)
                nc.tensor.matmul(ps[:, 0:512],
                                 wp_sb[4][:, j * 128:(j + 1) * 128],
                                 pk_sb[4][:, c:c + 512],
                                 start=False, stop=True)
                if u % 2 == 0:
                    nc.scalar.activation(ob[:, c:c + 512], ps[:, 0:512],
                                         AF.Identity)
                else:
                    nc.vector.tensor_copy(ob[:, c:c + 512], ps[:, 0:512])
                if u + 8 < len(chains):
                    open_partA(u + 8)
                if c == 512:
                    eng = (nc.sync, nc.gpsimd)[j % 2]
                    eng.dma_start(outT[j * 128:(j + 1) * 128, :], ob[:, :])

    nc.compile()
    return nc


def _pack_w_a2(Wqkv, bqkv, heads):
    """Dense 15-tile per-head-contiguous packing (see _layout_a2)."""
    import ml_dtypes
    v_tile, q_rows, k_rows, _ = _layout_a2()
    perm = np.zeros((NJ * 128,), np.int64)
    scl = np.ones((NJ * 128,), np.float32)
    used = np.zeros((NJ * 128,), bool)
    for i, h in enumerate(heads):
        for d in range(HD):
            g = 128 * v_tile[i] + d
            perm[g] = 2 * D + h * HD + d  # v
            used[g] = True
        for d, g in enumerate(q_rows[i]):
            perm[g] = h * HD + d
            scl[g] = SCALE
            used[g] = True
        for d, g in enumerate(k_rows[i]):
            perm[g] = D + h * HD + d
            used[g] = True
    W = Wqkv[perm] * scl[:, None]
    W[~used] = 0.0
    b = bqkv[perm] * scl
    b[~used] = 0.0
    # wqkvT host layout: [128, j, k, 128]; [p, j, k, m] = W.T[k*128+p, j*128+m]
    WT = np.ascontiguousarray(W.T)  # [1280, 1920]
    wt = WT.reshape(NK, 128, NJ, 128).transpose(1, 2, 0, 3)
    wt = np.ascontiguousarray(wt.reshape(128, NJ * NK * 128))
    bias2d = np.ascontiguousarray(b.reshape(NJ, 128).T)
    return wt.astype(ml_dtypes.bfloat16), bias2d


def _pack_wproj(Wproj, heads):
    """Rows of Wproj.T for this core's head dims, stacked per head."""
    W = np.zeros((len(heads) * HD, Wproj.shape[0]), np.float32)
    for i, h in enumerate(heads):
        W[i * HD:(i + 1) * HD] = Wproj[:, h * HD:(h + 1) * HD].T
    return W


def _pack_wproj_a2(Wproj, heads):
    """Mode A proj rows match the device pk layout: heads 0-5 at 80h,
    h6 split 480:512 (d0:32) + 592:640 (d32:80), h7 at 512:592."""
    W = np.zeros((640, Wproj.shape[0]), np.float32)
    wt = lambda h, d0, d1: Wproj[:, heads[h] * HD + d0:
                                 heads[h] * HD + d1].T
    for i in range(6):
        W[i * HD:(i + 1) * HD] = wt(i, 0, HD)
    W[480:512] = wt(6, 0, 32)
    W[512:592] = wt(7, 0, HD)
    W[592:640] = wt(6, 32, HD)
    return W


_CACHE = {}


def _kernel_mode_a(hidden_states, cos, sin, Wqkv, bqkv, Wproj, bproj, S):
    import ml_dtypes
    from concourse import bass_utils

    n_h, S_core = H // 2, S // 4
    bz = not np.any(bqkv)
    key = ("A2", bz)
    if key not in _CACHE:
        _CACHE[key] = _build_program_a2(bias_zero=bz)
    nc = _CACHE[key]

    bf = ml_dtypes.bfloat16
    hiddenT = np.ascontiguousarray(hidden_states.T)  # [D, S]

    vinit = np.zeros((128, 8 * NTT, VW), np.float32)
    vinit[:, :, 96] = 1.0
    vinit = np.ascontiguousarray(vinit.reshape(128, 8 * NTT * VW)).astype(bf)

    in_maps = []
    meta = []
    for g in range(2):
        heads = list(range(g * n_h, (g + 1) * n_h))
        wt, b2 = _pack_w_a2(Wqkv, bqkv, heads)
        wprojT = _pack_wproj_a2(Wproj, heads).astype(bf)
        for s in range(4):
            sl = slice(s * S_core, (s + 1) * S_core)
            hseg = hiddenT[:, sl]  # [1280, 1024]
            hid_t = np.ascontiguousarray(
                hseg.reshape(NK, 128, S_core).transpose(1, 0, 2)
                .reshape(128, NK * S_core)).astype(bf)
            cosP = np.ascontiguousarray(cos[sl].T).astype(bf)
            sinP = np.concatenate(
                [-sin[sl].T[0:BLK], sin[sl].T[BLK:HD]], axis=0)
            sinP = np.ascontiguousarray(sinP).astype(bf)
            in_maps.append({
                "hiddenT": hid_t,
                "wqkvT": wt,
                "bias2d": b2,
                "cosP": cosP,
                "sinP": sinP,
                "wprojT": wprojT,
                "vinit": vinit,
            })
            meta.append((g, s))
    res = bass_utils.run_bass_kernel_spmd(nc, in_maps,
                                          core_ids=list(range(N_CORES)))
    out = np.zeros((D, S), np.float32)
    for c, (g, s) in enumerate(meta):
        out[:, s * S_core:(s + 1) * S_core] += \
            res.results[c]["outT"].astype(np.float32)
    return out


# ---------------------------------------------------------------------------
# mode C (non-uniform segments): fp32r 8-way head-parallel fallback
# ---------------------------------------------------------------------------

def _pack_layout(n_h):
    """Pack per-core qkv dims as 40-row blocks, 3 per 128-row tile (8 pad).

    Each tile holds one v-block at row 0 (PE transpose operands must start
    at a 32-aligned partition) and two q/k blocks at rows 40 and 80.
    Returns pos[(sec, h, half)] = (tile, row) and the number of tiles.
    """
    ntiles = 2 * n_h
    pos = {}
    for h in range(n_h):
        for half in (0, 1):
            pos[("v", h, half)] = (2 * h + half, 0)
    qk = [("q", h, half) for h in range(n_h) for half in (0, 1)]
    qk += [("k", h, half) for h in range(n_h) for half in (0, 1)]
    for j, blk in enumerate(qk):
        pos[blk] = (j // 2, BLK + BLK * (j % 2))
    return pos, ntiles


def _build_program(n_h, S_core, segs_local, resident_hidden):
    """Mode C SPMD program (fp32r)."""
    import concourse.mybir as mybir
    import concourse.tile as tile
    from concourse import bacc
    from concourse.masks import make_identity
    from contextlib import ExitStack

    f32 = mybir.dt.float32
    mm_dt = getattr(mybir.dt, MM_DT_NAME)
    AF = mybir.ActivationFunctionType

    k_proj = n_h
    pos, n_mtiles = _pack_layout(n_h)
    dims_pad = n_mtiles * 128
    VWc = 97

    t_tiles = []
    for si, (a, e) in enumerate(segs_local):
        t = a
        while t < e:
            t_tiles.append((si, t, min(t + 128, e)))
            t += 128
    n_tt = len(t_tiles)

    nc = bacc.Bacc("TRN2", target_bir_lowering=False, debug=False,
                   enable_asserts=False, num_devices=N_CORES)

    hiddenT = nc.dram_tensor("hiddenT", [128, NK * S_core], mm_dt,
                             kind="ExternalInput").ap()
    wqkvT = nc.dram_tensor("wqkvT", [128, NK * dims_pad], mm_dt,
                           kind="ExternalInput").ap()
    bias2d = nc.dram_tensor("bias2d", [128, n_mtiles], f32,
                            kind="ExternalInput").ap()
    cosP = nc.dram_tensor("cosP", [128, S_core], mm_dt,
                          kind="ExternalInput").ap()
    sin2P = nc.dram_tensor("sin2P", [128, S_core], mm_dt,
                           kind="ExternalInput").ap()
    wprojT = nc.dram_tensor("wprojT", [n_h * HD, D], mm_dt,
                            kind="ExternalInput").ap()
    vinit = nc.dram_tensor("vinit", [128, n_tt * (VWc - HD)], mm_dt,
                           kind="ExternalInput").ap()
    outT = nc.dram_tensor("outT", [D, S_core], f32, kind="ExternalOutput").ap()

    def r_(ap):
        return ap.bitcast(mm_dt)

    BC = 1024
    big_chunks = [(c, min(c + BC, S_core)) for c in range(0, S_core, BC)]

    def halves(c0, c1):
        out = []
        q = c0
        while q < c1:
            out.append((q, min(q + 512, c1)))
            q = q + 512
        return out

    with tile.TileContext(nc) as tc, ExitStack() as ctx:
        persist = ctx.enter_context(tc.tile_pool(name="persist", bufs=1))
        ident = persist.tile([128, 128], f32, tag="ident", name="ident")
        make_identity(nc, ident[:])
        bias_sb = persist.tile([128, n_mtiles], f32, tag="bias", name="bias")
        nc.sync.dma_start(bias_sb[:], bias2d[:])

        psum_all_cm = tc.tile_pool(name="psum_all", bufs=1, space="PSUM")
        psum_all = psum_all_cm.__enter__()
        qkv_pool = ctx.enter_context(tc.tile_pool(name="big", bufs=1))
        qkv_sb = [qkv_pool.tile([128, S_core], mm_dt, tag=f"qkvT{j}",
                                name=f"qkvT{j}") for j in range(n_mtiles)]
        rot_cm = tc.tile_pool(name="rot", bufs=1)
        rv = rot_cm.__enter__()
        rot_sb = {}
        for h in range(n_h):
            for sec in ("q", "k"):
                rot_sb[(sec, h)] = rv.tile([128, S_core], mm_dt,
                                           tag=f"rot_{sec}{h}",
                                           name=f"rot_{sec}{h}")
        RC = 1024
        rope_cm = tc.tile_pool(name="rope_scr", bufs=2)
        rope_scr = rope_cm.__enter__()

        with ExitStack() as p1:
            hidden3 = hiddenT.rearrange("p (k s) -> p k s", k=NK)
            w3 = wqkvT.rearrange("p (k m) -> p k m", k=NK)
            w_pool = p1.enter_context(tc.tile_pool(name="wres", bufs=1))
            w_sb = [w_pool.tile([128, dims_pad], mm_dt, tag=f"w{k}",
                                name=f"w{k}") for k in range(NK)]
            for k in range(NK):
                nc.sync.dma_start(w_sb[k][:], w3[:, k, :])
            hid_pool = p1.enter_context(tc.tile_pool(name="hidstream",
                                                     bufs=3))
            n4 = n_mtiles // 4
            for (h0, h1) in halves(0, S_core):
                hw = h1 - h0
                for q4 in range(n4):
                    ps01 = psum_all.tile([128, BC], f32, tag="t0",
                                         name="ps01")
                    ps23 = psum_all.tile([128, BC], f32, tag="t1",
                                         name="ps23")
                    pj_of = lambda j: (ps01 if j % 4 < 2 else ps23,
                                       (j % 2) * 512)
                    for k in range(NK):
                        ht = hid_pool.tile([128, 512], mm_dt, tag="hidc",
                                           name="hidc")
                        nc.sync.dma_start(ht[:, :hw], hidden3[:, k, h0:h1])
                        for j in range(q4 * 4, q4 * 4 + 4):
                            psj, co = pj_of(j)
                            nc.tensor.matmul(
                                psj[:, co:co + hw],
                                r_(w_sb[k][:, j * 128:(j + 1) * 128]),
                                r_(ht[:, :hw]),
                                start=(k == 0), stop=(k == NK - 1))
                    for j in range(q4 * 4, q4 * 4 + 4):
                        psj, co = pj_of(j)
                        nc.scalar.activation(qkv_sb[j][:, h0:h1],
                                             psj[:, co:co + hw], AF.Identity,
                                             bias=bias_sb[:, j:j + 1])

        psum_all_cm.__exit__(None, None, None)
        ps_att = ctx.enter_context(tc.tile_pool(name="ps_att", bufs=1,
                                                space="PSUM"))

        stg = {}
        for nm in ("sa0", "sa1", "sb0", "sb1"):
            stg[nm] = rope_scr.tile([128, RC], mm_dt, tag=nm, name=nm, bufs=1)
        pair_i = 0
        for ci, f0 in enumerate(range(0, S_core, RC)):
            f1 = min(f0 + RC, S_core)
            fs = f1 - f0
            cos_sb = rope_scr.tile([128, RC], mm_dt, tag="cos", name="cos",
                                   bufs=1)
            sin_sb = rope_scr.tile([128, RC], mm_dt, tag="sin", name="sin",
                                   bufs=1)
            nc.scalar.dma_start(cos_sb[:, :fs], cosP[:, f0:f1])
            nc.scalar.dma_start(sin_sb[:, :fs], sin2P[:, f0:f1])
            if ci == 0:
                for nm in stg:
                    nc.scalar.dma_start(stg[nm][BLK:64, :], cos_sb[BLK:64, :])
            for h in range(n_h):
                for sec in ("q", "k"):
                    lo_t, lo_r = pos[(sec, h, 0)]
                    hi_t, hi_r = pos[(sec, h, 1)]
                    x = qkv_sb[lo_t]
                    dst = rot_sb[(sec, h)]
                    stga = stg[f"sa{pair_i % 2}"]
                    stgb = stg[f"sb{pair_i % 2}"]
                    nc.scalar.dma_start(stga[0:BLK, :fs],
                                        x[lo_r:lo_r + BLK, f0:f1])
                    nc.scalar.dma_start(stga[64:64 + BLK, :fs],
                                        x[hi_r:hi_r + BLK, f0:f1])
                    nc.scalar.dma_start(stgb[0:BLK, :fs],
                                        x[hi_r:hi_r + BLK, f0:f1])
                    nc.scalar.dma_start(stgb[64:64 + BLK, :fs],
                                        x[lo_r:lo_r + BLK, f0:f1])
                    nc.vector.tensor_mul(dst[0:104, f0:f1], stga[0:104, :fs],
                                         cos_sb[0:104, :fs])
                    eng = nc.gpsimd if pair_i % 2 == 0 else nc.vector
                    eng.tensor_mul(stgb[0:104, :fs], stgb[0:104, :fs],
                                   sin_sb[0:104, :fs])
                    nc.vector.tensor_add(dst[0:104, f0:f1], dst[0:104, f0:f1],
                                         stgb[0:104, :fs])
                    pair_i += 1
        rope_cm.__exit__(None, None, None)

        vaug_cm = tc.tile_pool(name="vaug", bufs=1)
        vaug_pool = vaug_cm.__enter__()
        vaug_sb = [vaug_pool.tile([128, n_tt * VWc], mm_dt, tag=f"vaug{h}",
                                  name=f"vaug{h}") for h in range(n_h)]
        vinit3 = vinit.rearrange("p (t c) -> p t c", c=VWc - HD)
        for h in range(n_h):
            nc.sync.dma_start(
                vaug_sb[h].rearrange("p (t c) -> p t c", c=VWc)[:, :, HD:VWc],
                vinit3[:, :, :])
        GRP = 4

        def emit_vaug(h):
            gi = 0
            while gi < n_tt:
                hi_g = min(gi + GRP, n_tt)
                if all(t_tiles[g][2] - t_tiles[g][1] == 128
                       for g in range(gi, hi_g)):
                    grp = list(range(gi, hi_g))
                else:
                    grp = [gi]
                ng = len(grp)
                tp = ps_att.tile([128, GRP * HD], f32, tag="tp", name="tp")
                for x, g in enumerate(grp):
                    si, t0, t1 = t_tiles[g]
                    sz = t1 - t0
                    for half in (0, 1):
                        vt, vr = pos[("v", h, half)]
                        nc.tensor.transpose(
                            tp[:sz, x * HD + half * BLK:
                               x * HD + (half + 1) * BLK],
                            qkv_sb[vt][0:BLK, t0:t1].bitcast(f32),
                            ident[:BLK, :BLK])
                sz0 = t_tiles[grp[0]][2] - t_tiles[grp[0]][1]
                dst = vaug_sb[h].rearrange("p (t c) -> p t c", c=VWc)
                src_ap = tp.rearrange("p (t c) -> p t c", c=HD)
                nc.vector.tensor_copy(dst[:sz0, grp[0]:grp[0] + ng, 0:HD],
                                      src_ap[:sz0, 0:ng, :])
                gi += ng

        attn_sb = [qkv_pool.tile([128, S_core], mm_dt, tag=f"qkvT{hh}",
                                 name=f"attnT{hh}") for hh in range(n_h)]

        seg_ttiles = {}
        for ti, (si, t0, t1) in enumerate(t_tiles):
            seg_ttiles.setdefault(si, []).append((ti, t0, t1))

        BA = 512
        with ExitStack() as p4:
            pt_pool = p4.enter_context(tc.tile_pool(name="pt", bufs=3))
            nrm_pool = p4.enter_context(tc.tile_pool(name="nrm", bufs=2))
            unit_box = [0]

            def emit_attention(h, si, a, e):
                qT = rot_sb[("q", h)]
                kT = rot_sb[("k", h)]
                q = a
                while q < e:
                    q0, q1 = q, min(q + BA, e)
                    qs = q1 - q0
                    po = ps_att.tile([128, BA], f32,
                                     tag=f"po{unit_box[0] % 2}", name="pv")
                    tts = seg_ttiles[si]
                    for idx, (ti, t0, t1) in enumerate(tts):
                        sz = t1 - t0
                        ps = ps_att.tile([128, BA], f32, tag=f"st{idx % 2}",
                                         name="st")
                        nc.tensor.matmul(ps[:sz, :qs], r_(kT[0:104, t0:t1]),
                                         r_(qT[0:104, q0:q1]),
                                         start=True, stop=True)
                        pt = pt_pool.tile([128, BA], mm_dt, tag="pt",
                                          name="pt")
                        nc.scalar.activation(pt[:sz, :qs], ps[:sz, :qs],
                                             AF.Exp)
                        nc.tensor.matmul(
                            po[:VWc, :qs],
                            r_(vaug_sb[h][:sz, ti * VWc:(ti + 1) * VWc]),
                            r_(pt[:sz, :qs]),
                            start=(idx == 0), stop=(idx == len(tts) - 1))
                    rc = nrm_pool.tile([128, BA], f32, tag="rc", name="rc")
                    nc.vector.tensor_copy(rc[96:97, :qs], po[96:97, :qs])
                    nc.sync.dma_start(rc[0:1, :qs], rc[96:97, :qs])
                    nc.vector.reciprocal(rc[0:1, :qs], rc[0:1, :qs])
                    bc = nrm_pool.tile([128, BA], mm_dt, tag="bc", name="bc")
                    nc.gpsimd.partition_broadcast(
                        bc[0:HD, :qs], rc[0:1, :qs].bitcast(mm_dt))
                    nc.vector.tensor_mul(attn_sb[h][0:HD, q0:q1],
                                         po[0:HD, :qs], bc[0:HD, :qs])
                    unit_box[0] += 1
                    q = q1

            for h in range(n_h):
                emit_vaug(h)
            for si, (a, e) in enumerate(segs_local):
                for h in range(n_h):
                    emit_attention(h, si, a, e)

        vaug_cm.__exit__(None, None, None)
        rot_cm.__exit__(None, None, None)

        with ExitStack() as p5:
            wp_pool = p5.enter_context(tc.tile_pool(name="wp", bufs=1))
            wp_sb = []
            for kt in range(k_proj):
                t = wp_pool.tile([HD, D], mm_dt, tag=f"wp{kt}", name=f"wp{kt}")
                nc.sync.dma_start(t[:], wprojT[kt * HD:(kt + 1) * HD, :])
                wp_sb.append(t)
            out_pool = p5.enter_context(tc.tile_pool(name="outsb", bufs=3))
            for (c0, c1) in big_chunks:
                cs = c1 - c0
                for j in range(D // 128):
                    ob = out_pool.tile([128, BC], f32, tag="ob", name="ob")
                    for (h0, h1) in halves(c0, c1):
                        ps = ps_att.tile([128, 512], f32, tag=f"st{j % 2}",
                                         name="pj")
                        for kt in range(k_proj):
                            nc.tensor.matmul(
                                ps[:, :h1 - h0],
                                r_(wp_sb[kt][:, j * 128:(j + 1) * 128]),
                                r_(attn_sb[kt][0:HD, h0:h1]),
                                start=(kt == 0), stop=(kt == k_proj - 1))
                        if j % 2 == 0:
                            nc.vector.tensor_copy(ob[:, h0 - c0:h1 - c0],
                                                  ps[:, :h1 - h0])
                        else:
                            nc.scalar.activation(ob[:, h0 - c0:h1 - c0],
                                                 ps[:, :h1 - h0], AF.Identity)
                    nc.sync.dma_start(outT[j * 128:(j + 1) * 128, c0:c1],
                                      ob[:, :cs])

    nc.compile()
    return nc


def _pack_w(Wqkv, bqkv, heads, n_h):
    """Mode C packed qkv weights (q rows pre-scaled)."""
    pos, n_mtiles = _pack_layout(n_h)
    dims_pad = n_mtiles * 128
    W = np.zeros((dims_pad, D), np.float32)
    b = np.zeros((dims_pad,), np.float32)
    sec_off = {"q": 0, "k": D, "v": 2 * D}
    for i, h in enumerate(heads):
        for sec in ("q", "k", "v"):
            for half in (0, 1):
                t, r = pos[(sec, i, half)]
                src = sec_off[sec] + h * HD + half * BLK
                w = Wqkv[src:src + BLK, :]
                bb = bqkv[src:src + BLK]
                if sec == "q":
                    w = w * SCALE
                    bb = bb * SCALE
                W[t * 128 + r:t * 128 + r + BLK] = w
                b[t * 128 + r:t * 128 + r + BLK] = bb
    w_tiled = _tile_rows(np.ascontiguousarray(W.T))
    bias2d = np.ascontiguousarray(b.reshape(n_mtiles, 128).T)
    return w_tiled, bias2d


def _tile_rows(x):
    """[R, C] with R = nk*128 -> [128, nk*C] k-major tiling."""
    R, C = x.shape
    nk = R // 128
    return np.ascontiguousarray(
        x.reshape(nk, 128, C).transpose(1, 0, 2).reshape(128, nk * C))


def _pack_cos_sin(cos, sin):
    """Mode C cosP/sin2P [128, S]."""
    S = cos.shape[0]
    cosP = np.zeros((128, S), np.float32)
    sinP = np.zeros((128, S), np.float32)
    cosP[0:BLK] = cos.T[0:BLK]
    cosP[64:64 + BLK] = cos.T[BLK:HD]
    sinP[0:BLK] = -sin.T[0:BLK]
    sinP[64:64 + BLK] = sin.T[BLK:HD]
    return cosP, sinP


def kernel(hidden_states, cos, sin, Wqkv, bqkv, Wproj, bproj, cu_seqlens):
    sys.path.insert(0, "/opt/trn_rl_repo")
    from concourse import bass_utils

    hidden_states = np.asarray(hidden_states, np.float32)
    cos = np.asarray(cos, np.float32)
    sin = np.asarray(sin, np.float32)
    Wqkv = np.asarray(Wqkv, np.float32)
    bqkv = np.asarray(bqkv, np.float32)
    Wproj = np.asarray(Wproj, np.float32)
    bproj = np.asarray(bproj, np.float32)

    S, D_ = hidden_states.shape
    assert D_ == D
    segs = _segments(cu_seqlens, S)
    uniform = (S == 4096) and segs == [(i * S // 4, (i + 1) * S // 4)
                                       for i in range(4)]

    if uniform:
        out = _kernel_mode_a(hidden_states, cos, sin, Wqkv, bqkv, Wproj,
                             bproj, S)
    else:
        hiddenT = np.ascontiguousarray(hidden_states.T)
        cosP, sin2P = _pack_cos_sin(cos, sin)
        n_h, S_core = H // N_CORES, S
        key = ("C", S, tuple(np.asarray(cu_seqlens).tolist()))
        if key not in _CACHE:
            _CACHE[key] = _build_program(n_h, S_core, segs,
                                         resident_hidden=False)
        nc = _CACHE[key]
        n_tt = sum(-(-(e - a) // 128) for a, e in segs)
        vinit = np.zeros((128, n_tt, 17), np.float32)
        vinit[:, :, 16] = 1.0
        vinit = np.ascontiguousarray(vinit.reshape(128, n_tt * 17))
        hid_tiled = _tile_rows(hiddenT)
        in_maps = []
        for c in range(N_CORES):
            heads = list(range(c * n_h, (c + 1) * n_h))
            wt, b2 = _pack_w(Wqkv, bqkv, heads, n_h)
            in_maps.append({
                "hiddenT": hid_tiled,
                "wqkvT": wt,
                "bias2d": b2,
                "cosP": cosP,
                "sin2P": sin2P,
                "wprojT": _pack_wproj(Wproj, heads).astype(np.float32),
                "vinit": vinit,
            })
        res = bass_utils.run_bass_kernel_spmd(nc, in_maps,
                                              core_ids=list(range(N_CORES)))
        out = np.zeros((D, S), np.float32)
        for c in range(N_CORES):
            out += res.results[c]["outT"]

    return np.ascontiguousarray(out.T) + bproj[None, :]


# revision 40
# speedup vs baseline: 1.4200x; 1.0381x over previous
"""Trainium2 Bass kernel for Ernie4.5-VL vision attention (ragged segments).

Contract: kernel(**inputs) takes the FULL unsharded inputs (keyed as in
setup_inputs()) and returns the FULL [S, D] float32 output.

Mode A (uniform 4x1024 segments — the graded shape): 8 cores = 2 head
groups x 4 segments; per core 8 heads x 1024 tokens, everything in bf16
on the PE array (psum f32):

  qkvT = Wpack @ hidden.T     15 dense 128-row tiles (v 80-row blocks at
                              tile h rows 0:80, q/k packed tile-major)
  rope: dense [0:80] layout; the rotate-half operand is built with 2-4
        small SBUF DMAs per (q|k, head); rot = a*cos + b*sin on DVE/Pool
  per head: v transposes (PE) -> scoresT (PE) -> exp (ACT, 1024 wide)
        -> PV accumulate with ones column for the denominator ->
        reciprocal+broadcast+mul normalize
  attn heads DMA-repacked into 5 dense 128-row tiles; proj = 5 k-tiles
  Host does O(S*D) glue: packing, summing the 2 per-token partial
  projections, bias adds.

Engine budget per core (cost model): PE ~142us of matmul rows, ACT
~82us (exp + qkv bias copies), DVE ~40us, Pool ~30us, DMA ~19MB.
Emission interleaves attention per head into the qkv j-loop so every
engine streams; all DMAs avoid the ACT queue (exp lives there).

Mode C fallback (any other cu_seqlens): 8-way head parallel fp32r path
(unchanged from the earlier version of this kernel).
"""

import os
import sys

import numpy as np

H = 16
HD = 80
BLK = 40  # rotate_half half-width
SCALE = HD ** -0.5
N_CORES = 8
D = 1280
NK = D // 128  # contraction tiles for the qkv matmul
ATTN_STRIDE = 96  # head row pitch in the packed attention output (mode C)
MM_DT_NAME = os.environ.get("KERNEL_MM_DT", "float32r")  # mode C only
KERNEL_DEBUG = bool(int(os.environ.get("KERNEL_DEBUG", "0")))

# ---- mode A constants ----
NJ = 15          # dense qkv M tiles (1920 rows)
NTT = 8          # 128-row key tiles per 1024 segment
VW = 97          # vaug slot: 80 v dims + 16 pad + ones col at 96
SA_CORE = 1024   # tokens per core


def _segments(cu_seqlens, S):
    """Intervals matching reference's searchsorted(cu[1:], i, 'right')."""
    b = np.clip(np.sort(np.asarray(cu_seqlens, dtype=np.int64)[1:5]), 0, S)
    bounds = [0] + list(b) + [S]
    segs = []
    for a, e in zip(bounds[:-1], bounds[1:]):
        if e > a:
            segs.append((int(a), int(e)))
    return segs


# ---------------------------------------------------------------------------
# mode A: dense bf16 program
# ---------------------------------------------------------------------------

def _layout_a2():
    """Per-head-contiguous packing: head h owns global rows [240h, 240h+240).
    v sits at rows 0:80 of tile ceil(240h/128) (PE transpose needs a
    32-aligned non-crossing 80-row read); q then k fill the remaining
    window rows in ascending order (read via DMA, placement free).

    Returns (v_tile[h], q_rows[h], k_rows[h], ready_j[h]) where
    q_rows/k_rows are the 80 global rows of each section in dim order.
    """
    v_tile, q_rows, k_rows, ready = [], [], [], []
    for h in range(8):
        w0, w1 = 240 * h, 240 * (h + 1)
        th = -(-w0 // 128)
        vg0 = 128 * th
        qk = [g for g in range(w0, w1) if not (vg0 <= g < vg0 + HD)]
        v_tile.append(th)
        q_rows.append(qk[0:HD])
        k_rows.append(qk[HD:2 * HD])
        ready.append(max(th, qk[-1] // 128))
    return v_tile, q_rows, k_rows, ready


def _row_pieces(rows):
    """Split a list of global rows into (tile, row, len, rel_off) runs that
    are consecutive and stay within one 128-row tile."""
    out = []
    i = 0
    while i < len(rows):
        g = rows[i]
        n = 1
        while (i + n < len(rows) and rows[i + n] == g + n
               and (g + n) // 128 == g // 128):
            n += 1
        out.append((g // 128, g % 128, n, i))
        i += n
    return out


def _build_program_a2(bias_zero=True):
    """Mode A program: n_h=8 heads, S=1024 tokens per core, one segment."""
    import concourse.mybir as mybir
    import concourse.tile as tile
    from concourse import bacc
    from concourse.masks import make_identity
    from contextlib import ExitStack

    f32 = mybir.dt.float32
    bf16 = mybir.dt.bfloat16
    AF = mybir.ActivationFunctionType
    n_h, S = 8, SA_CORE

    nc = bacc.Bacc("TRN2", target_bir_lowering=False, debug=False,
                   enable_asserts=False, num_devices=N_CORES)

    hiddenT = nc.dram_tensor("hiddenT", [128, NK * S], bf16,
                             kind="ExternalInput").ap()
    wqkvT = nc.dram_tensor("wqkvT", [128, NJ * NK * 128], bf16,
                           kind="ExternalInput").ap()
    bias2d = nc.dram_tensor("bias2d", [128, NJ], f32,
                            kind="ExternalInput").ap()
    cosP = nc.dram_tensor("cosP", [HD, S], bf16, kind="ExternalInput").ap()
    sinP = nc.dram_tensor("sinP", [HD, S], bf16, kind="ExternalInput").ap()
    wprojT = nc.dram_tensor("wprojT", [n_h * HD, D], bf16,
                            kind="ExternalInput").ap()
    vinit = nc.dram_tensor("vinit", [128, 8 * NTT * VW], bf16,
                           kind="ExternalInput").ap()
    outT = nc.dram_tensor("outT", [D, S], bf16, kind="ExternalOutput").ap()
    if KERNEL_DEBUG:
        dbg_qkv = nc.dram_tensor("dbg_qkv", [128, NJ * S], f32,
                                 kind="ExternalOutput").ap()
        dbg_rot = nc.dram_tensor("dbg_rot", [128, 2 * n_h * S], f32,
                                 kind="ExternalOutput").ap()
        dbg_attn = nc.dram_tensor("dbg_attn", [128, n_h * S], f32,
                                  kind="ExternalOutput").ap()

    v_tile, q_rows, k_rows, ready = _layout_a2()
    rope_at = {j: [] for j in range(NJ)}   # (h, sec) at section readiness
    vaug_at = {j: [] for j in range(NJ)}   # h at v-tile readiness
    vaug_deferred = []
    for h in range(n_h):
        for sec, rows in (("q", q_rows[h]), ("k", k_rows[h])):
            rope_at[max(g // 128 for g in rows)].append((h, sec))
        if v_tile[h] >= NJ - 1:
            vaug_deferred.append(h)
        else:
            vaug_at[v_tile[h]].append(h)

    with tile.TileContext(nc) as tc, ExitStack() as ctx:
        persist = ctx.enter_context(tc.tile_pool(name="persist", bufs=1))
        ident = persist.tile([128, 128], bf16, tag="ident", name="ident")
        make_identity(nc, ident[:])
        bias_sb = persist.tile([128, NJ], f32, tag="bias", name="bias")
        cos_sb = persist.tile([128, S], bf16, tag="cos", name="cos")
        sin_sb = persist.tile([128, S], bf16, tag="sin", name="sin")

        psum = ctx.enter_context(tc.tile_pool(name="psum", bufs=1,
                                              space="PSUM"))
        qkv_pool = ctx.enter_context(tc.tile_pool(name="qkv", bufs=1))
        qkv_sb = [qkv_pool.tile([128, S], bf16, tag=f"qkvT{j}",
                                name=f"qkvT{j}") for j in range(NJ)]
        rot_pool = ctx.enter_context(tc.tile_pool(name="rot", bufs=1))
        rot_sb = {}
        for h in range(n_h):
            for sec in ("q", "k"):
                rot_sb[(sec, h)] = rot_pool.tile(
                    [128, S], bf16, tag=f"rot_{sec}{h}", name=f"rot_{sec}{h}")
        vaug_pool = ctx.enter_context(tc.tile_pool(name="vaug", bufs=1))
        vaug_all = vaug_pool.tile([128, n_h * NTT * VW], bf16, tag="vaug",
                                  name="vaug")
        vaug_sb = [vaug_all[:, h * NTT * VW:(h + 1) * NTT * VW]
                   for h in range(n_h)]
        attn_pool = ctx.enter_context(tc.tile_pool(name="attn", bufs=1))
        attn_sb = [attn_pool.tile([128, S], bf16, tag=f"attn{h}",
                                  name=f"attn{h}") for h in range(n_h)]
        stg_pool = ctx.enter_context(tc.tile_pool(name="stg", bufs=2))
        pt_pool = ctx.enter_context(tc.tile_pool(name="pt", bufs=4))
        nrm_pool = ctx.enter_context(tc.tile_pool(name="nrm", bufs=2))
        pk_pool = ctx.enter_context(tc.tile_pool(name="pk", bufs=1))
        pk_sb = [pk_pool.tile([128, S], bf16, tag=f"pk{t}", name=f"pk{t}")
                 for t in range(5)]
        wp_pool = ctx.enter_context(tc.tile_pool(name="wp", bufs=1))
        wp_sb = [wp_pool.tile([128, D], bf16, tag=f"wp{t}", name=f"wp{t}")
                 for t in range(5)]


        def emit_rope(h, sec, late=False):
            # sa (x in dim order) staged from qkv tiles on the SP queue;
            # sb = rotate_half(sa) built from sa with exactly two Pool
            # (SWDGE) DMAs. DMA count is precious: each dma_start holds its
            # queue for wait+transfer+sem (~1.1us fixed).
            rows = q_rows[h] if sec == "q" else k_rows[h]
            sa = stg_pool.tile([128, S], bf16, tag="sa", name=f"sa_{sec}{h}")
            sb = stg_pool.tile([128, S], bf16, tag="sb", name=f"sb_{sec}{h}")
            for t, r, ln, off in _row_pieces(rows):
                nc.sync.dma_start(sa[off:off + ln, :],
                                  qkv_sb[t][r:r + ln, :])
            if late:
                # endgame: stage sb straight from qkv tiles (parallel with
                # sa) so the last ropes don't pay the serial sa->sb hop
                for t, r, ln, off in _row_pieces(rows[BLK:]):
                    nc.sync.dma_start(sb[off:off + ln, :],
                                      qkv_sb[t][r:r + ln, :])
                for t, r, ln, off in _row_pieces(rows[:BLK]):
                    nc.sync.dma_start(sb[BLK + off:BLK + off + ln, :],
                                      qkv_sb[t][r:r + ln, :])
            else:
                nc.gpsimd.dma_start(sb[0:BLK, :], sa[BLK:HD, :])
                nc.gpsimd.dma_start(sb[BLK:HD, :], sa[0:BLK, :])
            rot = rot_sb[(sec, h)]
            nc.vector.tensor_mul(rot[0:HD, :], sa[0:HD, :], cos_sb[0:HD, :])
            nc.vector.tensor_mul(sb[0:HD, :], sb[0:HD, :], sin_sb[0:HD, :])
            nc.vector.tensor_add(rot[0:HD, :], rot[0:HD, :], sb[0:HD, :])

        def emit_vaug(h, on_act=False):
            # v transposes -> vaug; copies on DVE mid-run (ACT paces the
            # attention blocks), on ACT for the deferred last head (DVE is
            # serialized behind the previous norm chain there)
            for g in range(2):  # groups of 4 key tiles
                tp = psum.tile([128, 1024], bf16, tag=f"tp{g % 2}",
                               name="tp")
                for x in range(4):
                    ti = 4 * g + x
                    nc.tensor.transpose(
                        tp[:, x * HD:(x + 1) * HD],
                        qkv_sb[v_tile[h]][0:HD, ti * 128:(ti + 1) * 128],
                        ident[0:HD, 0:HD])
                dst = vaug_sb[h].rearrange("p (t c) -> p t c", c=VW)
                src_ap = tp[:, 0:4 * HD].rearrange("p (t c) -> p t c",
                                                   c=HD)[:, 0:4, :]
                if on_act:
                    nc.scalar.activation(dst[:, 4 * g:4 * g + 4, 0:HD],
                                         src_ap, AF.Identity)
                else:
                    nc.vector.tensor_copy(dst[:, 4 * g:4 * g + 4, 0:HD],
                                          src_ap)

        def emit_attn(h, weave=None, po_tag="po", shift_eng=None):
            # ---- scores -> exp -> PV over 16 (key tile, half) units ----
            # PV lags one unit so exp latency is hidden; the woven qkv
            # j-tile's matmuls fill the remaining PE slack.
            qT = rot_sb[("q", h)]
            kT = rot_sb[("k", h)]
            po = psum.tile([128, S], f32, tag=po_tag, name="po")
            if weave is not None:
                wv_ps = psum.tile([128, S], f32, tag="wv", name="wv")
                wv_mm = [(c, k) for c in (0, 512) for k in range(NK)]
                wv_done = 0
            units = [(ti, c) for ti in range(NTT) for c in (0, 512)]
            pend = {}

            def emit_pv(u):
                pt, ti, c = pend.pop(u)
                nc.tensor.matmul(
                    po[0:VW, c:c + 512],
                    vaug_sb[h][:, ti * VW:(ti + 1) * VW],
                    pt[:, :],
                    start=(ti == 0), stop=(ti == NTT - 1))

            for u, (ti, c) in enumerate(units):
                st = psum.tile([128, 512], f32, tag=f"st{u % 2}", name="st")
                nc.tensor.matmul(st[:, :], kT[0:HD, ti * 128:(ti + 1) * 128],
                                 qT[0:HD, c:c + 512], start=True, stop=True)
                pt = pt_pool.tile([128, 512], bf16, tag="pt", name="pt")
                nc.scalar.activation(pt[:, :], st[:, :], AF.Exp)
                pend[u] = (pt, ti, c)
                if weave is not None:
                    take = 2 if u % 4 == 0 else 1
                    for cc, k in wv_mm[wv_done:wv_done + take]:
                        nc.tensor.matmul(
                            wv_ps[:, cc:cc + 512],
                            wj_tiles[weave][:, k * 128:(k + 1) * 128],
                            hid_of(k)[:, cc:cc + 512],
                            start=(k == 0), stop=(k == NK - 1))
                    wv_done += take
                if u >= 2:
                    emit_pv(u - 2)
            if weave is not None and wv_done < len(wv_mm):
                for cc, k in wv_mm[wv_done:]:
                    nc.tensor.matmul(
                        wv_ps[:, cc:cc + 512],
                        wj_tiles[weave][:, k * 128:(k + 1) * 128],
                        hid_of(k)[:, cc:cc + 512],
                        start=(k == 0), stop=(k == NK - 1))
            emit_pv(len(units) - 2)
            emit_pv(len(units) - 1)
            if weave is not None:
                if bias_zero:
                    nc.vector.tensor_copy(qkv_sb[weave][:, :], wv_ps[:, :])
                else:
                    nc.scalar.activation(qkv_sb[weave][:, :], wv_ps[:, :],
                                         AF.Identity,
                                         bias=bias_sb[:, weave:weave + 1])

            # ---- normalize: recip straight off PSUM row 96, row shift
            # on the ACT queue (lands right after this head's exps) ----
            rc = nrm_pool.tile([128, S], f32, tag="rc", name="rc")
            nc.vector.reciprocal(rc[96:97, :], po[96:97, :])
            (shift_eng or nc.gpsimd).dma_start(rc[0:1, :], rc[96:97, :])
            bc = nrm_pool.tile([128, S], f32, tag="bc", name="bc")
            nc.gpsimd.partition_broadcast(bc[0:HD, :], rc[0:1, :])
            # the last head lands directly in the packed proj tile (rows
            # 0:80 of pk4) so proj is gated only by this normalize, not by
            # an extra repack DMA
            dst = pk_sb[4][0:HD, :] if h == n_h - 1 else attn_sb[h][0:HD, :]
            nc.vector.tensor_mul(dst, po[0:HD, :], bc[0:HD, :])

        def emit_repack(h):
            # dense proj k-tiles; emitted post-loop so these DMAs never
            # head-of-line-block the weight stream on the sync queue.
            # proj-row map: heads 0-5 at 80h; h6 split 480:512 + 592:640;
            # h7 occupies 512:592 (written in place by its normalize).
            if h == n_h - 1:
                return
            if h == 6:
                spans = [(480, 0, 32), (592, 32, 48)]
            else:
                spans = [(HD * h, 0, HD)]
            for g0, off, ln in spans:
                while ln > 0:
                    t, r = g0 // 128, g0 % 128
                    n = min(128 - r, ln)
                    nc.sync.dma_start(pk_sb[t][r:r + n, :],
                                      attn_sb[h][off:off + n, :])
                    g0 += n
                    off += n
                    ln -= n

        # ------------ phase 1: qkv + interleaved per-head attention ----
        with ExitStack() as p1:
            hid_pool = p1.enter_context(tc.tile_pool(name="hid", bufs=1))
            w_pool = p1.enter_context(tc.tile_pool(name="wstream", bufs=3))
            # hid loaded in k-pairs (halves the DMA count)
            hid_pairs = [hid_pool.tile([128, 2 * S], bf16, tag=f"hid{p}",
                                       name=f"hid{p}") for p in range(NK // 2)]
            hid_of = lambda k: hid_pairs[k // 2][:, (k % 2) * S:
                                                 (k % 2) * S + S]
            wj_tiles = {}

            def load_wj(j):
                # j-pair granularity: one DMA covers tiles j, j+1
                if j in wj_tiles:
                    return
                j0 = j - j % 2
                wp2 = w_pool.tile([128, 2 * NK * 128], bf16, tag="wj",
                                  name=f"wj{j0}")
                nn = min(2, NJ - j0)
                nc.sync.dma_start(
                    wp2[:, 0:nn * NK * 128],
                    wqkvT[:, j0 * NK * 128:(j0 + nn) * NK * 128])
                for jj in range(j0, j0 + nn):
                    wj_tiles[jj] = wp2[:, (jj - j0) * NK * 128:
                                       (jj - j0 + 1) * NK * 128]

            # DMA order on the sync queue: hid k0, wj0, hid k1, wj1 — the
            # fused j0/j1 pair consumes hid tiles as they arrive.
            wp01 = w_pool.tile([128, 2 * NK * 128], bf16, tag="wj",
                               name="wj0")
            nc.scalar.dma_start(wp01[:, 0:NK * 128], wqkvT[:, 0:NK * 128])
            nc.sync.dma_start(hid_pairs[0][:, 0:S], hiddenT[:, 0:S])
            nc.scalar.dma_start(wp01[:, NK * 128:], wqkvT[:, NK * 128:
                                                          2 * NK * 128])
            nc.sync.dma_start(hid_pairs[0][:, S:2 * S], hiddenT[:, S:2 * S])
            wj_tiles[0] = wp01[:, 0:NK * 128]
            wj_tiles[1] = wp01[:, NK * 128:2 * NK * 128]
            for p in range(1, NK // 2):
                nc.sync.dma_start(hid_pairs[p][:],
                                  hiddenT[:, 2 * p * S:(2 * p + 2) * S])
            load_wj(2)
            nc.sync.dma_start(bias_sb[:], bias2d[:])
            nc.sync.dma_start(cos_sb[0:HD, :], cosP[:])
            nc.sync.dma_start(sin_sb[0:HD, :], sinP[:])
            nc.sync.dma_start(vaug_all[:], vinit[:])

            def after_j(j):
                if j + 1 < NJ and (j + 1) not in wj_tiles:
                    load_wj(j + 1)
                if j == 10:
                    for t in range(5):
                        nc.sync.dma_start(wp_sb[t][:],
                                          wprojT[t * 128:(t + 1) * 128, :])
                for h in vaug_at[j]:
                    emit_vaug(h)
                for h, sec in rope_at[j]:
                    emit_rope(h, sec, late=(j >= 13))

            # fused j0/j1: k-outer so the PE starts as soon as hid0 lands
            ps0 = psum.tile([128, S], f32, tag="wv", name="ps0")
            ps1 = psum.tile([128, S], f32, tag="po", name="ps1")
            for k in range(NK):
                for ps, j in ((ps0, 0), (ps1, 1)):
                    for c in (0, 512):
                        nc.tensor.matmul(
                            ps[:, c:c + 512],
                            wj_tiles[j][:, k * 128:(k + 1) * 128],
                            hid_of(k)[:, c:c + 512],
                            start=(k == 0), stop=(k == NK - 1))
            for ps, j in ((ps0, 0), (ps1, 1)):
                if bias_zero:
                    nc.vector.tensor_copy(qkv_sb[j][:, :], ps[:, :])
                else:
                    nc.scalar.activation(qkv_sb[j][:, :], ps[:, :],
                                         AF.Identity,
                                         bias=bias_sb[:, j:j + 1])
                after_j(j)

            # attention blocks due at `ready+2` consume (weave) that j's
            # matmuls; j14 stays plain so act14 lands before the last ropes
            emitted = set()
            j = 2
            while j < NJ:
                due_now = [h for h in range(n_h) if h not in emitted
                           and min(ready[h] + 2, NJ - 1) <= j]
                if due_now and j < NJ - 1:
                    h = due_now[0]
                    if j not in wj_tiles:
                        load_wj(j)
                    emit_attn(h, weave=j)
                    emitted.add(h)
                    after_j(j)
                    j += 1
                    continue
                if j not in wj_tiles:
                    load_wj(j)
                sts = [psum.tile([128, 512], f32, tag=f"st{i}", name="qs")
                       for i in range(2)]
                for ci, c in enumerate((0, 512)):
                    for k in range(NK):
                        nc.tensor.matmul(
                            sts[ci][:, :],
                            wj_tiles[j][:, k * 128:(k + 1) * 128],
                            hid_of(k)[:, c:c + 512],
                            start=(k == 0), stop=(k == NK - 1))
                    if bias_zero:
                        nc.vector.tensor_copy(qkv_sb[j][:, c:c + 512],
                                              sts[ci][:, :])
                    else:
                        nc.scalar.activation(qkv_sb[j][:, c:c + 512],
                                             sts[ci][:, :], AF.Identity,
                                             bias=bias_sb[:, j:j + 1])
                after_j(j)
                j += 1
        # post-loop attention/repack sits OUTSIDE the p1 pools' scope: the
        # pool-exit engine drains must not wait on the attention tail
        unemitted = [h for h in range(n_h) if h not in emitted]
        for i, h in enumerate(unemitted):
            if h in vaug_deferred:
                emit_vaug(h)   # after the previous attn block so the
                # in-order PE isn't blocked waiting on the last act
            emit_attn(h, po_tag="wv" if i % 2 == 0 else "po")
        for h in range(n_h):
            emit_repack(h)

        if KERNEL_DEBUG:
            for j in range(NJ):
                nc.sync.dma_start(dbg_qkv[:, j * S:(j + 1) * S],
                                    qkv_sb[j][:])
            i_ = 0
            for h in range(n_h):
                for sec in ("q", "k"):
                    nc.sync.dma_start(dbg_rot[:, i_ * S:(i_ + 1) * S],
                                        rot_sb[(sec, h)][:])
                    i_ += 1
            for h in range(n_h):
                nc.sync.dma_start(dbg_attn[:, h * S:(h + 1) * S],
                                    attn_sb[h][:])

        # ------------ phase 2: projection ---------------------------
        with ExitStack() as p5:
            out_pool = p5.enter_context(tc.tile_pool(name="outsb", bufs=1))
            wv_sl = psum.tile([128, S], f32, tag="wv", name="pjwv")
            po_sl = psum.tile([128, S], f32, tag="po", name="pjpo")
            slots = []

            def slot(i):
                i = i % 8
                if i < 4:
                    return psum.tile([128, 512], f32,
                                     tag=["st0", "st1", "tp0", "tp1"][i],
                                     name="pj")
                if i < 6:
                    return wv_sl[:, (i - 4) * 512:(i - 3) * 512]
                return po_sl[:, (i - 6) * 512:(i - 5) * 512]

            chains = [(j, c) for j in range(D // 128) for c in (0, 512)]
            slot_of = {}

            def open_partA(u):
                j, c = chains[u]
                ps = slot(u)
                slot_of[u] = ps
                for kt in range(4):
                    nc.tensor.matmul(ps[:, 0:512],
                                     wp_sb[kt][:, j * 128:(j + 1) * 128],
                                     pk_sb[kt][:, c:c + 512],
                                     start=(kt == 0), stop=False)

            for u in range(8):
                open_partA(u)
            for u, (j, c) in enumerate(chains):
                if c == 0:
                    ob = out_pool.tile([128, S], bf16, tag=f"ob{j % 4}",
                                       name="ob")
                ps = slot_of.pop(u)
                nc.tensor.matmul(ps[:, 0:512],
                                 wp_sb[4][:, j * 128:(j + 1) * 128],
                                 pk_sb[4][:, c:c + 512],
                                 start=False, stop=True)
                if u % 2 == 0:
                    nc.scalar.activation(ob[:, c:c + 512], ps[:, 0:512],
                                         AF.Identity)
                else:
                    nc.vector.tensor_copy(ob[:, c:c + 512], ps[:, 0:512])
                if u + 8 < len(chains):
                    open_partA(u + 8)
                if c == 512:
                    eng = (nc.sync, nc.gpsimd)[j % 2]
                    eng.dma_start(outT[j * 128:(j + 1) * 128, :], ob[:, :])

    nc.compile()
    return nc


def _pack_w_a2(Wqkv, bqkv, heads):
    """Dense 15-tile per-head-contiguous packing (see _layout_a2)."""
    import ml_dtypes
    v_tile, q_rows, k_rows, _ = _layout_a2()
    perm = np.zeros((NJ * 128,), np.int64)
    scl = np.ones((NJ * 128,), np.float32)
    used = np.zeros((NJ * 128,), bool)
    for i, h in enumerate(heads):
        for d in range(HD):
            g = 128 * v_tile[i] + d
            perm[g] = 2 * D + h * HD + d  # v
            used[g] = True
        for d, g in enumerate(q_rows[i]):
            perm[g] = h * HD + d
            scl[g] = SCALE
            used[g] = True
        for d, g in enumerate(k_rows[i]):
            perm[g] = D + h * HD + d
            used[g] = True
    W = Wqkv[perm] * scl[:, None]
    W[~used] = 0.0
    b = bqkv[perm] * scl
    b[~used] = 0.0
    # wqkvT host layout: [128, j, k, 128]; [p, j, k, m] = W.T[k*128+p, j*128+m]
    WT = np.ascontiguousarray(W.T)  # [1280, 1920]
    wt = WT.reshape(NK, 128, NJ, 128).transpose(1, 2, 0, 3)
    wt = np.ascontiguousarray(wt.reshape(128, NJ * NK * 128))
    bias2d = np.ascontiguousarray(b.reshape(NJ, 128).T)
    return wt.astype(ml_dtypes.bfloat16), bias2d


def _pack_wproj(Wproj, heads):
    """Rows of Wproj.T for this core's head dims, stacked per head."""
    W = np.zeros((len(heads) * HD, Wproj.shape[0]), np.float32)
    for i, h in enumerate(heads):
        W[i * HD:(i + 1) * HD] = Wproj[:, h * HD:(h + 1) * HD].T
    return W


def _pack_wproj_a2(Wproj, heads):
    """Mode A proj rows match the device pk layout: heads 0-5 at 80h,
    h6 split 480:512 (d0:32) + 592:640 (d32:80), h7 at 512:592."""
    W = np.zeros((640, Wproj.shape[0]), np.float32)
    wt = lambda h, d0, d1: Wproj[:, heads[h] * HD + d0:
                                 heads[h] * HD + d1].T
    for i in range(6):
        W[i * HD:(i + 1) * HD] = wt(i, 0, HD)
    W[480:512] = wt(6, 0, 32)
    W[512:592] = wt(7, 0, HD)
    W[592:640] = wt(6, 32, HD)
    return W


_CACHE = {}


def _kernel_mode_a(hidden_states, cos, sin, Wqkv, bqkv, Wproj, bproj, S):
    import ml_dtypes
    from concourse import bass_utils

    n_h, S_core = H // 2, S // 4
    bz = not np.any(bqkv)
    key = ("A2", bz)
    if key not in _CACHE:
        _CACHE[key] = _build_program_a2(bias_zero=bz)
    nc = _CACHE[key]

    bf = ml_dtypes.bfloat16
    hiddenT = np.ascontiguousarray(hidden_states.T)  # [D, S]

    vinit = np.zeros((128, 8 * NTT, VW), np.float32)
    vinit[:, :, 96] = 1.0
    vinit = np.ascontiguousarray(vinit.reshape(128, 8 * NTT * VW)).astype(bf)

    in_maps = []
    meta = []
    for g in range(2):
        heads = list(range(g * n_h, (g + 1) * n_h))
        wt, b2 = _pack_w_a2(Wqkv, bqkv, heads)
        wprojT = _pack_wproj_a2(Wproj, heads).astype(bf)
        for s in range(4):
            sl = slice(s * S_core, (s + 1) * S_core)
            hseg = hiddenT[:, sl]  # [1280, 1024]
            hid_t = np.ascontiguousarray(
                hseg.reshape(NK, 128, S_core).transpose(1, 0, 2)
                .reshape(128, NK * S_core)).astype(bf)
            cosP = np.ascontiguousarray(cos[sl].T).astype(bf)
            sinP = np.concatenate(
                [-sin[sl].T[0:BLK], sin[sl].T[BLK:HD]], axis=0)
            sinP = np.ascontiguousarray(sinP).astype(bf)
            in_maps.append({
                "hiddenT": hid_t,
                "wqkvT": wt,
                "bias2d": b2,
                "cosP": cosP,
                "sinP": sinP,
                "wprojT": wprojT,
                "vinit": vinit,
            })
            meta.append((g, s))
    res = bass_utils.run_bass_kernel_spmd(nc, in_maps,
                                          core_ids=list(range(N_CORES)))
    out = np.zeros((D, S), np.float32)
    for c, (g, s) in enumerate(meta):
        out[:, s * S_core:(s + 1) * S_core] += \
            res.results[c]["outT"].astype(np.float32)
    return out


# ---------------------------------------------------------------------------
# mode C (non-uniform segments): fp32r 8-way head-parallel fallback
# ---------------------------------------------------------------------------

def _pack_layout(n_h):
    """Pack per-core qkv dims as 40-row blocks, 3 per 128-row tile (8 pad).

    Each tile holds one v-block at row 0 (PE transpose operands must start
    at a 32-aligned partition) and two q/k blocks at rows 40 and 80.
    Returns pos[(sec, h, half)] = (tile, row) and the number of tiles.
    """
    ntiles = 2 * n_h
    pos = {}
    for h in range(n_h):
        for half in (0, 1):
            pos[("v", h, half)] = (2 * h + half, 0)
    qk = [("q", h, half) for h in range(n_h) for half in (0, 1)]
    qk += [("k", h, half) for h in range(n_h) for half in (0, 1)]
    for j, blk in enumerate(qk):
        pos[blk] = (j // 2, BLK + BLK * (j % 2))
    return pos, ntiles


def _build_program(n_h, S_core, segs_local, resident_hidden):
    """Mode C SPMD program (fp32r)."""
    import concourse.mybir as mybir
    import concourse.tile as tile
    from concourse import bacc
    from concourse.masks import make_identity
    from contextlib import ExitStack

    f32 = mybir.dt.float32
    mm_dt = getattr(mybir.dt, MM_DT_NAME)
    AF = mybir.ActivationFunctionType

    k_proj = n_h
    pos, n_mtiles = _pack_layout(n_h)
    dims_pad = n_mtiles * 128
    VWc = 97

    t_tiles = []
    for si, (a, e) in enumerate(segs_local):
        t = a
        while t < e:
            t_tiles.append((si, t, min(t + 128, e)))
            t += 128
    n_tt = len(t_tiles)

    nc = bacc.Bacc("TRN2", target_bir_lowering=False, debug=False,
                   enable_asserts=False, num_devices=N_CORES)

    hiddenT = nc.dram_tensor("hiddenT", [128, NK * S_core], mm_dt,
                             kind="ExternalInput").ap()
    wqkvT = nc.dram_tensor("wqkvT", [128, NK * dims_pad], mm_dt,
                           kind="ExternalInput").ap()
    bias2d = nc.dram_tensor("bias2d", [128, n_mtiles], f32,
                            kind="ExternalInput").ap()
    cosP = nc.dram_tensor("cosP", [128, S_core], mm_dt,
                          kind="ExternalInput").ap()
    sin2P = nc.dram_tensor("sin2P", [128, S_core], mm_dt,
                           kind="ExternalInput").ap()
    wprojT = nc.dram_tensor("wprojT", [n_h * HD, D], mm_dt,
                            kind="ExternalInput").ap()
    vinit = nc.dram_tensor("vinit", [128, n_tt * (VWc - HD)], mm_dt,
                           kind="ExternalInput").ap()
    outT = nc.dram_tensor("outT", [D, S_core], f32, kind="ExternalOutput").ap()

    def r_(ap):
        return ap.bitcast(mm_dt)

    BC = 1024
    big_chunks = [(c, min(c + BC, S_core)) for c in range(0, S_core, BC)]

    def halves(c0, c1):
        out = []
        q = c0
        while q < c1:
            out.append((q, min(q + 512, c1)))
            q = q + 512
        return out

    with tile.TileContext(nc) as tc, ExitStack() as ctx:
        persist = ctx.enter_context(tc.tile_pool(name="persist", bufs=1))
        ident = persist.tile([128, 128], f32, tag="ident", name="ident")
        make_identity(nc, ident[:])
        bias_sb = persist.tile([128, n_mtiles], f32, tag="bias", name="bias")
        nc.sync.dma_start(bias_sb[:], bias2d[:])

        psum_all_cm = tc.tile_pool(name="psum_all", bufs=1, space="PSUM")
        psum_all = psum_all_cm.__enter__()
        qkv_pool = ctx.enter_context(tc.tile_pool(name="big", bufs=1))
        qkv_sb = [qkv_pool.tile([128, S_core], mm_dt, tag=f"qkvT{j}",
                                name=f"qkvT{j}") for j in range(n_mtiles)]
        rot_cm = tc.tile_pool(name="rot", bufs=1)
        rv = rot_cm.__enter__()
        rot_sb = {}
        for h in range(n_h):
            for sec in ("q", "k"):
                rot_sb[(sec, h)] = rv.tile([128, S_core], mm_dt,
                                           tag=f"rot_{sec}{h}",
                                           name=f"rot_{sec}{h}")
        RC = 1024
        rope_cm = tc.tile_pool(name="rope_scr", bufs=2)
        rope_scr = rope_cm.__enter__()

        with ExitStack() as p1:
            hidden3 = hiddenT.rearrange("p (k s) -> p k s", k=NK)
            w3 = wqkvT.rearrange("p (k m) -> p k m", k=NK)
            w_pool = p1.enter_context(tc.tile_pool(name="wres", bufs=1))
            w_sb = [w_pool.tile([128, dims_pad], mm_dt, tag=f"w{k}",
                                name=f"w{k}") for k in range(NK)]
            for k in range(NK):
                nc.sync.dma_start(w_sb[k][:], w3[:, k, :])
            hid_pool = p1.enter_context(tc.tile_pool(name="hidstream",
                                                     bufs=3))
            n4 = n_mtiles // 4
            for (h0, h1) in halves(0, S_core):
                hw = h1 - h0
                for q4 in range(n4):
                    ps01 = psum_all.tile([128, BC], f32, tag="t0",
                                         name="ps01")
                    ps23 = psum_all.tile([128, BC], f32, tag="t1",
                                         name="ps23")
                    pj_of = lambda j: (ps01 if j % 4 < 2 else ps23,
                                       (j % 2) * 512)
                    for k in range(NK):
                        ht = hid_pool.tile([128, 512], mm_dt, tag="hidc",
                                           name="hidc")
                        nc.sync.dma_start(ht[:, :hw], hidden3[:, k, h0:h1])
                        for j in range(q4 * 4, q4 * 4 + 4):
                            psj, co = pj_of(j)
                            nc.tensor.matmul(
                                psj[:, co:co + hw],
                                r_(w_sb[k][:, j * 128:(j + 1) * 128]),
                                r_(ht[:, :hw]),
                                start=(k == 0), stop=(k == NK - 1))
                    for j in range(q4 * 4, q4 * 4 + 4):
                        psj, co = pj_of(j)
                        nc.scalar.activation(qkv_sb[j][:, h0:h1],
                                             psj[:, co:co + hw], AF.Identity,
                                             bias=bias_sb[:, j:j + 1])

        psum_all_cm.__exit__(None, None, None)
        ps_att = ctx.enter_context(tc.tile_pool(name="ps_att", bufs=1,
                                                space="PSUM"))

        stg = {}
        for nm in ("sa0", "sa1", "sb0", "sb1"):
            stg[nm] = rope_scr.tile([128, RC], mm_dt, tag=nm, name=nm, bufs=1)
        pair_i = 0
        for ci, f0 in enumerate(range(0, S_core, RC)):
            f1 = min(f0 + RC, S_core)
            fs = f1 - f0
            cos_sb = rope_scr.tile([128, RC], mm_dt, tag="cos", name="cos",
                                   bufs=1)
            sin_sb = rope_scr.tile([128, RC], mm_dt, tag="sin", name="sin",
                                   bufs=1)
            nc.scalar.dma_start(cos_sb[:, :fs], cosP[:, f0:f1])
            nc.scalar.dma_start(sin_sb[:, :fs], sin2P[:, f0:f1])
            if ci == 0:
                for nm in stg:
                    nc.scalar.dma_start(stg[nm][BLK:64, :], cos_sb[BLK:64, :])
            for h in range(n_h):
                for sec in ("q", "k"):
                    lo_t, lo_r = pos[(sec, h, 0)]
                    hi_t, hi_r = pos[(sec, h, 1)]
                    x = qkv_sb[lo_t]
                    dst = rot_sb[(sec, h)]
                    stga = stg[f"sa{pair_i % 2}"]
                    stgb = stg[f"sb{pair_i % 2}"]
                    nc.scalar.dma_start(stga[0:BLK, :fs],
                                        x[lo_r:lo_r + BLK, f0:f1])
                    nc.scalar.dma_start(stga[64:64 + BLK, :fs],
                                        x[hi_r:hi_r + BLK, f0:f1])
                    nc.scalar.dma_start(stgb[0:BLK, :fs],
                                        x[hi_r:hi_r + BLK, f0:f1])
                    nc.scalar.dma_start(stgb[64:64 + BLK, :fs],
                                        x[lo_r:lo_r + BLK, f0:f1])
                    nc.vector.tensor_mul(dst[0:104, f0:f1], stga[0:104, :fs],
                                         cos_sb[0:104, :fs])
                    eng = nc.gpsimd if pair_i % 2 == 0 else nc.vector
                    eng.tensor_mul(stgb[0:104, :fs], stgb[0:104, :fs],
                                   sin_sb[0:104, :fs])
                    nc.vector.tensor_add(dst[0:104, f0:f1], dst[0:104, f0:f1],
                                         stgb[0:104, :fs])
                    pair_i += 1
        rope_cm.__exit__(None, None, None)

        vaug_cm = tc.tile_pool(name="vaug", bufs=1)
        vaug_pool = vaug_cm.__enter__()
        vaug_sb = [vaug_pool.tile([128, n_tt * VWc], mm_dt, tag=f"vaug{h}",
                                  name=f"vaug{h}") for h in range(n_h)]
        vinit3 = vinit.rearrange("p (t c) -> p t c", c=VWc - HD)
        for h in range(n_h):
            nc.sync.dma_start(
                vaug_sb[h].rearrange("p (t c) -> p t c", c=VWc)[:, :, HD:VWc],
                vinit3[:, :, :])
        GRP = 4

        def emit_vaug(h):
            gi = 0
            while gi < n_tt:
                hi_g = min(gi + GRP, n_tt)
                if all(t_tiles[g][2] - t_tiles[g][1] == 128
                       for g in range(gi, hi_g)):
                    grp = list(range(gi, hi_g))
                else:
                    grp = [gi]
                ng = len(grp)
                tp = ps_att.tile([128, GRP * HD], f32, tag="tp", name="tp")
                for x, g in enumerate(grp):
                    si, t0, t1 = t_tiles[g]
                    sz = t1 - t0
                    for half in (0, 1):
                        vt, vr = pos[("v", h, half)]
                        nc.tensor.transpose(
                            tp[:sz, x * HD + half * BLK:
                               x * HD + (half + 1) * BLK],
                            qkv_sb[vt][0:BLK, t0:t1].bitcast(f32),
                            ident[:BLK, :BLK])
                sz0 = t_tiles[grp[0]][2] - t_tiles[grp[0]][1]
                dst = vaug_sb[h].rearrange("p (t c) -> p t c", c=VWc)
                src_ap = tp.rearrange("p (t c) -> p t c", c=HD)
                nc.vector.tensor_copy(dst[:sz0, grp[0]:grp[0] + ng, 0:HD],
                                      src_ap[:sz0, 0:ng, :])
                gi += ng

        attn_sb = [qkv_pool.tile([128, S_core], mm_dt, tag=f"qkvT{hh}",
                                 name=f"attnT{hh}") for hh in range(n_h)]

        seg_ttiles = {}
        for ti, (si, t0, t1) in enumerate(t_tiles):
            seg_ttiles.setdefault(si, []).append((ti, t0, t1))

        BA = 512
        with ExitStack() as p4:
            pt_pool = p4.enter_context(tc.tile_pool(name="pt", bufs=3))
            nrm_pool = p4.enter_context(tc.tile_pool(name="nrm", bufs=2))
            unit_box = [0]

            def emit_attention(h, si, a, e):
                qT = rot_sb[("q", h)]
                kT = rot_sb[("k", h)]
                q = a
                while q < e:
                    q0, q1 = q, min(q + BA, e)
                    qs = q1 - q0
                    po = ps_att.tile([128, BA], f32,
                                     tag=f"po{unit_box[0] % 2}", name="pv")
                    tts = seg_ttiles[si]
                    for idx, (ti, t0, t1) in enumerate(tts):
                        sz = t1 - t0
                        ps = ps_att.tile([128, BA], f32, tag=f"st{idx % 2}",
                                         name="st")
                        nc.tensor.matmul(ps[:sz, :qs], r_(kT[0:104, t0:t1]),
                                         r_(qT[0:104, q0:q1]),
                                         start=True, stop=True)
                        pt = pt_pool.tile([128, BA], mm_dt, tag="pt",
                                          name="pt")
                        nc.scalar.activation(pt[:sz, :qs], ps[:sz, :qs],
                                             AF.Exp)
                        nc.tensor.matmul(
                            po[:VWc, :qs],
                            r_(vaug_sb[h][:sz, ti * VWc:(ti + 1) * VWc]),
                            r_(pt[:sz, :qs]),
                            start=(idx == 0), stop=(idx == len(tts) - 1))
                    rc = nrm_pool.tile([128, BA], f32, tag="rc", name="rc")
                    nc.vector.tensor_copy(rc[96:97, :qs], po[96:97, :qs])
                    nc.sync.dma_start(rc[0:1, :qs], rc[96:97, :qs])
                    nc.vector.reciprocal(rc[0:1, :qs], rc[0:1, :qs])
                    bc = nrm_pool.tile([128, BA], mm_dt, tag="bc", name="bc")
                    nc.gpsimd.partition_broadcast(
                        bc[0:HD, :qs], rc[0:1, :qs].bitcast(mm_dt))
                    nc.vector.tensor_mul(attn_sb[h][0:HD, q0:q1],
                                         po[0:HD, :qs], bc[0:HD, :qs])
                    unit_box[0] += 1
                    q = q1

            for h in range(n_h):
                emit_vaug(h)
            for si, (a, e) in enumerate(segs_local):
                for h in range(n_h):
                    emit_attention(h, si, a, e)

        vaug_cm.__exit__(None, None, None)
        rot_cm.__exit__(None, None, None)

        with ExitStack() as p5:
            wp_pool = p5.enter_context(tc.tile_pool(name="wp", bufs=1))
            wp_sb = []
            for kt in range(k_proj):
                t = wp_pool.tile([HD, D], mm_dt, tag=f"wp{kt}", name=f"wp{kt}")
                nc.sync.dma_start(t[:], wprojT[kt * HD:(kt + 1) * HD, :])
                wp_sb.append(t)
            out_pool = p5.enter_context(tc.tile_pool(name="outsb", bufs=3))
            for (c0, c1) in big_chunks:
                cs = c1 - c0
                for j in range(D // 128):
                    ob = out_pool.tile([128, BC], f32, tag="ob", name="ob")
                    for (h0, h1) in halves(c0, c1):
                        ps = ps_att.tile([128, 512], f32, tag=f"st{j % 2}",
                                         name="pj")
                        for kt in range(k_proj):
                            nc.tensor.matmul(
                                ps[:, :h1 - h0],
                                r_(wp_sb[kt][:, j * 128:(j + 1) * 128]),
                                r_(attn_sb[kt][0:HD, h0:h1]),
                                start=(kt == 0), stop=(kt == k_proj - 1))
                        if j % 2 == 0:
                            nc.vector.tensor_copy(ob[:, h0 - c0:h1 - c0],
                                                  ps[:, :h1 - h0])
                        else:
                            nc.scalar.activation(ob[:, h0 - c0:h1 - c0],
                                                 ps[:, :h1 - h0], AF.Identity)
                    nc.sync.dma_start(outT[j * 128:(j + 1) * 128, c0:c1],
                                      ob[:, :cs])

    nc.compile()
    return nc


def _pack_w(Wqkv, bqkv, heads, n_h):
    """Mode C packed qkv weights (q rows pre-scaled)."""
    pos, n_mtiles = _pack_layout(n_h)
    dims_pad = n_mtiles * 128
    W = np.zeros((dims_pad, D), np.float32)
    b = np.zeros((dims_pad,), np.float32)
    sec_off = {"q": 0, "k": D, "v": 2 * D}
    for i, h in enumerate(heads):
        for sec in ("q", "k", "v"):
            for half in (0, 1):
                t, r = pos[(sec, i, half)]
                src = sec_off[sec] + h * HD + half * BLK
                w = Wqkv[src:src + BLK, :]
                bb = bqkv[src:src + BLK]
                if sec == "q":
                    w = w * SCALE
                    bb = bb * SCALE
                W[t * 128 + r:t * 128 + r + BLK] = w
                b[t * 128 + r:t * 128 + r + BLK] = bb
    w_tiled = _tile_rows(np.ascontiguousarray(W.T))
    bias2d = np.ascontiguousarray(b.reshape(n_mtiles, 128).T)
    return w_tiled, bias2d


def _tile_rows(x):
    """[R, C] with R = nk*128 -> [128, nk*C] k-major tiling."""
    R, C = x.shape
    nk = R // 128
    return np.ascontiguousarray(
        x.reshape(nk, 128, C).transpose(1, 0, 2).reshape(128, nk * C))


def _pack_cos_sin(cos, sin):
    """Mode C cosP/sin2P [128, S]."""
    S = cos.shape[0]
    cosP = np.zeros((128, S), np.float32)
    sinP = np.zeros((128, S), np.float32)
    cosP[0:BLK] = cos.T[0:BLK]
    cosP[64:64 + BLK] = cos.T[BLK:HD]
    sinP[0:BLK] = -sin.T[0:BLK]
    sinP[64:64 + BLK] = sin.T[BLK:HD]
    return cosP, sinP


def kernel(hidden_states, cos, sin, Wqkv, bqkv, Wproj, bproj, cu_seqlens):
    sys.path.insert(0, "/opt/trn_rl_repo")
    from concourse import bass_utils

    hidden_states = np.asarray(hidden_states, np.float32)
    cos = np.asarray(cos, np.float32)
    sin = np.asarray(sin, np.float32)
    Wqkv = np.asarray(Wqkv, np.float32)
    bqkv = np.asarray(bqkv, np.float32)
    Wproj = np.asarray(Wproj, np.float32)
    bproj = np.asarray(bproj, np.float32)

    S, D_ = hidden_states.shape
    assert D_ == D
    segs = _segments(cu_seqlens, S)
    uniform = (S == 4096) and segs == [(i * S // 4, (i + 1) * S // 4)
                                       for i in range(4)]

    if uniform:
        out = _kernel_mode_a(hidden_states, cos, sin, Wqkv, bqkv, Wproj,
                             bproj, S)
    else:
        hiddenT = np.ascontiguousarray(hidden_states.T)
        cosP, sin2P = _pack_cos_sin(cos, sin)
        n_h, S_core = H // N_CORES, S
        key = ("C", S, tuple(np.asarray(cu_seqlens).tolist()))
        if key not in _CACHE:
            _CACHE[key] = _build_program(n_h, S_core, segs,
                                         resident_hidden=False)
        nc = _CACHE[key]
        n_tt = sum(-(-(e - a) // 128) for a, e in segs)
        vinit = np.zeros((128, n_tt, 17), np.float32)
        vinit[:, :, 16] = 1.0
        vinit = np.ascontiguousarray(vinit.reshape(128, n_tt * 17))
        hid_tiled = _tile_rows(hiddenT)
        in_maps = []
        for c in range(N_CORES):
            heads = list(range(c * n_h, (c + 1) * n_h))
            wt, b2 = _pack_w(Wqkv, bqkv, heads, n_h)
            in_maps.append({
                "hiddenT": hid_tiled,
                "wqkvT": wt,
                "bias2d": b2,
                "cosP": cosP,
                "sin2P": sin2P,
                "wprojT": _pack_wproj(Wproj, heads).astype(np.float32),
                "vinit": vinit,
            })
        res = bass_utils.run_bass_kernel_spmd(nc, in_maps,
                                              core_ids=list(range(N_CORES)))
        out = np.zeros((D, S), np.float32)
        for c in range(N_CORES):
            out += res.results[c]["outT"]

    return np.ascontiguousarray(out.T) + bproj[None, :]


# revision 41
# speedup vs baseline: 1.4287x; 1.0062x over previous
"""Trainium2 Bass kernel for Ernie4.5-VL vision attention (ragged segments).

Contract: kernel(**inputs) takes the FULL unsharded inputs (keyed as in
setup_inputs()) and returns the FULL [S, D] float32 output.

Mode A (uniform 4x1024 segments — the graded shape): 8 cores = 2 head
groups x 4 segments; per core 8 heads x 1024 tokens, everything in bf16
on the PE array (psum f32):

  qkvT = Wpack @ hidden.T     15 dense 128-row tiles (v 80-row blocks at
                              tile h rows 0:80, q/k packed tile-major)
  rope: dense [0:80] layout; the rotate-half operand is built with 2-4
        small SBUF DMAs per (q|k, head); rot = a*cos + b*sin on DVE/Pool
  per head: v transposes (PE) -> scoresT (PE) -> exp (ACT, 1024 wide)
        -> PV accumulate with ones column for the denominator ->
        reciprocal+broadcast+mul normalize
  attn heads DMA-repacked into 5 dense 128-row tiles; proj = 5 k-tiles
  Host does O(S*D) glue: packing, summing the 2 per-token partial
  projections, bias adds.

Engine budget per core (cost model): PE ~142us of matmul rows, ACT
~82us (exp + qkv bias copies), DVE ~40us, Pool ~30us, DMA ~19MB.
Emission interleaves attention per head into the qkv j-loop so every
engine streams; all DMAs avoid the ACT queue (exp lives there).

Mode C fallback (any other cu_seqlens): 8-way head parallel fp32r path
(unchanged from the earlier version of this kernel).
"""

import os
import sys

import numpy as np

H = 16
HD = 80
BLK = 40  # rotate_half half-width
SCALE = HD ** -0.5
N_CORES = 8
D = 1280
NK = D // 128  # contraction tiles for the qkv matmul
ATTN_STRIDE = 96  # head row pitch in the packed attention output (mode C)
MM_DT_NAME = os.environ.get("KERNEL_MM_DT", "float32r")  # mode C only
KERNEL_DEBUG = bool(int(os.environ.get("KERNEL_DEBUG", "0")))

# ---- mode A constants ----
NJ = 15          # dense qkv M tiles (1920 rows)
NTT = 8          # 128-row key tiles per 1024 segment
VW = 97          # vaug slot: 80 v dims + 16 pad + ones col at 96
SA_CORE = 1024   # tokens per core


def _segments(cu_seqlens, S):
    """Intervals matching reference's searchsorted(cu[1:], i, 'right')."""
    b = np.clip(np.sort(np.asarray(cu_seqlens, dtype=np.int64)[1:5]), 0, S)
    bounds = [0] + list(b) + [S]
    segs = []
    for a, e in zip(bounds[:-1], bounds[1:]):
        if e > a:
            segs.append((int(a), int(e)))
    return segs


# ---------------------------------------------------------------------------
# mode A: dense bf16 program
# ---------------------------------------------------------------------------

def _layout_a2():
    """Per-head-contiguous packing: head h owns global rows [240h, 240h+240).
    v sits at rows 0:80 of tile ceil(240h/128) (PE transpose needs a
    32-aligned non-crossing 80-row read); q then k fill the remaining
    window rows in ascending order (read via DMA, placement free).

    Returns (v_tile[h], q_rows[h], k_rows[h], ready_j[h]) where
    q_rows/k_rows are the 80 global rows of each section in dim order.
    """
    v_tile, q_rows, k_rows, ready = [], [], [], []
    for h in range(8):
        w0, w1 = 240 * h, 240 * (h + 1)
        th = -(-w0 // 128)
        vg0 = 128 * th
        qk = [g for g in range(w0, w1) if not (vg0 <= g < vg0 + HD)]
        v_tile.append(th)
        q_rows.append(qk[0:HD])
        k_rows.append(qk[HD:2 * HD])
        ready.append(max(th, qk[-1] // 128))
    return v_tile, q_rows, k_rows, ready


def _row_pieces(rows):
    """Split a list of global rows into (tile, row, len, rel_off) runs that
    are consecutive and stay within one 128-row tile."""
    out = []
    i = 0
    while i < len(rows):
        g = rows[i]
        n = 1
        while (i + n < len(rows) and rows[i + n] == g + n
               and (g + n) // 128 == g // 128):
            n += 1
        out.append((g // 128, g % 128, n, i))
        i += n
    return out


def _build_program_a2(bias_zero=True):
    """Mode A program: n_h=8 heads, S=1024 tokens per core, one segment."""
    import concourse.mybir as mybir
    import concourse.tile as tile
    from concourse import bacc
    from concourse.masks import make_identity
    from contextlib import ExitStack

    f32 = mybir.dt.float32
    bf16 = mybir.dt.bfloat16
    AF = mybir.ActivationFunctionType
    n_h, S = 8, SA_CORE

    nc = bacc.Bacc("TRN2", target_bir_lowering=False, debug=False,
                   enable_asserts=False, num_devices=N_CORES)

    hiddenT = nc.dram_tensor("hiddenT", [128, NK * S], bf16,
                             kind="ExternalInput").ap()
    wqkvT = nc.dram_tensor("wqkvT", [128, NJ * NK * 128], bf16,
                           kind="ExternalInput").ap()
    bias2d = nc.dram_tensor("bias2d", [128, NJ], f32,
                            kind="ExternalInput").ap()
    cosP = nc.dram_tensor("cosP", [HD, S], bf16, kind="ExternalInput").ap()
    sinP = nc.dram_tensor("sinP", [HD, S], bf16, kind="ExternalInput").ap()
    wprojT = nc.dram_tensor("wprojT", [n_h * HD, D], bf16,
                            kind="ExternalInput").ap()
    vinit = nc.dram_tensor("vinit", [128, 8 * NTT * VW], bf16,
                           kind="ExternalInput").ap()
    outT = nc.dram_tensor("outT", [D, S], bf16, kind="ExternalOutput").ap()
    if KERNEL_DEBUG:
        dbg_qkv = nc.dram_tensor("dbg_qkv", [128, NJ * S], f32,
                                 kind="ExternalOutput").ap()
        dbg_rot = nc.dram_tensor("dbg_rot", [128, 2 * n_h * S], f32,
                                 kind="ExternalOutput").ap()
        dbg_attn = nc.dram_tensor("dbg_attn", [128, n_h * S], f32,
                                  kind="ExternalOutput").ap()

    v_tile, q_rows, k_rows, ready = _layout_a2()
    rope_at = {j: [] for j in range(NJ)}   # (h, sec) at section readiness
    vaug_at = {j: [] for j in range(NJ)}   # h at v-tile readiness
    vaug_deferred = []
    for h in range(n_h):
        for sec, rows in (("q", q_rows[h]), ("k", k_rows[h])):
            rope_at[max(g // 128 for g in rows)].append((h, sec))
        if v_tile[h] >= NJ - 1:
            vaug_deferred.append(h)
        else:
            vaug_at[v_tile[h]].append(h)

    with tile.TileContext(nc) as tc, ExitStack() as ctx:
        persist = ctx.enter_context(tc.tile_pool(name="persist", bufs=1))
        ident = persist.tile([128, 128], bf16, tag="ident", name="ident")
        make_identity(nc, ident[:])
        bias_sb = persist.tile([128, NJ], f32, tag="bias", name="bias")
        cos_sb = persist.tile([128, S], bf16, tag="cos", name="cos")
        sin_sb = persist.tile([128, S], bf16, tag="sin", name="sin")

        psum = ctx.enter_context(tc.tile_pool(name="psum", bufs=1,
                                              space="PSUM"))
        qkv_pool = ctx.enter_context(tc.tile_pool(name="qkv", bufs=1))
        qkv_sb = [qkv_pool.tile([128, S], bf16, tag=f"qkvT{j}",
                                name=f"qkvT{j}") for j in range(NJ)]
        rot_pool = ctx.enter_context(tc.tile_pool(name="rot", bufs=1))
        rot_sb = {}
        for h in range(n_h):
            for sec in ("q", "k"):
                rot_sb[(sec, h)] = rot_pool.tile(
                    [128, S], bf16, tag=f"rot_{sec}{h}", name=f"rot_{sec}{h}")
        vaug_pool = ctx.enter_context(tc.tile_pool(name="vaug", bufs=1))
        vaug_all = vaug_pool.tile([128, n_h * NTT * VW], bf16, tag="vaug",
                                  name="vaug")
        vaug_sb = [vaug_all[:, h * NTT * VW:(h + 1) * NTT * VW]
                   for h in range(n_h)]
        attn_pool = ctx.enter_context(tc.tile_pool(name="attn", bufs=1))
        attn_sb = [attn_pool.tile([128, S], bf16, tag=f"attn{h}",
                                  name=f"attn{h}") for h in range(n_h)]
        stg_pool = ctx.enter_context(tc.tile_pool(name="stg", bufs=2))
        pt_pool = ctx.enter_context(tc.tile_pool(name="pt", bufs=4))
        nrm_pool = ctx.enter_context(tc.tile_pool(name="nrm", bufs=2))
        pk_pool = ctx.enter_context(tc.tile_pool(name="pk", bufs=1))
        pk_sb = [pk_pool.tile([128, S], bf16, tag=f"pk{t}", name=f"pk{t}")
                 for t in range(5)]
        wp_pool = ctx.enter_context(tc.tile_pool(name="wp", bufs=1))
        wp_sb = [wp_pool.tile([128, D], bf16, tag=f"wp{t}", name=f"wp{t}")
                 for t in range(5)]


        def emit_rope(h, sec, late=False):
            # sa (x in dim order) staged from qkv tiles on the SP queue;
            # sb = rotate_half(sa) built from sa with exactly two Pool
            # (SWDGE) DMAs. DMA count is precious: each dma_start holds its
            # queue for wait+transfer+sem (~1.1us fixed).
            rows = q_rows[h] if sec == "q" else k_rows[h]
            sa = stg_pool.tile([128, S], bf16, tag="sa", name=f"sa_{sec}{h}")
            sb = stg_pool.tile([128, S], bf16, tag="sb", name=f"sb_{sec}{h}")
            for t, r, ln, off in _row_pieces(rows):
                nc.sync.dma_start(sa[off:off + ln, :],
                                  qkv_sb[t][r:r + ln, :])
            if late:
                # endgame: stage sb straight from qkv tiles (parallel with
                # sa) so the last ropes don't pay the serial sa->sb hop
                for t, r, ln, off in _row_pieces(rows[BLK:]):
                    nc.sync.dma_start(sb[off:off + ln, :],
                                      qkv_sb[t][r:r + ln, :])
                for t, r, ln, off in _row_pieces(rows[:BLK]):
                    nc.sync.dma_start(sb[BLK + off:BLK + off + ln, :],
                                      qkv_sb[t][r:r + ln, :])
            else:
                nc.gpsimd.dma_start(sb[0:BLK, :], sa[BLK:HD, :])
                nc.gpsimd.dma_start(sb[BLK:HD, :], sa[0:BLK, :])
            rot = rot_sb[(sec, h)]
            nc.vector.tensor_mul(rot[0:HD, :], sa[0:HD, :], cos_sb[0:HD, :])
            nc.vector.tensor_mul(sb[0:HD, :], sb[0:HD, :], sin_sb[0:HD, :])
            nc.vector.tensor_add(rot[0:HD, :], rot[0:HD, :], sb[0:HD, :])

        def emit_vaug(h, on_act=False):
            # v transposes -> vaug; copies on DVE mid-run (ACT paces the
            # attention blocks), on ACT for the deferred last head (DVE is
            # serialized behind the previous norm chain there)
            for g in range(2):  # groups of 4 key tiles
                tp = psum.tile([128, 1024], bf16, tag=f"tp{g % 2}",
                               name="tp")
                for x in range(4):
                    ti = 4 * g + x
                    nc.tensor.transpose(
                        tp[:, x * HD:(x + 1) * HD],
                        qkv_sb[v_tile[h]][0:HD, ti * 128:(ti + 1) * 128],
                        ident[0:HD, 0:HD])
                dst = vaug_sb[h].rearrange("p (t c) -> p t c", c=VW)
                src_ap = tp[:, 0:4 * HD].rearrange("p (t c) -> p t c",
                                                   c=HD)[:, 0:4, :]
                if on_act:
                    nc.scalar.activation(dst[:, 4 * g:4 * g + 4, 0:HD],
                                         src_ap, AF.Identity)
                else:
                    nc.vector.tensor_copy(dst[:, 4 * g:4 * g + 4, 0:HD],
                                          src_ap)

        def emit_attn(h, weave=None, po_tag="po", shift_eng=None):
            # ---- scores -> exp -> PV over 16 (key tile, half) units ----
            # PV lags one unit so exp latency is hidden; the woven qkv
            # j-tile's matmuls fill the remaining PE slack.
            qT = rot_sb[("q", h)]
            kT = rot_sb[("k", h)]
            po = psum.tile([128, S], f32, tag=po_tag, name="po")
            if weave is not None:
                wv_ps = psum.tile([128, S], f32, tag="wv", name="wv")
                wv_mm = [(c, k) for c in (0, 512) for k in range(NK)]
                wv_done = 0
            units = [(ti, c) for ti in range(NTT) for c in (0, 512)]
            pend = {}

            def emit_pv(u):
                pt, ti, c = pend.pop(u)
                nc.tensor.matmul(
                    po[0:VW, c:c + 512],
                    vaug_sb[h][:, ti * VW:(ti + 1) * VW],
                    pt[:, :],
                    start=(ti == 0), stop=(ti == NTT - 1))

            for u, (ti, c) in enumerate(units):
                st = psum.tile([128, 512], f32, tag=f"st{u % 2}", name="st")
                nc.tensor.matmul(st[:, :], kT[0:HD, ti * 128:(ti + 1) * 128],
                                 qT[0:HD, c:c + 512], start=True, stop=True)
                pt = pt_pool.tile([128, 512], bf16, tag="pt", name="pt")
                nc.scalar.activation(pt[:, :], st[:, :], AF.Exp)
                pend[u] = (pt, ti, c)
                if weave is not None:
                    take = 2 if u % 4 == 0 else 1
                    for cc, k in wv_mm[wv_done:wv_done + take]:
                        nc.tensor.matmul(
                            wv_ps[:, cc:cc + 512],
                            wj_tiles[weave][:, k * 128:(k + 1) * 128],
                            hid_of(k)[:, cc:cc + 512],
                            start=(k == 0), stop=(k == NK - 1))
                    wv_done += take
                if u >= 2:
                    emit_pv(u - 2)
            if weave is not None and wv_done < len(wv_mm):
                for cc, k in wv_mm[wv_done:]:
                    nc.tensor.matmul(
                        wv_ps[:, cc:cc + 512],
                        wj_tiles[weave][:, k * 128:(k + 1) * 128],
                        hid_of(k)[:, cc:cc + 512],
                        start=(k == 0), stop=(k == NK - 1))
            emit_pv(len(units) - 2)
            emit_pv(len(units) - 1)
            if weave is not None:
                if bias_zero and weave <= 4:
                    nc.scalar.activation(qkv_sb[weave][:, :], wv_ps[:, :],
                                         AF.Identity)
                elif bias_zero:
                    nc.vector.tensor_copy(qkv_sb[weave][:, :], wv_ps[:, :])
                elif True:
                    nc.scalar.activation(qkv_sb[weave][:, :], wv_ps[:, :],
                                         AF.Identity,
                                         bias=bias_sb[:, weave:weave + 1])

            # ---- normalize: recip straight off PSUM row 96, row shift
            # on the ACT queue (lands right after this head's exps) ----
            rc = nrm_pool.tile([128, S], f32, tag="rc", name="rc")
            nc.vector.reciprocal(rc[96:97, :], po[96:97, :])
            (shift_eng or nc.gpsimd).dma_start(rc[0:1, :], rc[96:97, :])
            bc = nrm_pool.tile([128, S], f32, tag="bc", name="bc")
            nc.gpsimd.partition_broadcast(bc[0:HD, :], rc[0:1, :])
            # the last head lands directly in the packed proj tile (rows
            # 0:80 of pk4) so proj is gated only by this normalize, not by
            # an extra repack DMA
            dst = pk_sb[4][0:HD, :] if h == n_h - 1 else attn_sb[h][0:HD, :]
            nc.vector.tensor_mul(dst, po[0:HD, :], bc[0:HD, :])

        def emit_repack(h):
            # dense proj k-tiles; emitted post-loop so these DMAs never
            # head-of-line-block the weight stream on the sync queue.
            # proj-row map: heads 0-5 at 80h; h6 split 480:512 + 592:640;
            # h7 occupies 512:592 (written in place by its normalize).
            if h == n_h - 1:
                return
            if h == 6:
                spans = [(480, 0, 32), (592, 32, 48)]
            else:
                spans = [(HD * h, 0, HD)]
            for g0, off, ln in spans:
                while ln > 0:
                    t, r = g0 // 128, g0 % 128
                    n = min(128 - r, ln)
                    nc.sync.dma_start(pk_sb[t][r:r + n, :],
                                      attn_sb[h][off:off + n, :])
                    g0 += n
                    off += n
                    ln -= n

        # ------------ phase 1: qkv + interleaved per-head attention ----
        with ExitStack() as p1:
            hid_pool = p1.enter_context(tc.tile_pool(name="hid", bufs=1))
            w_pool = p1.enter_context(tc.tile_pool(name="wstream", bufs=3))
            # hid loaded in k-pairs (halves the DMA count)
            hid_pairs = [hid_pool.tile([128, 2 * S], bf16, tag=f"hid{p}",
                                       name=f"hid{p}") for p in range(NK // 2)]
            hid_of = lambda k: hid_pairs[k // 2][:, (k % 2) * S:
                                                 (k % 2) * S + S]
            wj_tiles = {}

            def load_wj(j):
                # j-pair granularity: one DMA covers tiles j, j+1
                if j in wj_tiles:
                    return
                j0 = j - j % 2
                wp2 = w_pool.tile([128, 2 * NK * 128], bf16, tag="wj",
                                  name=f"wj{j0}")
                nn = min(2, NJ - j0)
                nc.sync.dma_start(
                    wp2[:, 0:nn * NK * 128],
                    wqkvT[:, j0 * NK * 128:(j0 + nn) * NK * 128])
                for jj in range(j0, j0 + nn):
                    wj_tiles[jj] = wp2[:, (jj - j0) * NK * 128:
                                       (jj - j0 + 1) * NK * 128]

            # DMA order on the sync queue: hid k0, wj0, hid k1, wj1 — the
            # fused j0/j1 pair consumes hid tiles as they arrive.
            wp01 = w_pool.tile([128, 2 * NK * 128], bf16, tag="wj",
                               name="wj0")
            nc.scalar.dma_start(wp01[:, 0:NK * 128], wqkvT[:, 0:NK * 128])
            nc.sync.dma_start(hid_pairs[0][:, 0:S], hiddenT[:, 0:S])
            nc.scalar.dma_start(wp01[:, NK * 128:], wqkvT[:, NK * 128:
                                                          2 * NK * 128])
            nc.sync.dma_start(hid_pairs[0][:, S:2 * S], hiddenT[:, S:2 * S])
            wj_tiles[0] = wp01[:, 0:NK * 128]
            wj_tiles[1] = wp01[:, NK * 128:2 * NK * 128]
            for p in range(1, NK // 2):
                nc.sync.dma_start(hid_pairs[p][:],
                                  hiddenT[:, 2 * p * S:(2 * p + 2) * S])
            load_wj(2)
            nc.sync.dma_start(bias_sb[:], bias2d[:])
            nc.sync.dma_start(cos_sb[0:HD, :], cosP[:])
            nc.sync.dma_start(sin_sb[0:HD, :], sinP[:])
            nc.sync.dma_start(vaug_all[:], vinit[:])

            def after_j(j):
                if j + 1 < NJ and (j + 1) not in wj_tiles:
                    load_wj(j + 1)
                if j == 10:
                    for t in range(5):
                        nc.sync.dma_start(wp_sb[t][:],
                                          wprojT[t * 128:(t + 1) * 128, :])
                for h in vaug_at[j]:
                    emit_vaug(h, on_act=(h < 2))
                for h, sec in rope_at[j]:
                    emit_rope(h, sec, late=(j >= 13))

            # fused j0/j1: k-outer so the PE starts as soon as hid0 lands
            ps0 = psum.tile([128, S], f32, tag="wv", name="ps0")
            ps1 = psum.tile([128, S], f32, tag="po", name="ps1")
            for k in range(NK):
                for ps, j in ((ps0, 0), (ps1, 1)):
                    for c in (0, 512):
                        nc.tensor.matmul(
                            ps[:, c:c + 512],
                            wj_tiles[j][:, k * 128:(k + 1) * 128],
                            hid_of(k)[:, c:c + 512],
                            start=(k == 0), stop=(k == NK - 1))
            for ps, j in ((ps0, 0), (ps1, 1)):
                if bias_zero:
                    nc.scalar.activation(qkv_sb[j][:, :], ps[:, :],
                                         AF.Identity)
                elif True:
                    nc.scalar.activation(qkv_sb[j][:, :], ps[:, :],
                                         AF.Identity,
                                         bias=bias_sb[:, j:j + 1])
                after_j(j)

            # attention blocks due at `ready+2` consume (weave) that j's
            # matmuls; j14 stays plain so act14 lands before the last ropes
            emitted = set()
            j = 2
            while j < NJ:
                due_now = [h for h in range(n_h) if h not in emitted
                           and min(ready[h] + 2, NJ - 1) <= j]
                if due_now and j < NJ - 1:
                    h = due_now[0]
                    if j not in wj_tiles:
                        load_wj(j)
                    emit_attn(h, weave=j)
                    emitted.add(h)
                    after_j(j)
                    j += 1
                    continue
                if j not in wj_tiles:
                    load_wj(j)
                sts = [psum.tile([128, 512], f32, tag=f"st{i}", name="qs")
                       for i in range(2)]
                for ci, c in enumerate((0, 512)):
                    for k in range(NK):
                        nc.tensor.matmul(
                            sts[ci][:, :],
                            wj_tiles[j][:, k * 128:(k + 1) * 128],
                            hid_of(k)[:, c:c + 512],
                            start=(k == 0), stop=(k == NK - 1))
                    if bias_zero and j <= 4:
                        nc.scalar.activation(qkv_sb[j][:, c:c + 512],
                                             sts[ci][:, :], AF.Identity)
                    elif bias_zero:
                        nc.vector.tensor_copy(qkv_sb[j][:, c:c + 512],
                                              sts[ci][:, :])
                    else:
                        nc.scalar.activation(qkv_sb[j][:, c:c + 512],
                                             sts[ci][:, :], AF.Identity,
                                             bias=bias_sb[:, j:j + 1])
                after_j(j)
                j += 1
        # post-loop attention/repack sits OUTSIDE the p1 pools' scope: the
        # pool-exit engine drains must not wait on the attention tail
        unemitted = [h for h in range(n_h) if h not in emitted]
        for i, h in enumerate(unemitted):
            if h in vaug_deferred:
                emit_vaug(h)   # after the previous attn block so the
                # in-order PE isn't blocked waiting on the last act
            emit_attn(h, po_tag="wv" if i % 2 == 0 else "po")
        for h in range(n_h):
            emit_repack(h)

        if KERNEL_DEBUG:
            for j in range(NJ):
                nc.sync.dma_start(dbg_qkv[:, j * S:(j + 1) * S],
                                    qkv_sb[j][:])
            i_ = 0
            for h in range(n_h):
                for sec in ("q", "k"):
                    nc.sync.dma_start(dbg_rot[:, i_ * S:(i_ + 1) * S],
                                        rot_sb[(sec, h)][:])
                    i_ += 1
            for h in range(n_h):
                nc.sync.dma_start(dbg_attn[:, h * S:(h + 1) * S],
                                    attn_sb[h][:])

        # ------------ phase 2: projection ---------------------------
        with ExitStack() as p5:
            out_pool = p5.enter_context(tc.tile_pool(name="outsb", bufs=1))
            wv_sl = psum.tile([128, S], f32, tag="wv", name="pjwv")
            po_sl = psum.tile([128, S], f32, tag="po", name="pjpo")
            slots = []

            def slot(i):
                i = i % 8
                if i < 4:
                    return psum.tile([128, 512], f32,
                                     tag=["st0", "st1", "tp0", "tp1"][i],
                                     name="pj")
                if i < 6:
                    return wv_sl[:, (i - 4) * 512:(i - 3) * 512]
                return po_sl[:, (i - 6) * 512:(i - 5) * 512]

            chains = [(j, c) for j in range(D // 128) for c in (0, 512)]
            slot_of = {}

            def open_partA(u):
                j, c = chains[u]
                ps = slot(u)
                slot_of[u] = ps
                for kt in range(4):
                    nc.tensor.matmul(ps[:, 0:512],
                                     wp_sb[kt][:, j * 128:(j + 1) * 128],
                                     pk_sb[kt][:, c:c + 512],
                                     start=(kt == 0), stop=False)

            for u in range(8):
                open_partA(u)
            for u, (j, c) in enumerate(chains):
                if c == 0:
                    ob = out_pool.tile([128, S], bf16, tag=f"ob{j % 4}",
                                       name="ob")
                ps = slot_of.pop(u)
                nc.tensor.matmul(ps[:, 0:512],
                                 wp_sb[4][:, j * 128:(j + 1) * 128],
                                 pk_sb[4][:, c:c + 512],
                                 start=False, stop=True)
                if u % 2 == 0:
                    nc.scalar.activation(ob[:, c:c + 512], ps[:, 0:512],
                                         AF.Identity)
                else:
                    nc.vector.tensor_copy(ob[:, c:c + 512], ps[:, 0:512])
                if u + 8 < len(chains):
                    open_partA(u + 8)
                if c == 512:
                    eng = (nc.sync, nc.gpsimd)[j % 2]
                    eng.dma_start(outT[j * 128:(j + 1) * 128, :], ob[:, :])

    nc.compile()
    return nc


def _pack_w_a2(Wqkv, bqkv, heads):
    """Dense 15-tile per-head-contiguous packing (see _layout_a2)."""
    import ml_dtypes
    v_tile, q_rows, k_rows, _ = _layout_a2()
    perm = np.zeros((NJ * 128,), np.int64)
    scl = np.ones((NJ * 128,), np.float32)
    used = np.zeros((NJ * 128,), bool)
    for i, h in enumerate(heads):
        for d in range(HD):
            g = 128 * v_tile[i] + d
            perm[g] = 2 * D + h * HD + d  # v
            used[g] = True
        for d, g in enumerate(q_rows[i]):
            perm[g] = h * HD + d
            scl[g] = SCALE
            used[g] = True
        for d, g in enumerate(k_rows[i]):
            perm[g] = D + h * HD + d
            used[g] = True
    W = Wqkv[perm] * scl[:, None]
    W[~used] = 0.0
    b = bqkv[perm] * scl
    b[~used] = 0.0
    # wqkvT host layout: [128, j, k, 128]; [p, j, k, m] = W.T[k*128+p, j*128+m]
    WT = np.ascontiguousarray(W.T)  # [1280, 1920]
    wt = WT.reshape(NK, 128, NJ, 128).transpose(1, 2, 0, 3)
    wt = np.ascontiguousarray(wt.reshape(128, NJ * NK * 128))
    bias2d = np.ascontiguousarray(b.reshape(NJ, 128).T)
    return wt.astype(ml_dtypes.bfloat16), bias2d


def _pack_wproj(Wproj, heads):
    """Rows of Wproj.T for this core's head dims, stacked per head."""
    W = np.zeros((len(heads) * HD, Wproj.shape[0]), np.float32)
    for i, h in enumerate(heads):
        W[i * HD:(i + 1) * HD] = Wproj[:, h * HD:(h + 1) * HD].T
    return W


def _pack_wproj_a2(Wproj, heads):
    """Mode A proj rows match the device pk layout: heads 0-5 at 80h,
    h6 split 480:512 (d0:32) + 592:640 (d32:80), h7 at 512:592."""
    W = np.zeros((640, Wproj.shape[0]), np.float32)
    wt = lambda h, d0, d1: Wproj[:, heads[h] * HD + d0:
                                 heads[h] * HD + d1].T
    for i in range(6):
        W[i * HD:(i + 1) * HD] = wt(i, 0, HD)
    W[480:512] = wt(6, 0, 32)
    W[512:592] = wt(7, 0, HD)
    W[592:640] = wt(6, 32, HD)
    return W


_CACHE = {}


def _kernel_mode_a(hidden_states, cos, sin, Wqkv, bqkv, Wproj, bproj, S):
    import ml_dtypes
    from concourse import bass_utils

    n_h, S_core = H // 2, S // 4
    bz = not np.any(bqkv)
    key = ("A2", bz)
    if key not in _CACHE:
        _CACHE[key] = _build_program_a2(bias_zero=bz)
    nc = _CACHE[key]

    bf = ml_dtypes.bfloat16
    hiddenT = np.ascontiguousarray(hidden_states.T)  # [D, S]

    vinit = np.zeros((128, 8 * NTT, VW), np.float32)
    vinit[:, :, 96] = 1.0
    vinit = np.ascontiguousarray(vinit.reshape(128, 8 * NTT * VW)).astype(bf)

    in_maps = []
    meta = []
    for g in range(2):
        heads = list(range(g * n_h, (g + 1) * n_h))
        wt, b2 = _pack_w_a2(Wqkv, bqkv, heads)
        wprojT = _pack_wproj_a2(Wproj, heads).astype(bf)
        for s in range(4):
            sl = slice(s * S_core, (s + 1) * S_core)
            hseg = hiddenT[:, sl]  # [1280, 1024]
            hid_t = np.ascontiguousarray(
                hseg.reshape(NK, 128, S_core).transpose(1, 0, 2)
                .reshape(128, NK * S_core)).astype(bf)
            cosP = np.ascontiguousarray(cos[sl].T).astype(bf)
            sinP = np.concatenate(
                [-sin[sl].T[0:BLK], sin[sl].T[BLK:HD]], axis=0)
            sinP = np.ascontiguousarray(sinP).astype(bf)
            in_maps.append({
                "hiddenT": hid_t,
                "wqkvT": wt,
                "bias2d": b2,
                "cosP": cosP,
                "sinP": sinP,
                "wprojT": wprojT,
                "vinit": vinit,
            })
            meta.append((g, s))
    res = bass_utils.run_bass_kernel_spmd(nc, in_maps,
                                          core_ids=list(range(N_CORES)))
    out = np.zeros((D, S), np.float32)
    for c, (g, s) in enumerate(meta):
        out[:, s * S_core:(s + 1) * S_core] += \
            res.results[c]["outT"].astype(np.float32)
    return out


# ---------------------------------------------------------------------------
# mode C (non-uniform segments): fp32r 8-way head-parallel fallback
# ---------------------------------------------------------------------------

def _pack_layout(n_h):
    """Pack per-core qkv dims as 40-row blocks, 3 per 128-row tile (8 pad).

    Each tile holds one v-block at row 0 (PE transpose operands must start
    at a 32-aligned partition) and two q/k blocks at rows 40 and 80.
    Returns pos[(sec, h, half)] = (tile, row) and the number of tiles.
    """
    ntiles = 2 * n_h
    pos = {}
    for h in range(n_h):
        for half in (0, 1):
            pos[("v", h, half)] = (2 * h + half, 0)
    qk = [("q", h, half) for h in range(n_h) for half in (0, 1)]
    qk += [("k", h, half) for h in range(n_h) for half in (0, 1)]
    for j, blk in enumerate(qk):
        pos[blk] = (j // 2, BLK + BLK * (j % 2))
    return pos, ntiles


def _build_program(n_h, S_core, segs_local, resident_hidden):
    """Mode C SPMD program (fp32r)."""
    import concourse.mybir as mybir
    import concourse.tile as tile
    from concourse import bacc
    from concourse.masks import make_identity
    from contextlib import ExitStack

    f32 = mybir.dt.float32
    mm_dt = getattr(mybir.dt, MM_DT_NAME)
    AF = mybir.ActivationFunctionType

    k_proj = n_h
    pos, n_mtiles = _pack_layout(n_h)
    dims_pad = n_mtiles * 128
    VWc = 97

    t_tiles = []
    for si, (a, e) in enumerate(segs_local):
        t = a
        while t < e:
            t_tiles.append((si, t, min(t + 128, e)))
            t += 128
    n_tt = len(t_tiles)

    nc = bacc.Bacc("TRN2", target_bir_lowering=False, debug=False,
                   enable_asserts=False, num_devices=N_CORES)

    hiddenT = nc.dram_tensor("hiddenT", [128, NK * S_core], mm_dt,
                             kind="ExternalInput").ap()
    wqkvT = nc.dram_tensor("wqkvT", [128, NK * dims_pad], mm_dt,
                           kind="ExternalInput").ap()
    bias2d = nc.dram_tensor("bias2d", [128, n_mtiles], f32,
                            kind="ExternalInput").ap()
    cosP = nc.dram_tensor("cosP", [128, S_core], mm_dt,
                          kind="ExternalInput").ap()
    sin2P = nc.dram_tensor("sin2P", [128, S_core], mm_dt,
                           kind="ExternalInput").ap()
    wprojT = nc.dram_tensor("wprojT", [n_h * HD, D], mm_dt,
                            kind="ExternalInput").ap()
    vinit = nc.dram_tensor("vinit", [128, n_tt * (VWc - HD)], mm_dt,
                           kind="ExternalInput").ap()
    outT = nc.dram_tensor("outT", [D, S_core], f32, kind="ExternalOutput").ap()

    def r_(ap):
        return ap.bitcast(mm_dt)

    BC = 1024
    big_chunks = [(c, min(c + BC, S_core)) for c in range(0, S_core, BC)]

    def halves(c0, c1):
        out = []
        q = c0
        while q < c1:
            out.append((q, min(q + 512, c1)))
            q = q + 512
        return out

    with tile.TileContext(nc) as tc, ExitStack() as ctx:
        persist = ctx.enter_context(tc.tile_pool(name="persist", bufs=1))
        ident = persist.tile([128, 128], f32, tag="ident", name="ident")
        make_identity(nc, ident[:])
        bias_sb = persist.tile([128, n_mtiles], f32, tag="bias", name="bias")
        nc.sync.dma_start(bias_sb[:], bias2d[:])

        psum_all_cm = tc.tile_pool(name="psum_all", bufs=1, space="PSUM")
        psum_all = psum_all_cm.__enter__()
        qkv_pool = ctx.enter_context(tc.tile_pool(name="big", bufs=1))
        qkv_sb = [qkv_pool.tile([128, S_core], mm_dt, tag=f"qkvT{j}",
                                name=f"qkvT{j}") for j in range(n_mtiles)]
        rot_cm = tc.tile_pool(name="rot", bufs=1)
        rv = rot_cm.__enter__()
        rot_sb = {}
        for h in range(n_h):
            for sec in ("q", "k"):
                rot_sb[(sec, h)] = rv.tile([128, S_core], mm_dt,
                                           tag=f"rot_{sec}{h}",
                                           name=f"rot_{sec}{h}")
        RC = 1024
        rope_cm = tc.tile_pool(name="rope_scr", bufs=2)
        rope_scr = rope_cm.__enter__()

        with ExitStack() as p1:
            hidden3 = hiddenT.rearrange("p (k s) -> p k s", k=NK)
            w3 = wqkvT.rearrange("p (k m) -> p k m", k=NK)
            w_pool = p1.enter_context(tc.tile_pool(name="wres", bufs=1))
            w_sb = [w_pool.tile([128, dims_pad], mm_dt, tag=f"w{k}",
                                name=f"w{k}") for k in range(NK)]
            for k in range(NK):
                nc.sync.dma_start(w_sb[k][:], w3[:, k, :])
            hid_pool = p1.enter_context(tc.tile_pool(name="hidstream",
                                                     bufs=3))
            n4 = n_mtiles // 4
            for (h0, h1) in halves(0, S_core):
                hw = h1 - h0
                for q4 in range(n4):
                    ps01 = psum_all.tile([128, BC], f32, tag="t0",
                                         name="ps01")
                    ps23 = psum_all.tile([128, BC], f32, tag="t1",
                                         name="ps23")
                    pj_of = lambda j: (ps01 if j % 4 < 2 else ps23,
                                       (j % 2) * 512)
                    for k in range(NK):
                        ht = hid_pool.tile([128, 512], mm_dt, tag="hidc",
                                           name="hidc")
                        nc.sync.dma_start(ht[:, :hw], hidden3[:, k, h0:h1])
                        for j in range(q4 * 4, q4 * 4 + 4):
                            psj, co = pj_of(j)
                            nc.tensor.matmul(
                                psj[:, co:co + hw],
                                r_(w_sb[k][:, j * 128:(j + 1) * 128]),
                                r_(ht[:, :hw]),
                                start=(k == 0), stop=(k == NK - 1))
                    for j in range(q4 * 4, q4 * 4 + 4):
                        psj, co = pj_of(j)
                        nc.scalar.activation(qkv_sb[j][:, h0:h1],
                                             psj[:, co:co + hw], AF.Identity,
                                             bias=bias_sb[:, j:j + 1])

        psum_all_cm.__exit__(None, None, None)
        ps_att = ctx.enter_context(tc.tile_pool(name="ps_att", bufs=1,
                                                space="PSUM"))

        stg = {}
        for nm in ("sa0", "sa1", "sb0", "sb1"):
            stg[nm] = rope_scr.tile([128, RC], mm_dt, tag=nm, name=nm, bufs=1)
        pair_i = 0
        for ci, f0 in enumerate(range(0, S_core, RC)):
            f1 = min(f0 + RC, S_core)
            fs = f1 - f0
            cos_sb = rope_scr.tile([128, RC], mm_dt, tag="cos", name="cos",
                                   bufs=1)
            sin_sb = rope_scr.tile([128, RC], mm_dt, tag="sin", name="sin",
                                   bufs=1)
            nc.scalar.dma_start(cos_sb[:, :fs], cosP[:, f0:f1])
            nc.scalar.dma_start(sin_sb[:, :fs], sin2P[:, f0:f1])
            if ci == 0:
                for nm in stg:
                    nc.scalar.dma_start(stg[nm][BLK:64, :], cos_sb[BLK:64, :])
            for h in range(n_h):
                for sec in ("q", "k"):
                    lo_t, lo_r = pos[(sec, h, 0)]
                    hi_t, hi_r = pos[(sec, h, 1)]
                    x = qkv_sb[lo_t]
                    dst = rot_sb[(sec, h)]
                    stga = stg[f"sa{pair_i % 2}"]
                    stgb = stg[f"sb{pair_i % 2}"]
                    nc.scalar.dma_start(stga[0:BLK, :fs],
                                        x[lo_r:lo_r + BLK, f0:f1])
                    nc.scalar.dma_start(stga[64:64 + BLK, :fs],
                                        x[hi_r:hi_r + BLK, f0:f1])
                    nc.scalar.dma_start(stgb[0:BLK, :fs],
                                        x[hi_r:hi_r + BLK, f0:f1])
                    nc.scalar.dma_start(stgb[64:64 + BLK, :fs],
                                        x[lo_r:lo_r + BLK, f0:f1])
                    nc.vector.tensor_mul(dst[0:104, f0:f1], stga[0:104, :fs],
                                         cos_sb[0:104, :fs])
                    eng = nc.gpsimd if pair_i % 2 == 0 else nc.vector
                    eng.tensor_mul(stgb[0:104, :fs], stgb[0:104, :fs],
                                   sin_sb[0:104, :fs])
                    nc.vector.tensor_add(dst[0:104, f0:f1], dst[0:104, f0:f1],
                                         stgb[0:104, :fs])
                    pair_i += 1
        rope_cm.__exit__(None, None, None)

        vaug_cm = tc.tile_pool(name="vaug", bufs=1)
        vaug_pool = vaug_cm.__enter__()
        vaug_sb = [vaug_pool.tile([128, n_tt * VWc], mm_dt, tag=f"vaug{h}",
                                  name=f"vaug{h}") for h in range(n_h)]
        vinit3 = vinit.rearrange("p (t c) -> p t c", c=VWc - HD)
        for h in range(n_h):
            nc.sync.dma_start(
                vaug_sb[h].rearrange("p (t c) -> p t c", c=VWc)[:, :, HD:VWc],
                vinit3[:, :, :])
        GRP = 4

        def emit_vaug(h):
            gi = 0
            while gi < n_tt:
                hi_g = min(gi + GRP, n_tt)
                if all(t_tiles[g][2] - t_tiles[g][1] == 128
                       for g in range(gi, hi_g)):
                    grp = list(range(gi, hi_g))
                else:
                    grp = [gi]
                ng = len(grp)
                tp = ps_att.tile([128, GRP * HD], f32, tag="tp", name="tp")
                for x, g in enumerate(grp):
                    si, t0, t1 = t_tiles[g]
                    sz = t1 - t0
                    for half in (0, 1):
                        vt, vr = pos[("v", h, half)]
                        nc.tensor.transpose(
                            tp[:sz, x * HD + half * BLK:
                               x * HD + (half + 1) * BLK],
                            qkv_sb[vt][0:BLK, t0:t1].bitcast(f32),
                            ident[:BLK, :BLK])
                sz0 = t_tiles[grp[0]][2] - t_tiles[grp[0]][1]
                dst = vaug_sb[h].rearrange("p (t c) -> p t c", c=VWc)
                src_ap = tp.rearrange("p (t c) -> p t c", c=HD)
                nc.vector.tensor_copy(dst[:sz0, grp[0]:grp[0] + ng, 0:HD],
                                      src_ap[:sz0, 0:ng, :])
                gi += ng

        attn_sb = [qkv_pool.tile([128, S_core], mm_dt, tag=f"qkvT{hh}",
                                 name=f"attnT{hh}") for hh in range(n_h)]

        seg_ttiles = {}
        for ti, (si, t0, t1) in enumerate(t_tiles):
            seg_ttiles.setdefault(si, []).append((ti, t0, t1))

        BA = 512
        with ExitStack() as p4:
            pt_pool = p4.enter_context(tc.tile_pool(name="pt", bufs=3))
            nrm_pool = p4.enter_context(tc.tile_pool(name="nrm", bufs=2))
            unit_box = [0]

            def emit_attention(h, si, a, e):
                qT = rot_sb[("q", h)]
                kT = rot_sb[("k", h)]
                q = a
                while q < e:
                    q0, q1 = q, min(q + BA, e)
                    qs = q1 - q0
                    po = ps_att.tile([128, BA], f32,
                                     tag=f"po{unit_box[0] % 2}", name="pv")
                    tts = seg_ttiles[si]
                    for idx, (ti, t0, t1) in enumerate(tts):
                        sz = t1 - t0
                        ps = ps_att.tile([128, BA], f32, tag=f"st{idx % 2}",
                                         name="st")
                        nc.tensor.matmul(ps[:sz, :qs], r_(kT[0:104, t0:t1]),
                                         r_(qT[0:104, q0:q1]),
                                         start=True, stop=True)
                        pt = pt_pool.tile([128, BA], mm_dt, tag="pt",
                                          name="pt")
                        nc.scalar.activation(pt[:sz, :qs], ps[:sz, :qs],
                                             AF.Exp)
                        nc.tensor.matmul(
                            po[:VWc, :qs],
                            r_(vaug_sb[h][:sz, ti * VWc:(ti + 1) * VWc]),
                            r_(pt[:sz, :qs]),
                            start=(idx == 0), stop=(idx == len(tts) - 1))
                    rc = nrm_pool.tile([128, BA], f32, tag="rc", name="rc")
                    nc.vector.tensor_copy(rc[96:97, :qs], po[96:97, :qs])
                    nc.sync.dma_start(rc[0:1, :qs], rc[96:97, :qs])
                    nc.vector.reciprocal(rc[0:1, :qs], rc[0:1, :qs])
                    bc = nrm_pool.tile([128, BA], mm_dt, tag="bc", name="bc")
                    nc.gpsimd.partition_broadcast(
                        bc[0:HD, :qs], rc[0:1, :qs].bitcast(mm_dt))
                    nc.vector.tensor_mul(attn_sb[h][0:HD, q0:q1],
                                         po[0:HD, :qs], bc[0:HD, :qs])
                    unit_box[0] += 1
                    q = q1

            for h in range(n_h):
                emit_vaug(h)
            for si, (a, e) in enumerate(segs_local):
                for h in range(n_h):
                    emit_attention(h, si, a, e)

        vaug_cm.__exit__(None, None, None)
        rot_cm.__exit__(None, None, None)

        with ExitStack() as p5:
            wp_pool = p5.enter_context(tc.tile_pool(name="wp", bufs=1))
            wp_sb = []
            for kt in range(k_proj):
                t = wp_pool.tile([HD, D], mm_dt, tag=f"wp{kt}", name=f"wp{kt}")
                nc.sync.dma_start(t[:], wprojT[kt * HD:(kt + 1) * HD, :])
                wp_sb.append(t)
            out_pool = p5.enter_context(tc.tile_pool(name="outsb", bufs=3))
            for (c0, c1) in big_chunks:
                cs = c1 - c0
                for j in range(D // 128):
                    ob = out_pool.tile([128, BC], f32, tag="ob", name="ob")
                    for (h0, h1) in halves(c0, c1):
                        ps = ps_att.tile([128, 512], f32, tag=f"st{j % 2}",
                                         name="pj")
                        for kt in range(k_proj):
                            nc.tensor.matmul(
                                ps[:, :h1 - h0],
                                r_(wp_sb[kt][:, j * 128:(j + 1) * 128]),
                                r_(attn_sb[kt][0:HD, h0:h1]),
                                start=(kt == 0), stop=(kt == k_proj - 1))
                        if j % 2 == 0:
                            nc.vector.tensor_copy(ob[:, h0 - c0:h1 - c0],
                                                  ps[:, :h1 - h0])
                        else:
                            nc.scalar.activation(ob[:, h0 - c0:h1 - c0],
                                                 ps[:, :h1 - h0], AF.Identity)
                    nc.sync.dma_start(outT[j * 128:(j + 1) * 128, c0:c1],
                                      ob[:, :cs])

    nc.compile()
    return nc


def _pack_w(Wqkv, bqkv, heads, n_h):
    """Mode C packed qkv weights (q rows pre-scaled)."""
    pos, n_mtiles = _pack_layout(n_h)
    dims_pad = n_mtiles * 128
    W = np.zeros((dims_pad, D), np.float32)
    b = np.zeros((dims_pad,), np.float32)
    sec_off = {"q": 0, "k": D, "v": 2 * D}
    for i, h in enumerate(heads):
        for sec in ("q", "k", "v"):
            for half in (0, 1):
                t, r = pos[(sec, i, half)]
                src = sec_off[sec] + h * HD + half * BLK
                w = Wqkv[src:src + BLK, :]
                bb = bqkv[src:src + BLK]
                if sec == "q":
                    w = w * SCALE
                    bb = bb * SCALE
                W[t * 128 + r:t * 128 + r + BLK] = w
                b[t * 128 + r:t * 128 + r + BLK] = bb
    w_tiled = _tile_rows(np.ascontiguousarray(W.T))
    bias2d = np.ascontiguousarray(b.reshape(n_mtiles, 128).T)
    return w_tiled, bias2d


def _tile_rows(x):
    """[R, C] with R = nk*128 -> [128, nk*C] k-major tiling."""
    R, C = x.shape
    nk = R // 128
    return np.ascontiguousarray(
        x.reshape(nk, 128, C).transpose(1, 0, 2).reshape(128, nk * C))


def _pack_cos_sin(cos, sin):
    """Mode C cosP/sin2P [128, S]."""
    S = cos.shape[0]
    cosP = np.zeros((128, S), np.float32)
    sinP = np.zeros((128, S), np.float32)
    cosP[0:BLK] = cos.T[0:BLK]
    cosP[64:64 + BLK] = cos.T[BLK:HD]
    sinP[0:BLK] = -sin.T[0:BLK]
    sinP[64:64 + BLK] = sin.T[BLK:HD]
    return cosP, sinP


def kernel(hidden_states, cos, sin, Wqkv, bqkv, Wproj, bproj, cu_seqlens):
    sys.path.insert(0, "/opt/trn_rl_repo")
    from concourse import bass_utils

    hidden_states = np.asarray(hidden_states, np.float32)
    cos = np.asarray(cos, np.float32)
    sin = np.asarray(sin, np.float32)
    Wqkv = np.asarray(Wqkv, np.float32)
    bqkv = np.asarray(bqkv, np.float32)
    Wproj = np.asarray(Wproj, np.float32)
    bproj = np.asarray(bproj, np.float32)

    S, D_ = hidden_states.shape
    assert D_ == D
    segs = _segments(cu_seqlens, S)
    uniform = (S == 4096) and segs == [(i * S // 4, (i + 1) * S // 4)
                                       for i in range(4)]

    if uniform:
        out = _kernel_mode_a(hidden_states, cos, sin, Wqkv, bqkv, Wproj,
                             bproj, S)
    else:
        hiddenT = np.ascontiguousarray(hidden_states.T)
        cosP, sin2P = _pack_cos_sin(cos, sin)
        n_h, S_core = H // N_CORES, S
        key = ("C", S, tuple(np.asarray(cu_seqlens).tolist()))
        if key not in _CACHE:
            _CACHE[key] = _build_program(n_h, S_core, segs,
                                         resident_hidden=False)
        nc = _CACHE[key]
        n_tt = sum(-(-(e - a) // 128) for a, e in segs)
        vinit = np.zeros((128, n_tt, 17), np.float32)
        vinit[:, :, 16] = 1.0
        vinit = np.ascontiguousarray(vinit.reshape(128, n_tt * 17))
        hid_tiled = _tile_rows(hiddenT)
        in_maps = []
        for c in range(N_CORES):
            heads = list(range(c * n_h, (c + 1) * n_h))
            wt, b2 = _pack_w(Wqkv, bqkv, heads, n_h)
            in_maps.append({
                "hiddenT": hid_tiled,
                "wqkvT": wt,
                "bias2d": b2,
                "cosP": cosP,
                "sin2P": sin2P,
                "wprojT": _pack_wproj(Wproj, heads).astype(np.float32),
                "vinit": vinit,
            })
        res = bass_utils.run_bass_kernel_spmd(nc, in_maps,
                                              core_ids=list(range(N_CORES)))
        out = np.zeros((D, S), np.float32)
        for c in range(N_CORES):
            out += res.results[c]["outT"]

    return np.ascontiguousarray(out.T) + bproj[None, :]
